# revision 1
# baseline (speedup 1.0000x reference)
"""Deformable-Transformer encoder on 8 trn2 NeuronCores — v3.

Like v2 (3 launches, bf16, host gather) but restructured for op-count:
  - LayerNorm stats: per-tile bn_stats into a batched stats tile, then
    aggregation/rsqrt for 5-6 tiles in one strided op each.
  - C (FFN1) and A (projections) matmuls grouped over 512-token spans:
    4x fewer, 4x wider matmuls and PSUM drains.
  - Biases added on the PE via rank-1 ones-row matmuls, so PSUM drains
    are pure copies/relu that cover several chunks at once.
  - GPSIMD carries the SBUF-only elementwise ops (residual scaling,
    LN2 apply, q construction).
  - g2/be2 of each layer folded into the next-layer projection weights
    (host); device outputs the pre-affine LN2 result.
"""
import os
import sys
import types
import contextlib
import ctypes
import numpy as np

sys.path.insert(0, "/opt/trn_rl_repo")


def _install_ntff_hook():
    try:
        import antenv

        if hasattr(antenv, "axon_hooks"):
            return
        so_path = "/opt/axon/libaxon_pjrt.so"
        lib = ctypes.CDLL(so_path)
        if not hasattr(lib, "axon_start_nrt_profile"):
            hook = None
        else:
            lib.axon_start_nrt_profile.argtypes = [
                ctypes.POINTER(ctypes.c_int64), ctypes.c_size_t]
            lib.axon_start_nrt_profile.restype = ctypes.c_int64
            lib.axon_stop_nrt_profile.argtypes = [ctypes.c_char_p]
            lib.axon_stop_nrt_profile.restype = ctypes.c_int64

            @contextlib.contextmanager
            def hook(output_dir, device_ids):
                import jax
                jax.devices()
                if device_ids:
                    ids = (ctypes.c_int64 * len(device_ids))(*device_ids)
                    rc = lib.axon_start_nrt_profile(ids, len(device_ids))
                else:
                    rc = lib.axon_start_nrt_profile(None, 0)
                if rc != 0:
                    raise RuntimeError(f"start_nrt_profile rc={rc}")
                try:
                    yield
                finally:
                    lib.axon_stop_nrt_profile(str(output_dir).encode())

        m = types.ModuleType("antenv.axon_hooks")
        m.get_axon_ntff_profile_hook = lambda: hook
        m.set_axon_ntff_profile_hook = lambda h: None
        sys.modules["antenv.axon_hooks"] = m
        antenv.axon_hooks = m
    except Exception:
        pass


_install_ntff_hook()

import ml_dtypes  # noqa: E402
from concourse import bacc, tile, mybir, bass  # noqa: E402
from concourse.bass_utils import run_bass_kernel_spmd  # noqa: E402
from contextlib import ExitStack  # noqa: E402

F32 = mybir.dt.float32
BF16 = mybir.dt.bfloat16
NPBF = ml_dtypes.bfloat16
AF = mybir.ActivationFunctionType
ALU = mybir.AluOpType

SHAPES = ((64, 64), (32, 32), (16, 16), (8, 8))
LEVEL_STARTS = [0, 4096, 5120, 5376, 5440]
N_LEVELS, N_HEADS, N_POINTS = 4, 8, 4
D_MODEL, HEAD_DIM, D_FFN = 256, 32, 1024
LEN_IN, BATCH, NCORE = 5440, 2, 8
TPC = LEN_IN * BATCH // NCORE  # 1360 tokens per core
NT = 11                        # 128-token tiles per core
GROUPS = [(0, 512, range(0, 4)), (512, 512, range(4, 8)),
          (1024, 336, range(8, 11))]
HALVES = [(0, 6), (6, 11)]

HW_EXEC_NS = []
_PROGS = {}


def _nc():
    return bacc.Bacc("TRN2", target_bir_lowering=False, debug=False,
                     num_devices=NCORE)


def _tsz(ti):
    return min(128, TPC - ti * 128)


def _ccn(d):
    return d.rearrange("(c p) n -> p c n", p=128)


def _tchunks(step):
    out = []
    t0 = 0
    while t0 < TPC:
        out.append((t0, min(step, TPC - t0)))
        t0 += step
    return out


def _build_A():
    """Layer-0 projections, channel-major world (same as v2)."""
    nc = _nc()
    xT_d = nc.dram_tensor("xT", [D_MODEL, TPC], BF16, kind="ExternalInput").ap()
    qT_d = nc.dram_tensor("qT", [D_MODEL, TPC], BF16, kind="ExternalInput").ap()
    wv_d = nc.dram_tensor("Wv", [D_MODEL, 256], BF16, kind="ExternalInput").ap()
    woa_d = nc.dram_tensor("Woa", [D_MODEL, 384], BF16,
                           kind="ExternalInput").ap()
    prm_d = nc.dram_tensor("prm", [128, 5], F32, kind="ExternalInput").ap()
    valT_d = nc.dram_tensor("valT", [256, TPC], BF16,
                            kind="ExternalOutput").ap()
    oaT_d = nc.dram_tensor("offawT", [384, TPC], BF16,
                           kind="ExternalOutput").ap()

    with tile.TileContext(nc) as tc, ExitStack() as ctx:
        sb = ctx.enter_context(tc.tile_pool(name="sb", bufs=1))
        ps = ctx.enter_context(tc.tile_pool(name="ps", bufs=1, space="PSUM"))
        ob = ctx.enter_context(tc.tile_pool(name="ob", bufs=1))

        wv = sb.tile([128, 2, 256], BF16, tag="wv")
        nc.sync.dma_start(wv[:], _ccn(wv_d))
        woa = sb.tile([128, 2, 384], BF16, tag="woa")
        nc.sync.dma_start(woa[:], _ccn(woa_d))
        prm = sb.tile([128, 5], F32, tag="prm")
        nc.sync.dma_start(prm[:], prm_d[:])
        xT = sb.tile([128, 2, TPC], BF16, tag="xT")
        qT = sb.tile([128, 2, TPC], BF16, tag="qT")
        for t0, tsz in _tchunks(512):
            nc.sync.dma_start(xT[:, :, t0:t0 + tsz],
                              _ccn(xT_d)[:, :, t0:t0 + tsz])
            nc.scalar.dma_start(qT[:, :, t0:t0 + tsz],
                                _ccn(qT_d)[:, :, t0:t0 + tsz])

        for t0, tsz in _tchunks(512):
            vsb = ob.tile([128, 2, 512], BF16, tag="vsb", bufs=2)
            osb = ob.tile([128, 3, 512], BF16, tag="osb", bufs=2)
            for m in range(5):  # 0-1: val (from x), 2-4: offaw (from q)
                src = xT if m < 2 else qT
                w = wv if m < 2 else woa
                mm = m if m < 2 else m - 2
                p = ps.tile([128, 512], F32, tag="p", bufs=3)
                for k in range(2):
                    nc.tensor.matmul(p[:, :tsz],
                                     w[:, k, mm * 128:mm * 128 + 128],
                                     src[:, k, t0:t0 + tsz],
                                     start=(k == 0), stop=(k == 1))
                dst = (vsb if m < 2 else osb)[:, mm, :tsz]
                if m % 2 == 0:
                    nc.scalar.activation(dst, p[:, :tsz], AF.Identity,
                                         bias=prm[:, m:m + 1])
                else:
                    nc.vector.tensor_scalar(dst, p[:, :tsz], prm[:, m:m + 1],
                                            None, ALU.add)
            nc.scalar.dma_start(_ccn(valT_d)[:, :, t0:t0 + tsz],
                                vsb[:, :, :tsz])
            nc.sync.dma_start(
                oaT_d.rearrange("(c p) n -> p c n", p=128)[:, :, t0:t0 + tsz],
                osb[:, :, :tsz])
    nc.compile()
    return nc


def _build_BCDA(with_A, final_out):
    """Fused out-proj + LN1 + FFN + LN2 (+ next-layer projections).

    in: attnT[256,TPC] bf16, xb[TPC,256] bf16 (= x + bo, host-folded),
        Wo[256,256] bf16, Wl1g[256,1024] bf16 (= diag(g1) Wl1),
        Wl2[1024,256] bf16, ident[128,128] bf16,
        rows[1,1664] bf16 (bl1+be1@Wl1 | next-layer bva, g2/be2-folded),
        prm[128,3] f32 (col0 4*eps, col1-2 g2 chunks),
        rep[128,512] bf16 (g1 | be1+bl2, replicated)
      with_A: posbT[256,TPC] bf16 (= (pos+be2)^T), Wv/Woa g2-folded bf16
      final_out: rep2[128,512] f32 (g2 | be2 replicated)
    out with_A: x1n[TPC,256] bf16 (pre-affine LN2 out; host applies
        g2,be2), valT[256,TPC] bf16, offawT[384,TPC] bf16
    out final_out: out[TPC,256] f32
    """
    nc = _nc()
    aT_d = nc.dram_tensor("attnT", [D_MODEL, TPC], BF16,
                          kind="ExternalInput").ap()
    xbT_d = nc.dram_tensor("xbT", [D_MODEL, TPC], BF16,
                           kind="ExternalInput").ap()
    wo_d = nc.dram_tensor("Wo", [256, 256], BF16, kind="ExternalInput").ap()
    wl1_d = nc.dram_tensor("Wl1g", [256, 1024], BF16,
                           kind="ExternalInput").ap()
    wl2_d = nc.dram_tensor("Wl2", [1024, 256], BF16,
                           kind="ExternalInput").ap()
    id_d = nc.dram_tensor("ident", [128, 128], BF16,
                          kind="ExternalInput").ap()
    rows_d = nc.dram_tensor("rows", [1, 1920], BF16,
                            kind="ExternalInput").ap()
    prm_d = nc.dram_tensor("prm", [128, 18], F32, kind="ExternalInput").ap()
    dg1_d = nc.dram_tensor("dg1", [256, 256], BF16, kind="ExternalInput").ap()
    if with_A:
        posT_d = nc.dram_tensor("posT", [D_MODEL, TPC], BF16,
                                kind="ExternalInput").ap()
        wv_d = nc.dram_tensor("Wv", [D_MODEL, 256], BF16,
                              kind="ExternalInput").ap()
        woa_d = nc.dram_tensor("Woa", [D_MODEL, 384], BF16,
                               kind="ExternalInput").ap()
        x1n_d = nc.dram_tensor("x1n", [128, NT * 256], BF16,
                               kind="ExternalOutput").ap()
        valT_d = nc.dram_tensor("valT", [256, TPC], BF16,
                                kind="ExternalOutput").ap()
        oaT_d = nc.dram_tensor("offawT", [384, TPC], BF16,
                               kind="ExternalOutput").ap()
    if final_out:
        out_d = nc.dram_tensor("out", [128, NT * 256], BF16,
                               kind="ExternalOutput").ap()

    with tile.TileContext(nc) as tc, ExitStack() as ctx:
        sb = ctx.enter_context(tc.tile_pool(name="sb", bufs=1))
        ps = ctx.enter_context(tc.tile_pool(name="ps", bufs=1, space="PSUM"))
        ob = ctx.enter_context(tc.tile_pool(name="ob", bufs=1))

        wo = sb.tile([128, 2, 256], BF16, tag="wo")
        nc.sync.dma_start(wo[:], _ccn(wo_d))
        idn = sb.tile([128, 128], BF16, tag="idn")
        nc.sync.dma_start(idn[:], id_d[:])
        aT = sb.tile([128, 2, TPC], BF16, tag="aT")
        for t0, tsz in _tchunks(688):
            nc.sync.dma_start(aT[:, :, t0:t0 + tsz],
                              _ccn(aT_d)[:, :, t0:t0 + tsz])
        prm = sb.tile([128, 18], F32, tag="prm")
        nc.sync.dma_start(prm[:], prm_d[:])
        rows = sb.tile([1, 1920], BF16, tag="rows")
        nc.sync.dma_start(rows[:], rows_d[:])
        dg1 = sb.tile([128, 2, 256], BF16, tag="dg1")
        nc.sync.dma_start(dg1[:], _ccn(dg1_d))
        xbT = sb.tile([128, 2, TPC], BF16, tag="xbT")
        for t0, tsz in _tchunks(688):
            nc.sync.dma_start(xbT[:, :, t0:t0 + tsz],
                              _ccn(xbT_d)[:, :, t0:t0 + tsz])
        wl1 = sb.tile([128, 2, 1024], BF16, tag="wl1")
        nc.scalar.dma_start(wl1[:], _ccn(wl1_d))
        wl2 = sb.tile([128, 8, 256], BF16, tag="wl2")
        nc.scalar.dma_start(wl2[:], _ccn(wl2_d))
        ones = sb.tile([1, 512], BF16, tag="ones")
        nc.gpsimd.memset(ones[:], 1.0)
        if with_A:
            posT = sb.tile([128, 2, TPC], BF16, tag="posT")
            for t0, tsz in _tchunks(688):
                nc.sync.dma_start(posT[:, :, t0:t0 + tsz],
                                  _ccn(posT_d)[:, :, t0:t0 + tsz])
            wv = sb.tile([128, 2, 256], BF16, tag="wv")
            nc.sync.dma_start(wv[:], _ccn(wv_d))
            woa = sb.tile([128, 2, 384], BF16, tag="woa")
            nc.sync.dma_start(woa[:], _ccn(woa_d))
            valTs = sb.tile([128, 2, TPC], BF16, tag="valTs")
            oaTs = sb.tile([128, 3, TPC], BF16, tag="oaTs")
            q1Ts = sb.tile([128, 2, TPC], BF16, tag="q1Ts")

        # persistent intermediates
        r1a = sb.tile([128, NT, 256], F32, tag="r1a")
        r2a = sb.tile([128, NT, 256], F32, tag="r2a")
        xnTa = sb.tile([128, 2, TPC], BF16, tag="xnTa")
        xn2Ta = sb.tile([128, 2, TPC], BF16, tag="xn2Ta")
        hta = sb.tile([128, 8, TPC], BF16, tag="hta")
        xout = sb.tile([128, NT, 256], BF16, tag="xout")
        xna = sb.tile([128, NT, 256], BF16, tag="xna")
        bst1 = sb.tile([128, NT, 6], F32, tag="bst1")
        bst2 = sb.tile([128, NT, 6], F32, tag="bst2")
        st1 = [sb.tile([128, NT, 1], F32, tag=f"st1_{i}", name=f"st1_{i}")
               for i in range(2)]
        st2 = [sb.tile([128, NT, 1], F32, tag=f"st2_{i}", name=f"st2_{i}")
               for i in range(2)]

        def batch_stats(bst, dst, h0, h1):
            """bst[:, h0:h1, :] -> dst[0]=rstd, dst[1]=-mean*rstd."""
            n = h1 - h0
            msum = ob.tile([128, 6, 1], F32, tag="msum", bufs=2)
            nc.vector.tensor_tensor(msum[:, :n, :], bst[:, h0:h1, 1:2],
                                    bst[:, h0:h1, 4:5], op=ALU.add)
            mdif = ob.tile([128, 6, 1], F32, tag="mdif", bufs=2)
            nc.vector.tensor_tensor(mdif[:, :n, :], bst[:, h0:h1, 1:2],
                                    bst[:, h0:h1, 4:5], op=ALU.subtract)
            cvs = ob.tile([128, 6, 1], F32, tag="cvs", bufs=2)
            nc.vector.tensor_tensor(cvs[:, :n, :], bst[:, h0:h1, 2:3],
                                    bst[:, h0:h1, 5:6], op=ALU.add)
            mdsq = ob.tile([128, 6, 1], F32, tag="mdsq", bufs=2)
            nc.vector.tensor_tensor(mdsq[:, :n, :], mdif[:, :n, :],
                                    mdif[:, :n, :], op=ALU.mult)
            v4 = ob.tile([128, 6, 1], F32, tag="v4", bufs=2)
            nc.vector.scalar_tensor_tensor(v4[:, :n, :], cvs[:, :n, :],
                                           1.0 / 64.0, mdsq[:, :n, :],
                                           op0=ALU.mult, op1=ALU.add)
            sd = ob.tile([128, 6, 1], F32, tag="sd", bufs=2)
            nc.scalar.activation(sd[:, :n, :], v4[:, :n, :], AF.Sqrt,
                                 bias=prm[:, 0:1])
            rs = ob.tile([128, 6, 1], F32, tag="rs", bufs=2)
            nc.vector.reciprocal(rs[:, :n, :], sd[:, :n, :])
            # rstd = 2*rs ; nmr = -msum*rs
            nc.scalar.mul(dst[0][:, h0:h1, :], rs[:, :n, :], 2.0)
            nc.vector.scalar_tensor_tensor(dst[1][:, h0:h1, :],
                                           msum[:, :n, :], -1.0,
                                           rs[:, :n, :],
                                           op0=ALU.mult, op1=ALU.mult)

        # ---- PE warm-up: dense dummy matmuls so the HAM clock gate
        # reaches 8/8 before the real compute begins ----
        for w in range(16):
            pw = ps.tile([128, 256], F32, tag="pb", bufs=2)
            nc.tensor.matmul(pw[:], wo[:, 0, 0:128], wo[:, 1, :],
                             start=True, stop=True)
            nc.tensor.matmul(pw[:], wo[:, 1, 0:128], wo[:, 0, :],
                             start=False, stop=True, skip_group_check=True)

        # ---- sweep 1: B matmul + residual + LN1 stats ----
        for ti in range(NT):
            sz = _tsz(ti)
            t0 = ti * 128
            pb = ps.tile([128, 256], F32, tag="pb", bufs=2)
            for k in range(2):
                nc.tensor.matmul(pb[:sz], aT[:, k, t0:t0 + sz], wo[:, k, :],
                                 start=(k == 0), stop=False)
            for k in range(2):
                nc.tensor.matmul(pb[:sz, k * 128:k * 128 + 128],
                                 xbT[:, k, t0:t0 + sz], idn[:, :],
                                 start=False, stop=(k == 1),
                                 skip_group_check=True)
            if ti % 2 == 0:
                nc.scalar.copy(r1a[:sz, ti, :], pb[:sz])
            else:
                nc.vector.tensor_copy(r1a[:sz, ti, :], pb[:sz])
            nc.vector.bn_stats(bst1[:sz, ti, :], r1a[:sz, ti, :])
            pwf = ps.tile([128, 2, 128], BF16, tag="ptr", bufs=2)
            for c in range(2):
                nc.tensor.transpose(pwf[:, c, :], idn[:, :], idn[:, :])
            if ti == 3:
                batch_stats(bst1, st1, 0, 4)
            elif ti == 7:
                batch_stats(bst1, st1, 4, 8)
        batch_stats(bst1, st1, 8, 11)

        # ---- sweep 2: LN1 apply, transpose, C, D, LN2 stats ----
        for g0, gsz, tis in GROUPS:
            for ti in tis:
                sz = _tsz(ti)
                t0 = ti * 128
                nc.scalar.activation(xna[:sz, ti, :], r1a[:sz, ti, :],
                                     AF.Identity,
                                     bias=st1[1][:sz, ti, :],
                                     scale=st1[0][:sz, ti, :])
                pt = ps.tile([128, 2, 128], BF16, tag="ptr", bufs=2)
                for c in range(2):
                    nc.tensor.transpose(pt[:, c, :sz],
                                        xna[:sz, ti, c * 128:c * 128 + 128],
                                        idn[:sz, :sz])
                if ti % 2 == 0:
                    nc.scalar.copy(xnTa[:, :, t0:t0 + sz], pt[:, :, :sz])
                else:
                    nc.vector.tensor_copy(xnTa[:, :, t0:t0 + sz],
                                          pt[:, :, :sz])
            # C over the whole group: hT = relu(Wl1g.T @ xnT + bl1row)
            for m in range(8):
                pc = ps.tile([128, 512], F32, tag="pca", bufs=2)
                for k in range(2):
                    nc.tensor.matmul(pc[:, :gsz],
                                     wl1[:, k, m * 128:m * 128 + 128],
                                     xnTa[:, k, g0:g0 + gsz],
                                     start=(k == 0), stop=(k == 1))
                if m % 2 == 0:
                    nc.scalar.activation(hta[:, m, g0:g0 + gsz], pc[:, :gsz],
                                         AF.Relu, bias=prm[:, 5 + m:6 + m])
                else:
                    nc.vector.tensor_scalar(hta[:, m, g0:g0 + gsz],
                                            pc[:, :gsz], prm[:, 5 + m:6 + m],
                                            0.0, ALU.add, ALU.max)
            # D per tile + LN2 stats
            for ti in tis:
                sz = _tsz(ti)
                t0 = ti * 128
                pd = ps.tile([128, 256], F32, tag="pd", bufs=2)
                for k in range(8):
                    nc.tensor.matmul(pd[:sz], hta[:, k, t0:t0 + sz],
                                     wl2[:, k, :],
                                     start=(k == 0), stop=False)
                for k in range(2):
                    nc.tensor.matmul(pd[:sz], xnTa[:, k, t0:t0 + sz],
                                     dg1[:, k, :], start=False, stop=False)
                nc.tensor.matmul(pd[:sz], ones[0:1, :sz],
                                 rows[:, 1664:1920], start=False, stop=True)
                nc.vector.tensor_copy(r2a[:sz, ti, :], pd[:sz])
                nc.vector.bn_stats(bst2[:sz, ti, :], r2a[:sz, ti, :])
            batch_stats(bst2, st2, tis[0], tis[-1] + 1)
        

        # ---- sweep 3: LN2 apply (+ A projections / final output) ----
        for g0, gsz, tis in GROUPS:
            for ti in tis:
                sz = _tsz(ti)
                t0 = ti * 128
                nc.scalar.activation(xout[:sz, ti, :], r2a[:sz, ti, :],
                                     AF.Identity, bias=st2[1][:sz, ti, :],
                                     scale=st2[0][:sz, ti, :])
                if with_A:
                    pt2 = ps.tile([128, 2, 128], BF16, tag="ptr", bufs=2)
                    for c in range(2):
                        nc.tensor.transpose(
                            pt2[:, c, :sz],
                            xout[:sz, ti, c * 128:c * 128 + 128],
                            idn[:sz, :sz])
                    nc.scalar.copy(xn2Ta[:, :, t0:t0 + sz], pt2[:, :, :sz])
                    pwf3 = ps.tile([128, 2, 128], BF16, tag="ptr", bufs=2)
                    for c in range(2):
                        nc.tensor.transpose(pwf3[:, c, :], idn[:, :],
                                            idn[:, :])
            if with_A:
                # q1T = (g2*xn2 + be2 + pos)^T, per channel-chunk
                qp = ob.tile([128, 2, 512], BF16, tag="qp", bufs=2)
                for c in range(2):
                    nc.scalar.activation(qp[:, c, :gsz],
                                         xn2Ta[:, c, g0:g0 + gsz],
                                         AF.Identity,
                                         bias=prm[:, 3 + c:4 + c],
                                         scale=prm[:, 1 + c:2 + c])
                nc.vector.tensor_tensor(q1Ts[:, :, g0:g0 + gsz],
                                        qp[:, :, :gsz],
                                        posT[:, :, g0:g0 + gsz], op=ALU.add)
                # A projections over the group (T-world, grouped)
                for m in range(5):
                    src = xn2Ta if m < 2 else q1Ts
                    w = wv if m < 2 else woa
                    mm = m if m < 2 else m - 2
                    pa = ps.tile([128, 512], F32, tag="pca", bufs=2)
                    for k in range(2):
                        nc.tensor.matmul(pa[:, :gsz],
                                         w[:, k, mm * 128:mm * 128 + 128],
                                         src[:, k, g0:g0 + gsz],
                                         start=(k == 0), stop=(k == 1))
                    dst = (valTs if m < 2 else oaTs)[:, mm, g0:g0 + gsz]
                    if m % 2 == 0:
                        nc.scalar.activation(dst, pa[:, :gsz], AF.Identity,
                                             bias=prm[:, 13 + m:14 + m])
                    else:
                        nc.vector.tensor_scalar(dst, pa[:, :gsz],
                                                prm[:, 13 + m:14 + m],
                                                None, ALU.add)
            # output DMAs per group
            lo, hi = tis[0], tis[-1] + 1
            if with_A:
                nc.scalar.dma_start(_ccn(valT_d)[:, :, g0:g0 + gsz],
                                    valTs[:, :, g0:g0 + gsz])
                nc.sync.dma_start(
                    oaT_d.rearrange("(c p) n -> p c n", p=128)[:, :,
                                                              g0:g0 + gsz],
                    oaTs[:, :, g0:g0 + gsz])
                nc.scalar.dma_start(
                    x1n_d[:, lo * 256:hi * 256], xout[:, lo:hi, :])
            if final_out:
                nc.scalar.dma_start(
                    out_d[:, lo * 256:hi * 256], xout[:, lo:hi, :])
    nc.compile()
    return nc


def _run(prog, in_maps):
    trace = bool(os.environ.get("BASS_TRACE"))
    res = run_bass_kernel_spmd(prog, in_maps, core_ids=list(range(NCORE)),
                               trace=trace)
    if res.exec_time_ns:
        HW_EXEC_NS.append(res.exec_time_ns)
    return res.results


def _bf(a):
    return np.ascontiguousarray(np.asarray(a, np.float32).astype(NPBF))


def _rep2(a, b, dt):
    v = np.concatenate([np.asarray(a, np.float32), np.asarray(b, np.float32)])
    return np.ascontiguousarray(
        np.broadcast_to(v[None, :], (128, 512)).astype(dt))


def _chunked(v, nch):
    v = np.asarray(v, np.float32)
    return np.ascontiguousarray(v.reshape(nch, 128).T.astype(np.float32))


def _ref_points(valid_ratios):
    refs = []
    for lvl, (H, W) in enumerate(SHAPES):
        gy, gx = np.meshgrid(np.arange(H, dtype=np.float32) + 0.5,
                             np.arange(W, dtype=np.float32) + 0.5,
                             indexing="ij")
        ry = gy.reshape(-1)[None] / (valid_ratios[:, lvl, 1][:, None] * H)
        rx = gx.reshape(-1)[None] / (valid_ratios[:, lvl, 0][:, None] * W)
        refs.append(np.stack([rx, ry], -1))
    ref = np.concatenate(refs, 1)
    return ref[:, :, None, :] * valid_ratios[:, None]


def _host_sample(value, off, aw, ref_pts):
    N, Lq = off.shape[:2]
    off = off.reshape(N, Lq, N_HEADS, N_LEVELS, N_POINTS, 2)
    aw = aw.reshape(N, Lq, N_HEADS, N_LEVELS, N_POINTS)
    normalizer = np.array([[w, h] for h, w in SHAPES], np.float32)
    loc = (ref_pts[:, :, None, :, None, :]
           + off / normalizer[None, None, None, :, None, :])
    acc = np.zeros((N, N_HEADS, Lq, HEAD_DIM), np.float32)
    for lvl, (H, W) in enumerate(SHAPES):
        s = LEVEL_STARTS[lvl]
        val = value[:, s:s + H * W].transpose(0, 2, 1, 3)
        x = loc[:, :, :, lvl, :, 0] * W - 0.5
        y = loc[:, :, :, lvl, :, 1] * H - 0.5
        x0 = np.floor(x)
        y0 = np.floor(y)
        wx1 = x - x0
        wy1 = y - y0
        ix0 = x0.astype(np.int64)
        iy0 = y0.astype(np.int64)

        def corner(ix, iy, w):
            valid = (ix >= 0) & (ix < W) & (iy >= 0) & (iy < H)
            idx = np.clip(iy, 0, H - 1) * W + np.clip(ix, 0, W - 1)
            idx = idx.transpose(0, 2, 1, 3).reshape(N, N_HEADS, Lq * N_POINTS)
            g = np.take_along_axis(val, idx[..., None], axis=2)
            g = g.reshape(N, N_HEADS, Lq, N_POINTS, HEAD_DIM)
            w = np.where(valid, w, 0.0).transpose(0, 2, 1, 3)
            return g * w[..., None].astype(np.float32)

        sampled = (corner(ix0, iy0, (1 - wx1) * (1 - wy1))
                   + corner(ix0 + 1, iy0, wx1 * (1 - wy1))
                   + corner(ix0, iy0 + 1, (1 - wx1) * wy1)
                   + corner(ix0 + 1, iy0 + 1, wx1 * wy1))
        acc += (sampled * aw[:, :, :, lvl].transpose(0, 2, 1, 3)[..., None]
                ).sum(3)
    return acc.transpose(0, 2, 1, 3).reshape(N, Lq, D_MODEL)


def _shardT(fullT):
    return [np.ascontiguousarray(fullT[c // 4, :, (c % 4) * TPC:
                                       (c % 4 + 1) * TPC])
            for c in range(NCORE)]


def _unshardT(parts):
    F = parts[0].shape[0]
    out = np.empty((BATCH, LEN_IN, F), np.float32)
    for c in range(NCORE):
        out[c // 4, (c % 4) * TPC:(c % 4 + 1) * TPC] = \
            np.asarray(parts[c], np.float32).T
    return out


def _unshard_pm(parts):  # partition-major parts [128, NT*256]
    out = np.empty((BATCH, LEN_IN, 256), np.float32)
    for c in range(NCORE):
        a = np.asarray(parts[c], np.float32).reshape(128, NT, 256)
        a = a.transpose(1, 0, 2).reshape(NT * 128, 256)[:TPC]
        out[c // 4, (c % 4) * TPC:(c % 4 + 1) * TPC] = a
    return out


def _shard_tok(full):  # [2, 5440, F] -> 8 x [TPC, F]
    return [np.ascontiguousarray(full[c // 4, (c % 4) * TPC:
                                      (c % 4 + 1) * TPC])
            for c in range(NCORE)]


_IDENT = np.eye(128, dtype=NPBF)


def kernel(src, pos, valid_ratios, Wv, bv, Woff, boff, Wa, ba, Wo, bo,
           g1, be1, Wl1, bl1, Wl2, bl2, g2, be2):
    src = np.asarray(src, np.float32)
    pos = np.asarray(pos, np.float32)
    valid_ratios = np.asarray(valid_ratios, np.float32)
    asf = lambda a: np.asarray(a, np.float32)
    HW_EXEC_NS.clear()

    if "A" not in _PROGS:
        _PROGS["A"] = _build_A()
        _PROGS["BCDA"] = _build_BCDA(with_A=True, final_out=False)
        _PROGS["BCD"] = _build_BCDA(with_A=False, final_out=True)

    ref_pts = _ref_points(valid_ratios)

    Woa = [np.concatenate([asf(Woff[l]), asf(Wa[l])], axis=1)
           for l in range(2)]
    bva = [np.concatenate([asf(bv[l]), asf(boff[l]), asf(ba[l])])
           for l in range(2)]
    Wl1g = [asf(g1[l])[:, None] * asf(Wl1[l]) for l in range(2)]
    bl1f = [asf(bl1[l]) + asf(be1[l]) @ asf(Wl1[l]) for l in range(2)]
    # layer-1 value-proj with layer-0 g2/be2 folded in (q-path keeps
    # plain Woa; q is built on device as g2*xn2 + be2 + pos)
    Wv1f = asf(g2[0])[:, None] * asf(Wv[1])
    bva1f = np.concatenate([asf(bv[1]) + asf(be2[0]) @ asf(Wv[1]),
                            bva[1][256:]])
    cr = [asf(be1[l]) + asf(bl2[l]) for l in range(2)]
    rows = [np.concatenate([bl1f[l], bva1f if l == 0 else np.zeros(640),
                            cr[l]])[None, :].astype(NPBF) for l in range(2)]
    prm = [np.concatenate([np.full((128, 1), 4e-5, np.float32),
                           _chunked(g2[l], 2), _chunked(be2[l], 2),
                           _chunked(bl1f[l], 8),
                           _chunked(bva1f if l == 0 else np.zeros(640), 5)],
                          axis=1) for l in range(2)]
    dg1 = [np.ascontiguousarray((np.diag(asf(g1[l]))).astype(NPBF))
           for l in range(2)]

    xT = np.ascontiguousarray(src.transpose(0, 2, 1))
    qT = np.ascontiguousarray((src + pos).transpose(0, 2, 1))
    posT = np.ascontiguousarray(pos.transpose(0, 2, 1))
    xTs = _shardT(xT.astype(NPBF))
    qTs = _shardT(qT.astype(NPBF))
    posTs = _shardT(posT.astype(NPBF))

    # ---- launch 1: layer-0 projections ----
    in_maps = [{
        "xT": xTs[c], "qT": qTs[c],
        "Wv": _bf(Wv[0]), "Woa": _bf(Woa[0]), "prm": _chunked(bva[0], 5),
    } for c in range(NCORE)]
    resA = _run(_PROGS["A"], in_maps)

    def gather_attn(value, offaw, layer, x_full):
        aw = offaw[:, :, 256:].reshape(BATCH, LEN_IN, N_HEADS, 16)
        aw = aw - aw.max(-1, keepdims=True)
        e = np.exp(aw)
        aw = (e / e.sum(-1, keepdims=True)).reshape(BATCH, LEN_IN, 128)
        attn = _host_sample(value.reshape(BATCH, LEN_IN, N_HEADS, HEAD_DIM),
                            offaw[:, :, :256], aw, ref_pts)
        attnT = np.ascontiguousarray(attn.transpose(0, 2, 1))
        xbf = (x_full + asf(bo[layer])[None, None, :]).transpose(0, 2, 1)
        return (_shardT(attnT.astype(NPBF)),
                _shardT(np.ascontiguousarray(xbf).astype(NPBF)))

    # ---- launch 2: layer-0 BCD + layer-1 projections ----
    value = _unshardT([resA[c]["valT"] for c in range(NCORE)])
    offaw = _unshardT([resA[c]["offawT"] for c in range(NCORE)])
    attnTs, xbs = gather_attn(value, offaw, 0, src)
    in_maps = [{
        "attnT": attnTs[c], "xbT": xbs[c],
        "Wo": _bf(Wo[0]), "Wl1g": _bf(Wl1g[0]), "Wl2": _bf(Wl2[0]),
        "ident": _IDENT, "rows": rows[0], "prm": prm[0], "dg1": dg1[0],
        "posT": posTs[c], "Wv": _bf(Wv1f), "Woa": _bf(Woa[1]),
    } for c in range(NCORE)]
    resB = _run(_PROGS["BCDA"], in_maps)

    # x1 = g2*xn2 + be2 (host applies the folded affine)
    xn2 = _unshard_pm([resB[c]["x1n"] for c in range(NCORE)])
    x1 = xn2 * asf(g2[0])[None, None, :] + asf(be2[0])[None, None, :]

    # ---- launch 3: layer-1 BCD -> final ----
    val1 = _unshardT([resB[c]["valT"] for c in range(NCORE)])
    oa1 = _unshardT([resB[c]["offawT"] for c in range(NCORE)])
    attnTs, xbs = gather_attn(val1, oa1, 1, x1)
    in_maps = [{
        "attnT": attnTs[c], "xbT": xbs[c],
        "Wo": _bf(Wo[1]), "Wl1g": _bf(Wl1g[1]), "Wl2": _bf(Wl2[1]),
        "ident": _IDENT, "rows": rows[1], "prm": prm[1], "dg1": dg1[1],
    } for c in range(NCORE)]
    resC = _run(_PROGS["BCD"], in_maps)

    xn2f = _unshard_pm([resC[c]["out"] for c in range(NCORE)])
    return (xn2f * asf(g2[1])[None, None, :]
            + asf(be2[1])[None, None, :]).astype(np.float32)



# revision 8
# speedup vs baseline: 1.0169x; 1.0169x over previous
"""Deformable-Transformer encoder on 8 trn2 NeuronCores — v4.

v3 + latency restructuring driven by NTFF traces:
  - Scratch-tile PE warmup at body start (no DMA dependency) so the HAM
    clock-gate is at 8/8 by the time real matmuls start; removes the
    wo-dependent warmup and the dummy transpose fillers.
  - Input DMAs packed (weights into 1-2 blobs) and spread across
    sync/scalar/vector/gpsimd queues so issue serialization (~0.65us per
    dma instr on one engine) stops gating the first matmul.
  - Drains paired: two 128-token tiles share one 2-slot PSUM tile, so
    PSUM->SBUF evacuation + bn_stats run at half the op count.
  - LN stats via bn_aggr (1 op) instead of a 9-op manual combine.
  - Final launch (BCD) skips LN2 entirely: it streams out the pre-LN2
    residual r2 per tile-pair and the host applies LN2+affine.
"""
import os
import sys
import types
import contextlib
import ctypes
import numpy as np

sys.path.insert(0, "/opt/trn_rl_repo")


def _install_ntff_hook():
    try:
        import antenv

        if hasattr(antenv, "axon_hooks"):
            return
        so_path = "/opt/axon/libaxon_pjrt.so"
        lib = ctypes.CDLL(so_path)
        if not hasattr(lib, "axon_start_nrt_profile"):
            hook = None
        else:
            lib.axon_start_nrt_profile.argtypes = [
                ctypes.POINTER(ctypes.c_int64), ctypes.c_size_t]
            lib.axon_start_nrt_profile.restype = ctypes.c_int64
            lib.axon_stop_nrt_profile.argtypes = [ctypes.c_char_p]
            lib.axon_stop_nrt_profile.restype = ctypes.c_int64

            @contextlib.contextmanager
            def hook(output_dir, device_ids):
                import jax
                jax.devices()
                if device_ids:
                    ids = (ctypes.c_int64 * len(device_ids))(*device_ids)
                    rc = lib.axon_start_nrt_profile(ids, len(device_ids))
                else:
                    rc = lib.axon_start_nrt_profile(None, 0)
                if rc != 0:
                    raise RuntimeError(f"start_nrt_profile rc={rc}")
                try:
                    yield
                finally:
                    lib.axon_stop_nrt_profile(str(output_dir).encode())

        m = types.ModuleType("antenv.axon_hooks")
        m.get_axon_ntff_profile_hook = lambda: hook
        m.set_axon_ntff_profile_hook = lambda h: None
        sys.modules["antenv.axon_hooks"] = m
        antenv.axon_hooks = m
    except Exception:
        pass


_install_ntff_hook()

import ml_dtypes  # noqa: E402
from concourse import bacc, tile, mybir, bass  # noqa: E402
from concourse.bass_utils import run_bass_kernel_spmd  # noqa: E402
from contextlib import ExitStack  # noqa: E402

F32 = mybir.dt.float32
BF16 = mybir.dt.bfloat16
NPBF = ml_dtypes.bfloat16
AF = mybir.ActivationFunctionType
ALU = mybir.AluOpType

SHAPES = ((64, 64), (32, 32), (16, 16), (8, 8))
LEVEL_STARTS = [0, 4096, 5120, 5376, 5440]
N_LEVELS, N_HEADS, N_POINTS = 4, 8, 4
D_MODEL, HEAD_DIM, D_FFN = 256, 32, 1024
LEN_IN, BATCH, NCORE = 5440, 2, 8
TPC = LEN_IN * BATCH // NCORE  # 1360 tokens per core
NT = 11                        # 128-token tiles per core
GROUPS = [(0, 512, range(0, 4)), (512, 512, range(4, 8)),
          (1024, 336, range(8, 11))]
PAIRS = [(0, 2), (2, 2), (4, 2), (6, 2), (8, 2), (10, 1)]
WARMUP_MM = 16

HW_EXEC_NS = []
LAST_RES = []
_PROGS = {}


def _nc():
    return bacc.Bacc("TRN2", target_bir_lowering=False, debug=False,
                     num_devices=NCORE)


def _tsz(ti):
    return min(128, TPC - ti * 128)


def _ccn(d):
    return d.rearrange("(c p) n -> p c n", p=128)


def _tchunks(step):
    out = []
    t0 = 0
    while t0 < TPC:
        out.append((t0, min(step, TPC - t0)))
        t0 += step
    return out


def _warmup(nc, sb, ps, ps_tag, bufs=2):
    """HAM warmup: dense matmuls on a memset scratch tile, no DMA deps."""
    wsc = sb.tile([128, 256], BF16, tag="wsc")
    nc.gpsimd.memset(wsc[:], 0.25)
    for _ in range(WARMUP_MM):
        pw = ps.tile([128, 256], F32, tag=ps_tag, bufs=bufs)
        nc.tensor.matmul(pw[:], wsc[:, 0:128], wsc[:],
                         start=True, stop=True)


def _build_A():
    """Layer-0 projections, channel-major world."""
    nc = _nc()
    xT_d = nc.dram_tensor("xT", [D_MODEL, TPC], BF16, kind="ExternalInput").ap()
    qT_d = nc.dram_tensor("qT", [D_MODEL, TPC], BF16, kind="ExternalInput").ap()
    wpk_d = nc.dram_tensor("wpk", [128, 1280], BF16, kind="ExternalInput").ap()
    prm_d = nc.dram_tensor("prm", [128, 5], F32, kind="ExternalInput").ap()
    valT_d = nc.dram_tensor("valT", [256, TPC], BF16,
                            kind="ExternalOutput").ap()
    oaT_d = nc.dram_tensor("offawT", [384, TPC], BF16,
                           kind="ExternalOutput").ap()

    with tile.TileContext(nc) as tc, ExitStack() as ctx:
        sb = ctx.enter_context(tc.tile_pool(name="sb", bufs=1))
        ps = ctx.enter_context(tc.tile_pool(name="ps", bufs=1, space="PSUM"))
        ob = ctx.enter_context(tc.tile_pool(name="ob", bufs=1))

        _warmup(nc, sb, ps, "p", bufs=3)

        wpk = sb.tile([128, 1280], BF16, tag="wpk")
        nc.sync.dma_start(wpk[:], wpk_d[:])
        wv = wpk[:, 0:512].rearrange("p (c n) -> p c n", c=2)
        woa = wpk[:, 512:1280].rearrange("p (c n) -> p c n", c=2)
        prm = sb.tile([128, 5], F32, tag="prm")
        nc.gpsimd.dma_start(prm[:], prm_d[:])

        for t0, tsz in _tchunks(512):
            xc = ob.tile([128, 2, 512], BF16, tag="xc", bufs=2)
            qc = ob.tile([128, 2, 512], BF16, tag="qc", bufs=2)
            nc.sync.dma_start(xc[:, :, :tsz], _ccn(xT_d)[:, :, t0:t0 + tsz])
            nc.scalar.dma_start(qc[:, :, :tsz], _ccn(qT_d)[:, :, t0:t0 + tsz])
            vsb = ob.tile([128, 2, 512], BF16, tag="vsb", bufs=2)
            osb = ob.tile([128, 3, 512], BF16, tag="osb", bufs=2)
            for m in range(5):  # 0-1: val (from x), 2-4: offaw (from q)
                src = xc if m < 2 else qc
                w = wv if m < 2 else woa
                mm = m if m < 2 else m - 2
                p = ps.tile([128, 512], F32, tag="p", bufs=3)
                for k in range(2):
                    nc.tensor.matmul(p[:, :tsz],
                                     w[:, k, mm * 128:mm * 128 + 128],
                                     src[:, k, :tsz],
                                     start=(k == 0), stop=(k == 1))
                dst = (vsb if m < 2 else osb)[:, mm, :tsz]
                if m % 2 == 0:
                    nc.scalar.activation(dst, p[:, :tsz], AF.Identity,
                                         bias=prm[:, m:m + 1])
                else:
                    nc.vector.tensor_scalar(dst, p[:, :tsz], prm[:, m:m + 1],
                                            None, ALU.add)
            nc.scalar.dma_start(_ccn(valT_d)[:, :, t0:t0 + tsz],
                                vsb[:, :, :tsz])
            nc.sync.dma_start(
                oaT_d.rearrange("(c p) n -> p c n", p=128)[:, :, t0:t0 + tsz],
                osb[:, :, :tsz])
    nc.compile()
    return nc


def _build_BCDA(with_A):
    """Fused out-proj + LN1 + FFN (+ LN2 + next-layer projections).

    with_A=True (layer 0): outputs x1n (pre-affine LN2), valT, offawT.
    with_A=False (layer 1): outputs out = r2 (pre-LN2 residual); the host
    applies LN2 + g2/be2.
    """
    nc = _nc()
    aT_d = nc.dram_tensor("attnT", [D_MODEL, TPC], BF16,
                          kind="ExternalInput").ap()
    xbT_d = nc.dram_tensor("xbT", [D_MODEL, TPC], BF16,
                           kind="ExternalInput").ap()
    # wpk1: wo (2x256) | ident (128)
    wpk1_d = nc.dram_tensor("wpk1", [128, 640], BF16,
                            kind="ExternalInput").ap()
    # wpk2: wl1 (2x1024) | dg1 (2x256) | wl2 (8x256)
    wpk2_d = nc.dram_tensor("wpk2", [128, 4608], BF16,
                            kind="ExternalInput").ap()
    rows_d = nc.dram_tensor("rows", [1, 256], BF16, kind="ExternalInput").ap()
    prm_d = nc.dram_tensor("prm", [128, 18], F32, kind="ExternalInput").ap()
    if with_A:
        posT_d = nc.dram_tensor("posT", [D_MODEL, TPC], BF16,
                                kind="ExternalInput").ap()
        # wpk3: wv (2x256) | woa (2x384)
        wpk3_d = nc.dram_tensor("wpk3", [128, 1280], BF16,
                                kind="ExternalInput").ap()
        x1n_d = nc.dram_tensor("x1n", [128, NT * 256], BF16,
                               kind="ExternalOutput").ap()
        valT_d = nc.dram_tensor("valT", [256, TPC], BF16,
                                kind="ExternalOutput").ap()
        oaT_d = nc.dram_tensor("offawT", [384, TPC], BF16,
                               kind="ExternalOutput").ap()
    else:
        out_d = nc.dram_tensor("out", [128, NT * 256], BF16,
                               kind="ExternalOutput").ap()

    with tile.TileContext(nc) as tc, ExitStack() as ctx:
        sb = ctx.enter_context(tc.tile_pool(name="sb", bufs=1))
        ps = ctx.enter_context(tc.tile_pool(name="ps", bufs=1, space="PSUM"))
        ob = ctx.enter_context(tc.tile_pool(name="ob", bufs=1))

        _warmup(nc, sb, ps, "pb")

        # ---- input DMAs, spread across engine queues ----
        wpk1 = sb.tile([128, 640], BF16, tag="wpk1")
        nc.sync.dma_start(wpk1[:], wpk1_d[:])
        wo = wpk1[:, 0:512].rearrange("p (c n) -> p c n", c=2)
        idn = wpk1[:, 512:640]
        aT = sb.tile([128, 2, TPC], BF16, tag="aT")
        nc.sync.dma_start(aT[:, :, 0:512], _ccn(aT_d)[:, :, 0:512])
        nc.sync.dma_start(aT[:, :, 512:TPC], _ccn(aT_d)[:, :, 512:TPC])
        xbT = sb.tile([128, 2, TPC], BF16, tag="xbT")
        nc.scalar.dma_start(xbT[:, :, 0:512], _ccn(xbT_d)[:, :, 0:512])
        nc.scalar.dma_start(xbT[:, :, 512:TPC], _ccn(xbT_d)[:, :, 512:TPC])
        wpk2 = sb.tile([128, 4608], BF16, tag="wpk2")
        nc.scalar.dma_start(wpk2[:], wpk2_d[:])
        wl1 = wpk2[:, 0:2048].rearrange("p (c n) -> p c n", c=2)
        dg1 = wpk2[:, 2048:2560].rearrange("p (c n) -> p c n", c=2)
        wl2 = wpk2[:, 2560:4608].rearrange("p (c n) -> p c n", c=8)
        prm = sb.tile([128, 18], F32, tag="prm")
        nc.gpsimd.dma_start(prm[:], prm_d[:])
        rows = sb.tile([1, 256], BF16, tag="rows")
        nc.gpsimd.dma_start(rows[:], rows_d[:])
        if with_A:
            posT = sb.tile([128, 2, TPC], BF16, tag="posT")
            nc.gpsimd.dma_start(posT[:], _ccn(posT_d))
            wpk3 = sb.tile([128, 1280], BF16, tag="wpk3")
            nc.gpsimd.dma_start(wpk3[:], wpk3_d[:])
            wv = wpk3[:, 0:512].rearrange("p (c n) -> p c n", c=2)
            woa = wpk3[:, 512:1280].rearrange("p (c n) -> p c n", c=2)
            valTs = sb.tile([128, 2, TPC], BF16, tag="valTs")
            oaTs = sb.tile([128, 3, TPC], BF16, tag="oaTs")
            q1Ts = sb.tile([128, 2, TPC], BF16, tag="q1Ts")
        ones = sb.tile([1, 512], BF16, tag="ones")
        nc.gpsimd.memset(ones[:], 1.0)

        # persistent intermediates
        r1a = sb.tile([128, NT, 256], F32, tag="r1a")
        xnTa = sb.tile([128, 2, TPC], BF16, tag="xnTa")
        hta = sb.tile([128, 8, TPC], BF16, tag="hta")
        xna = sb.tile([128, NT, 256], BF16, tag="xna")
        bst1 = sb.tile([128, NT, 6], F32, tag="bst1")
        mv1 = sb.tile([128, NT, 2], F32, tag="mv1")
        st1r = sb.tile([128, NT, 1], F32, tag="st1r", name="st1r")
        st1n = sb.tile([128, NT, 1], F32, tag="st1n", name="st1n")
        if with_A:
            r2a = sb.tile([128, NT, 256], F32, tag="r2a")
            xn2Ta = sb.tile([128, 2, TPC], BF16, tag="xn2Ta")
            xout = sb.tile([128, NT, 256], BF16, tag="xout")
            bst2 = sb.tile([128, NT, 6], F32, tag="bst2")
            mv2 = sb.tile([128, NT, 2], F32, tag="mv2")
            st2r = sb.tile([128, NT, 1], F32, tag="st2r", name="st2r")
            st2n = sb.tile([128, NT, 1], F32, tag="st2n", name="st2n")
        else:
            r2b = sb.tile([128, NT, 256], BF16, tag="r2b")

        def stats_chain(mv, str_, stn, h0, h1, eng):
            """mv[:, h0:h1] = (mean, var) -> str_=rstd, stn=-mean*rstd."""
            sd = ob.tile([128, NT, 1], F32, tag="sd", bufs=2)
            nc.scalar.activation(sd[:, h0:h1, :], mv[:, h0:h1, 1:2], AF.Sqrt,
                                 bias=prm[:, 0:1])
            nc.vector.reciprocal(str_[:, h0:h1, :], sd[:, h0:h1, :])
            eng.scalar_tensor_tensor(stn[:, h0:h1, :], mv[:, h0:h1, 0:1],
                                     -1.0, str_[:, h0:h1, :],
                                     op0=ALU.mult, op1=ALU.mult)

        # ---- sweep 1: B matmul + residual, paired drains + LN1 stats ----
        for pi, (t0i, np_) in enumerate(PAIRS):
            pbp = ps.tile([128, 2, 256], F32, tag="pb", bufs=2)
            for j in range(np_):
                ti = t0i + j
                sz = _tsz(ti)
                t0 = ti * 128
                for k in range(2):
                    nc.tensor.matmul(pbp[:sz, j, :], aT[:, k, t0:t0 + sz],
                                     wo[:, k, :], start=(k == 0), stop=False)
                for k in range(2):
                    nc.tensor.matmul(pbp[:sz, j, k * 128:k * 128 + 128],
                                     xbT[:, k, t0:t0 + sz], idn[:, :],
                                     start=False, stop=(k == 1),
                                     skip_group_check=True)
            sz0 = _tsz(t0i + np_ - 1)
            if np_ == 2:
                if pi % 2 == 0:
                    nc.scalar.copy(r1a[:, t0i:t0i + 2, :], pbp[:, :, :])
                else:
                    nc.vector.tensor_copy(r1a[:, t0i:t0i + 2, :], pbp[:, :, :])
            else:
                nc.vector.tensor_copy(r1a[:sz0, t0i, :], pbp[:sz0, 0, :])
            for j in range(np_):
                ti = t0i + j
                sz = _tsz(ti)
                nc.vector.bn_stats(bst1[:sz, ti, :], r1a[:sz, ti, :])
                nc.vector.bn_aggr(mv1[:sz, ti, :], bst1[:sz, ti, :])
            if t0i + np_ == 6:
                stats_chain(mv1, st1r, st1n, 0, 6, nc.vector)
        stats_chain(mv1, st1r, st1n, 6, NT, nc.vector)

        # ---- sweep 2: LN1 apply, transpose, C, D (+ LN2 stats) ----
        for gi, (g0, gsz, tis) in enumerate(GROUPS):
            for t0i, np_ in PAIRS:
                if t0i not in tis:
                    continue
                pt = ps.tile([128, 2, 2, 128], BF16, tag="ptr", bufs=2)
                for j in range(np_):
                    ti = t0i + j
                    sz = _tsz(ti)
                    nc.scalar.activation(xna[:sz, ti, :], r1a[:sz, ti, :],
                                         AF.Identity,
                                         bias=st1n[:sz, ti, :],
                                         scale=st1r[:sz, ti, :])
                    for c in range(2):
                        nc.tensor.transpose(
                            pt[:, j, c, :sz],
                            xna[:sz, ti, c * 128:c * 128 + 128],
                            idn[:sz, :sz])
                t0 = t0i * 128
                tw = sum(_tsz(t0i + j) for j in range(np_))
                if np_ == 2:
                    src = pt.transpose((0, 2, 1, 3))
                    dst = xnTa[:, :, t0:t0 + 256].rearrange(
                        "p c (a b) -> p c a b", a=2)
                    if t0i % 4 == 0:
                        nc.vector.tensor_copy(dst, src)
                    else:
                        nc.scalar.copy(dst, src)
                else:
                    nc.vector.tensor_copy(xnTa[:, :, t0:t0 + tw],
                                          pt[:, 0, :, :tw])
            # C over the whole group: hT = relu(Wl1g.T @ xnT + bl1row)
            for m in range(8):
                pc = ps.tile([128, 512], F32, tag="pca", bufs=2)
                for k in range(2):
                    nc.tensor.matmul(pc[:, :gsz],
                                     wl1[:, k, m * 128:m * 128 + 128],
                                     xnTa[:, k, g0:g0 + gsz],
                                     start=(k == 0), stop=(k == 1))
                if m % 2 == 0:
                    nc.scalar.activation(hta[:, m, g0:g0 + gsz], pc[:, :gsz],
                                         AF.Relu, bias=prm[:, 5 + m:6 + m])
                else:
                    nc.vector.tensor_scalar(hta[:, m, g0:g0 + gsz],
                                            pc[:, :gsz], prm[:, 5 + m:6 + m],
                                            0.0, ALU.add, ALU.max)
            # D, paired into 2-slot PSUM tiles
            for t0i, np_ in PAIRS:
                if t0i not in tis:
                    continue
                pdp = ps.tile([128, 2, 256], F32, tag="pd", bufs=2)
                for j in range(np_):
                    ti = t0i + j
                    sz = _tsz(ti)
                    t0 = ti * 128
                    for k in range(8):
                        nc.tensor.matmul(pdp[:sz, j, :], hta[:, k, t0:t0 + sz],
                                         wl2[:, k, :],
                                         start=(k == 0), stop=False)
                    for k in range(2):
                        nc.tensor.matmul(pdp[:sz, j, :],
                                         xnTa[:, k, t0:t0 + sz],
                                         dg1[:, k, :], start=False, stop=False)
                    nc.tensor.matmul(pdp[:sz, j, :], ones[0:1, :sz],
                                     rows[:, :], start=False, stop=True)
                sz0 = _tsz(t0i + np_ - 1)
                if with_A:
                    if np_ == 2:
                        nc.vector.tensor_copy(r2a[:, t0i:t0i + 2, :],
                                              pdp[:, :, :])
                    else:
                        nc.vector.tensor_copy(r2a[:sz0, t0i, :],
                                              pdp[:sz0, 0, :])
                    for j in range(np_):
                        ti = t0i + j
                        sz = _tsz(ti)
                        nc.vector.bn_stats(bst2[:sz, ti, :], r2a[:sz, ti, :])
                        nc.vector.bn_aggr(mv2[:sz, ti, :], bst2[:sz, ti, :])
                else:
                    if np_ == 2:
                        if t0i % 4 == 0:
                            nc.vector.tensor_copy(r2b[:, t0i:t0i + 2, :],
                                                  pdp[:, :, :])
                        else:
                            nc.scalar.copy(r2b[:, t0i:t0i + 2, :],
                                           pdp[:, :, :])
                        dma_eng = nc.sync if t0i % 4 == 0 else nc.scalar
                        dma_eng.dma_start(
                            out_d[:, t0i * 256:(t0i + 2) * 256],
                            r2b[:, t0i:t0i + 2, :])
                    else:
                        nc.vector.tensor_copy(r2b[:sz0, t0i, :],
                                              pdp[:sz0, 0, :])
                        nc.sync.dma_start(
                            out_d[:, t0i * 256:(t0i + 1) * 256],
                            r2b[:, t0i, :])
            if with_A:
                stats_chain(mv2, st2r, st2n, tis[0], tis[-1] + 1, nc.vector)

        # ---- sweep 3 (with_A): LN2 apply + next-layer projections ----
        if with_A:
            for g0, gsz, tis in GROUPS:
                for t0i, np_ in PAIRS:
                    if t0i not in tis:
                        continue
                    pt2 = ps.tile([128, 2, 2, 128], BF16, tag="ptr", bufs=2)
                    for j in range(np_):
                        ti = t0i + j
                        sz = _tsz(ti)
                        nc.scalar.activation(xout[:sz, ti, :], r2a[:sz, ti, :],
                                             AF.Identity,
                                             bias=st2n[:sz, ti, :],
                                             scale=st2r[:sz, ti, :])
                        for c in range(2):
                            nc.tensor.transpose(
                                pt2[:, j, c, :sz],
                                xout[:sz, ti, c * 128:c * 128 + 128],
                                idn[:sz, :sz])
                    t0 = t0i * 128
                    tw = sum(_tsz(t0i + j) for j in range(np_))
                    if np_ == 2:
                        src = pt2.transpose((0, 2, 1, 3))
                        dst = xn2Ta[:, :, t0:t0 + 256].rearrange(
                            "p c (a b) -> p c a b", a=2)
                        if t0i % 4 == 0:
                            nc.vector.tensor_copy(dst, src)
                        else:
                            nc.scalar.copy(dst, src)
                    else:
                        nc.vector.tensor_copy(xn2Ta[:, :, t0:t0 + tw],
                                              pt2[:, 0, :, :tw])
                # val projections first (no q dependency)
                for m in range(2):
                    pa = ps.tile([128, 512], F32, tag="pca", bufs=2)
                    for k in range(2):
                        nc.tensor.matmul(pa[:, :gsz],
                                         wv[:, k, m * 128:m * 128 + 128],
                                         xn2Ta[:, k, g0:g0 + gsz],
                                         start=(k == 0), stop=(k == 1))
                    dst = valTs[:, m, g0:g0 + gsz]
                    if m % 2 == 0:
                        nc.scalar.activation(dst, pa[:, :gsz], AF.Identity,
                                             bias=prm[:, 13 + m:14 + m])
                    else:
                        nc.vector.tensor_scalar(dst, pa[:, :gsz],
                                                prm[:, 13 + m:14 + m],
                                                None, ALU.add)
                # q1T = (g2*xn2 + be2 + pos)^T, built on ACT + gpsimd
                qp = ob.tile([128, 2, 512], BF16, tag="qp", bufs=2)
                for c in range(2):
                    nc.scalar.activation(qp[:, c, :gsz],
                                         xn2Ta[:, c, g0:g0 + gsz],
                                         AF.Identity,
                                         bias=prm[:, 3 + c:4 + c],
                                         scale=prm[:, 1 + c:2 + c])
                nc.gpsimd.tensor_tensor(q1Ts[:, :, g0:g0 + gsz],
                                        qp[:, :, :gsz],
                                        posT[:, :, g0:g0 + gsz], op=ALU.add)
                for m in range(3):
                    pa = ps.tile([128, 512], F32, tag="pca", bufs=2)
                    for k in range(2):
                        nc.tensor.matmul(pa[:, :gsz],
                                         woa[:, k, m * 128:m * 128 + 128],
                                         q1Ts[:, k, g0:g0 + gsz],
                                         start=(k == 0), stop=(k == 1))
                    dst = oaTs[:, m, g0:g0 + gsz]
                    if m % 2 == 1:
                        nc.scalar.activation(dst, pa[:, :gsz], AF.Identity,
                                             bias=prm[:, 15 + m:16 + m])
                    else:
                        nc.vector.tensor_scalar(dst, pa[:, :gsz],
                                                prm[:, 15 + m:16 + m],
                                                None, ALU.add)
                # output DMAs per group
                lo, hi = tis[0], tis[-1] + 1
                nc.scalar.dma_start(_ccn(valT_d)[:, :, g0:g0 + gsz],
                                    valTs[:, :, g0:g0 + gsz])
                nc.sync.dma_start(
                    oaT_d.rearrange("(c p) n -> p c n", p=128)[:, :,
                                                              g0:g0 + gsz],
                    oaTs[:, :, g0:g0 + gsz])
                nc.gpsimd.dma_start(
                    x1n_d[:, lo * 256:hi * 256], xout[:, lo:hi, :])
    nc.compile()
    return nc


def _run(prog, in_maps):
    trace = bool(os.environ.get("BASS_TRACE"))
    res = run_bass_kernel_spmd(prog, in_maps, core_ids=list(range(NCORE)),
                               trace=trace)
    if res.exec_time_ns:
        HW_EXEC_NS.append(res.exec_time_ns)
    if trace:
        LAST_RES.append(res)
    return res.results


def _bf(a):
    return np.ascontiguousarray(np.asarray(a, np.float32).astype(NPBF))


def _ccn_host(w):
    """[K, M] -> [128, K//128 * M] channel-major pack block."""
    w = np.asarray(w, np.float32)
    k, m = w.shape
    return w.reshape(k // 128, 128, m).transpose(1, 0, 2).reshape(128, -1)


def _chunked(v, nch):
    v = np.asarray(v, np.float32)
    return np.ascontiguousarray(v.reshape(nch, 128).T.astype(np.float32))


def _ref_points(valid_ratios):
    refs = []
    for lvl, (H, W) in enumerate(SHAPES):
        gy, gx = np.meshgrid(np.arange(H, dtype=np.float32) + 0.5,
                             np.arange(W, dtype=np.float32) + 0.5,
                             indexing="ij")
        ry = gy.reshape(-1)[None] / (valid_ratios[:, lvl, 1][:, None] * H)
        rx = gx.reshape(-1)[None] / (valid_ratios[:, lvl, 0][:, None] * W)
        refs.append(np.stack([rx, ry], -1))
    ref = np.concatenate(refs, 1)
    return ref[:, :, None, :] * valid_ratios[:, None]


def _host_sample(value, off, aw, ref_pts):
    N, Lq = off.shape[:2]
    off = off.reshape(N, Lq, N_HEADS, N_LEVELS, N_POINTS, 2)
    aw = aw.reshape(N, Lq, N_HEADS, N_LEVELS, N_POINTS)
    normalizer = np.array([[w, h] for h, w in SHAPES], np.float32)
    loc = (ref_pts[:, :, None, :, None, :]
           + off / normalizer[None, None, None, :, None, :])
    acc = np.zeros((N, N_HEADS, Lq, HEAD_DIM), np.float32)
    for lvl, (H, W) in enumerate(SHAPES):
        s = LEVEL_STARTS[lvl]
        val = value[:, s:s + H * W].transpose(0, 2, 1, 3)
        x = loc[:, :, :, lvl, :, 0] * W - 0.5
        y = loc[:, :, :, lvl, :, 1] * H - 0.5
        x0 = np.floor(x)
        y0 = np.floor(y)
        wx1 = x - x0
        wy1 = y - y0
        ix0 = x0.astype(np.int64)
        iy0 = y0.astype(np.int64)

        def corner(ix, iy, w):
            valid = (ix >= 0) & (ix < W) & (iy >= 0) & (iy < H)
            idx = np.clip(iy, 0, H - 1) * W + np.clip(ix, 0, W - 1)
            idx = idx.transpose(0, 2, 1, 3).reshape(N, N_HEADS, Lq * N_POINTS)
            g = np.take_along_axis(val, idx[..., None], axis=2)
            g = g.reshape(N, N_HEADS, Lq, N_POINTS, HEAD_DIM)
            w = np.where(valid, w, 0.0).transpose(0, 2, 1, 3)
            return g * w[..., None].astype(np.float32)

        sampled = (corner(ix0, iy0, (1 - wx1) * (1 - wy1))
                   + corner(ix0 + 1, iy0, wx1 * (1 - wy1))
                   + corner(ix0, iy0 + 1, (1 - wx1) * wy1)
                   + corner(ix0 + 1, iy0 + 1, wx1 * wy1))
        acc += (sampled * aw[:, :, :, lvl].transpose(0, 2, 1, 3)[..., None]
                ).sum(3)
    return acc.transpose(0, 2, 1, 3).reshape(N, Lq, D_MODEL)


def _shardT(fullT):
    return [np.ascontiguousarray(fullT[c // 4, :, (c % 4) * TPC:
                                       (c % 4 + 1) * TPC])
            for c in range(NCORE)]


def _unshardT(parts):
    F = parts[0].shape[0]
    out = np.empty((BATCH, LEN_IN, F), np.float32)
    for c in range(NCORE):
        out[c // 4, (c % 4) * TPC:(c % 4 + 1) * TPC] = \
            np.asarray(parts[c], np.float32).T
    return out


def _unshard_pm(parts):  # partition-major parts [128, NT*256]
    out = np.empty((BATCH, LEN_IN, 256), np.float32)
    for c in range(NCORE):
        a = np.asarray(parts[c], np.float32).reshape(128, NT, 256)
        a = a.transpose(1, 0, 2).reshape(NT * 128, 256)[:TPC]
        out[c // 4, (c % 4) * TPC:(c % 4 + 1) * TPC] = a
    return out


_IDENT = np.eye(128, dtype=np.float32)


def kernel(src, pos, valid_ratios, Wv, bv, Woff, boff, Wa, ba, Wo, bo,
           g1, be1, Wl1, bl1, Wl2, bl2, g2, be2):
    src = np.asarray(src, np.float32)
    pos = np.asarray(pos, np.float32)
    valid_ratios = np.asarray(valid_ratios, np.float32)
    asf = lambda a: np.asarray(a, np.float32)
    HW_EXEC_NS.clear()
    LAST_RES.clear()

    if "A" not in _PROGS:
        _PROGS["A"] = _build_A()
        _PROGS["BCDA"] = _build_BCDA(with_A=True)
        _PROGS["BCD"] = _build_BCDA(with_A=False)

    ref_pts = _ref_points(valid_ratios)

    Woa = [np.concatenate([asf(Woff[l]), asf(Wa[l])], axis=1)
           for l in range(2)]
    bva = [np.concatenate([asf(bv[l]), asf(boff[l]), asf(ba[l])])
           for l in range(2)]
    Wl1g = [asf(g1[l])[:, None] * asf(Wl1[l]) for l in range(2)]
    bl1f = [asf(bl1[l]) + asf(be1[l]) @ asf(Wl1[l]) for l in range(2)]
    # layer-1 value-proj with layer-0 g2/be2 folded in (q-path keeps
    # plain Woa; q is built on device as g2*xn2 + be2 + pos)
    Wv1f = asf(g2[0])[:, None] * asf(Wv[1])
    bva1f = np.concatenate([asf(bv[1]) + asf(be2[0]) @ asf(Wv[1]),
                            bva[1][256:]])
    cr = [asf(be1[l]) + asf(bl2[l]) for l in range(2)]
    rows = [np.ascontiguousarray(cr[l][None, :].astype(NPBF))
            for l in range(2)]
    prm = [np.concatenate([np.full((128, 1), 1e-5, np.float32),
                           _chunked(g2[l], 2), _chunked(be2[l], 2),
                           _chunked(bl1f[l], 8),
                           _chunked(bva1f if l == 0 else np.zeros(640), 5)],
                          axis=1) for l in range(2)]
    dg1 = [np.diag(asf(g1[l])) for l in range(2)]

    # packed weight blobs
    wpkA = np.concatenate([_ccn_host(Wv[0]), _ccn_host(Woa[0])],
                          axis=1).astype(NPBF)
    wpk1 = [np.concatenate([_ccn_host(Wo[l]), _IDENT],
                           axis=1).astype(NPBF) for l in range(2)]
    wpk2 = [np.concatenate([_ccn_host(Wl1g[l]), _ccn_host(dg1[l]),
                            _ccn_host(Wl2[l])], axis=1).astype(NPBF)
            for l in range(2)]
    wpk3 = np.concatenate([_ccn_host(Wv1f), _ccn_host(Woa[1])],
                          axis=1).astype(NPBF)

    xT = np.ascontiguousarray(src.transpose(0, 2, 1))
    qT = np.ascontiguousarray((src + pos).transpose(0, 2, 1))
    posT = np.ascontiguousarray(pos.transpose(0, 2, 1))
    xTs = _shardT(xT.astype(NPBF))
    qTs = _shardT(qT.astype(NPBF))
    posTs = _shardT(posT.astype(NPBF))

    # ---- launch 1: layer-0 projections ----
    in_maps = [{
        "xT": xTs[c], "qT": qTs[c],
        "wpk": wpkA, "prm": _chunked(bva[0], 5),
    } for c in range(NCORE)]
    resA = _run(_PROGS["A"], in_maps)

    def gather_attn(value, offaw, layer, x_full):
        aw = offaw[:, :, 256:].reshape(BATCH, LEN_IN, N_HEADS, 16)
        aw = aw - aw.max(-1, keepdims=True)
        e = np.exp(aw)
        aw = (e / e.sum(-1, keepdims=True)).reshape(BATCH, LEN_IN, 128)
        attn = _host_sample(value.reshape(BATCH, LEN_IN, N_HEADS, HEAD_DIM),
                            offaw[:, :, :256], aw, ref_pts)
        attnT = np.ascontiguousarray(attn.transpose(0, 2, 1))
        xbf = (x_full + asf(bo[layer])[None, None, :]).transpose(0, 2, 1)
        return (_shardT(attnT.astype(NPBF)),
                _shardT(np.ascontiguousarray(xbf).astype(NPBF)))

    # ---- launch 2: layer-0 BCD + layer-1 projections ----
    value = _unshardT([resA[c]["valT"] for c in range(NCORE)])
    offaw = _unshardT([resA[c]["offawT"] for c in range(NCORE)])
    attnTs, xbs = gather_attn(value, offaw, 0, src)
    in_maps = [{
        "attnT": attnTs[c], "xbT": xbs[c],
        "wpk1": wpk1[0], "wpk2": wpk2[0], "wpk3": wpk3,
        "rows": rows[0], "prm": prm[0], "posT": posTs[c],
    } for c in range(NCORE)]
    resB = _run(_PROGS["BCDA"], in_maps)

    # x1 = g2*xn2 + be2 (host applies the folded affine)
    xn2 = _unshard_pm([resB[c]["x1n"] for c in range(NCORE)])
    x1 = xn2 * asf(g2[0])[None, None, :] + asf(be2[0])[None, None, :]

    # ---- launch 3: layer-1 BCD -> final ----
    val1 = _unshardT([resB[c]["valT"] for c in range(NCORE)])
    oa1 = _unshardT([resB[c]["offawT"] for c in range(NCORE)])
    attnTs, xbs = gather_attn(val1, oa1, 1, x1)
    in_maps = [{
        "attnT": attnTs[c], "xbT": xbs[c],
        "wpk1": wpk1[1], "wpk2": wpk2[1],
        "rows": rows[1], "prm": prm[1],
    } for c in range(NCORE)]
    resC = _run(_PROGS["BCD"], in_maps)

    # host LN2 + affine for the final layer
    r2 = _unshard_pm([resC[c]["out"] for c in range(NCORE)])
    m = r2.mean(-1, keepdims=True)
    v = np.square(r2 - m).mean(-1, keepdims=True)
    xn = (r2 - m) / np.sqrt(v + 1e-5)
    return (xn * asf(g2[1])[None, None, :]
            + asf(be2[1])[None, None, :]).astype(np.float32)


# revision 16
# speedup vs baseline: 1.0910x; 1.0728x over previous
"""Deformable-Transformer encoder on 8 trn2 NeuronCores — v4.

v3 + latency restructuring driven by NTFF traces:
  - Scratch-tile PE warmup at body start (no DMA dependency) so the HAM
    clock-gate is at 8/8 by the time real matmuls start; removes the
    wo-dependent warmup and the dummy transpose fillers.
  - Input DMAs packed (weights into 1-2 blobs) and spread across
    sync/scalar/vector/gpsimd queues so issue serialization (~0.65us per
    dma instr on one engine) stops gating the first matmul.
  - Drains paired: two 128-token tiles share one 2-slot PSUM tile, so
    PSUM->SBUF evacuation + bn_stats run at half the op count.
  - LN stats via bn_aggr (1 op) instead of a 9-op manual combine.
  - Final launch (BCD) skips LN2 entirely: it streams out the pre-LN2
    residual r2 per tile-pair and the host applies LN2+affine.
"""
import os
import sys
import types
import contextlib
import ctypes
import numpy as np

sys.path.insert(0, "/opt/trn_rl_repo")


def _install_ntff_hook():
    try:
        import antenv

        if hasattr(antenv, "axon_hooks"):
            return
        so_path = "/opt/axon/libaxon_pjrt.so"
        lib = ctypes.CDLL(so_path)
        if not hasattr(lib, "axon_start_nrt_profile"):
            hook = None
        else:
            lib.axon_start_nrt_profile.argtypes = [
                ctypes.POINTER(ctypes.c_int64), ctypes.c_size_t]
            lib.axon_start_nrt_profile.restype = ctypes.c_int64
            lib.axon_stop_nrt_profile.argtypes = [ctypes.c_char_p]
            lib.axon_stop_nrt_profile.restype = ctypes.c_int64

            @contextlib.contextmanager
            def hook(output_dir, device_ids):
                import jax
                jax.devices()
                if device_ids:
                    ids = (ctypes.c_int64 * len(device_ids))(*device_ids)
                    rc = lib.axon_start_nrt_profile(ids, len(device_ids))
                else:
                    rc = lib.axon_start_nrt_profile(None, 0)
                if rc != 0:
                    raise RuntimeError(f"start_nrt_profile rc={rc}")
                try:
                    yield
                finally:
                    lib.axon_stop_nrt_profile(str(output_dir).encode())

        m = types.ModuleType("antenv.axon_hooks")
        m.get_axon_ntff_profile_hook = lambda: hook
        m.set_axon_ntff_profile_hook = lambda h: None
        sys.modules["antenv.axon_hooks"] = m
        antenv.axon_hooks = m
    except Exception:
        pass


_install_ntff_hook()

import ml_dtypes  # noqa: E402
from concourse import bacc, tile, mybir, bass  # noqa: E402
from concourse.tile import add_dep_helper  # noqa: E402
from concourse.bass_utils import run_bass_kernel_spmd  # noqa: E402
from contextlib import ExitStack  # noqa: E402

F32 = mybir.dt.float32
BF16 = mybir.dt.bfloat16
NPBF = ml_dtypes.bfloat16
AF = mybir.ActivationFunctionType
ALU = mybir.AluOpType

SHAPES = ((64, 64), (32, 32), (16, 16), (8, 8))
LEVEL_STARTS = [0, 4096, 5120, 5376, 5440]
N_LEVELS, N_HEADS, N_POINTS = 4, 8, 4
D_MODEL, HEAD_DIM, D_FFN = 256, 32, 1024
LEN_IN, BATCH, NCORE = 5440, 2, 8
TPC = LEN_IN * BATCH // NCORE  # 1360 tokens per core
NT = 11                        # 128-token tiles per core
GROUPS = [(0, 512, range(0, 4)), (512, 512, range(4, 8)),
          (1024, 336, range(8, 11))]
PAIRS = [(0, 2), (2, 2), (4, 2), (6, 2), (8, 2), (10, 1)]
WARMUP_MM = 16

HW_EXEC_NS = []
LAST_RES = []
_PROGS = {}


def _nc():
    return bacc.Bacc("TRN2", target_bir_lowering=False, debug=False,
                     num_devices=NCORE)


def _tsz(ti):
    return min(128, TPC - ti * 128)


def _ccn(d):
    return d.rearrange("(c p) n -> p c n", p=128)


def _tchunks(step):
    out = []
    t0 = 0
    while t0 < TPC:
        out.append((t0, min(step, TPC - t0)))
        t0 += step
    return out


def _warmup(nc, sb, ps, ps_tag, bufs=2):
    """HAM warmup: dense matmuls on a memset scratch tile, no DMA deps."""
    wsc = sb.tile([128, 256], BF16, tag="wsc")
    nc.gpsimd.memset(wsc[:], 0.25)
    for _ in range(WARMUP_MM):
        pw = ps.tile([128, 256], F32, tag=ps_tag, bufs=bufs)
        nc.tensor.matmul(pw[:], wsc[:, 0:128], wsc[:],
                         start=True, stop=True)


def _build_A():
    """Layer-0 projections, channel-major world."""
    nc = _nc()
    xT_d = nc.dram_tensor("xT", [D_MODEL, TPC], BF16, kind="ExternalInput").ap()
    qT_d = nc.dram_tensor("qT", [D_MODEL, TPC], BF16, kind="ExternalInput").ap()
    wpk_d = nc.dram_tensor("wpk", [128, 1280], BF16, kind="ExternalInput").ap()
    prm_d = nc.dram_tensor("prm", [128, 5], F32, kind="ExternalInput").ap()
    valT_d = nc.dram_tensor("valT", [256, TPC], BF16,
                            kind="ExternalOutput").ap()
    oaT_d = nc.dram_tensor("offawT", [384, TPC], BF16,
                           kind="ExternalOutput").ap()

    with tile.TileContext(nc) as tc, ExitStack() as ctx:
        sb = ctx.enter_context(tc.tile_pool(name="sb", bufs=1))
        ps = ctx.enter_context(tc.tile_pool(name="ps", bufs=1, space="PSUM"))
        ob = ctx.enter_context(tc.tile_pool(name="ob", bufs=1))

        _warmup(nc, sb, ps, "p", bufs=3)

        wpk = sb.tile([128, 1280], BF16, tag="wpk")
        nc.sync.dma_start(wpk[:], wpk_d[:])
        wv = wpk[:, 0:512].rearrange("p (c n) -> p c n", c=2)
        woa = wpk[:, 512:1280].rearrange("p (c n) -> p c n", c=2)
        prm = sb.tile([128, 5], F32, tag="prm")
        nc.gpsimd.dma_start(prm[:], prm_d[:])

        chunks = _tchunks(512)
        xcs, qcs = [], []
        # chunk-0 input DMAs up front; later chunks deferred on PE progress
        xc = ob.tile([128, 2, 512], BF16, tag="xc", bufs=2)
        qc = ob.tile([128, 2, 512], BF16, tag="qc", bufs=2)
        t0, tsz = chunks[0]
        nc.sync.dma_start(xc[:, :, :tsz], _ccn(xT_d)[:, :, t0:t0 + tsz])
        nc.sync.dma_start(qc[:, :, :tsz], _ccn(qT_d)[:, :, t0:t0 + tsz])
        xcs.append(xc)
        qcs.append(qc)

        for ci, (t0, tsz) in enumerate(chunks):
            xc, qc = xcs[ci], qcs[ci]
            vsb = ob.tile([128, 2, 512], BF16, tag="vsb", bufs=2)
            osb = ob.tile([128, 3, 512], BF16, tag="osb", bufs=2)
            first_mm = None
            for m in range(5):  # 0-1: val (from x), 2-4: offaw (from q)
                src = xc if m < 2 else qc
                w = wv if m < 2 else woa
                mm = m if m < 2 else m - 2
                p = ps.tile([128, 512], F32, tag="p", bufs=3)
                for k in range(2):
                    mi = nc.tensor.matmul(
                        p[:, :tsz], w[:, k, mm * 128:mm * 128 + 128],
                        src[:, k, :tsz], start=(k == 0), stop=(k == 1))
                    if first_mm is None:
                        first_mm = mi
                dst = (vsb if m < 2 else osb)[:, mm, :tsz]
                if m % 2 == 0:
                    nc.scalar.activation(dst, p[:, :tsz], AF.Identity,
                                         bias=prm[:, m:m + 1])
                else:
                    nc.vector.tensor_scalar(dst, p[:, :tsz], prm[:, m:m + 1],
                                            None, ALU.add)
            if ci + 1 < len(chunks):
                t1, tsz1 = chunks[ci + 1]
                xn = ob.tile([128, 2, 512], BF16, tag="xc", bufs=2)
                qn = ob.tile([128, 2, 512], BF16, tag="qc", bufs=2)
                d1 = nc.sync.dma_start(xn[:, :, :tsz1],
                                       _ccn(xT_d)[:, :, t1:t1 + tsz1])
                nc.sync.dma_start(qn[:, :, :tsz1],
                                  _ccn(qT_d)[:, :, t1:t1 + tsz1])
                add_dep_helper(d1.ins, first_mm.ins, sync=True,
                               reason="defer chunk dma")
                xcs.append(xn)
                qcs.append(qn)
            nc.scalar.dma_start(_ccn(valT_d)[:, :, t0:t0 + tsz],
                                vsb[:, :, :tsz])
            nc.sync.dma_start(
                oaT_d.rearrange("(c p) n -> p c n", p=128)[:, :, t0:t0 + tsz],
                osb[:, :, :tsz])
    nc.compile()
    return nc


def _build_BCDA(with_A):
    """Fused out-proj + LN1 + FFN (+ LN2 + next-layer projections).

    with_A=True (layer 0): outputs x1n (pre-affine LN2), valT, offawT.
    with_A=False (layer 1): outputs out = r2 (pre-LN2 residual); the host
    applies LN2 + g2/be2.
    """
    nc = _nc()
    aT_d = nc.dram_tensor("attnT", [D_MODEL, TPC], BF16,
                          kind="ExternalInput").ap()
    xbT_d = nc.dram_tensor("xbT", [D_MODEL, TPC], BF16,
                           kind="ExternalInput").ap()
    # wpk1: wo (2x256) | ident (128)
    wpk1_d = nc.dram_tensor("wpk1", [128, 640], BF16,
                            kind="ExternalInput").ap()
    # wpk2: wl1 (2x1024) | dg1 (2x256) | wl2 (8x256)
    wpk2_d = nc.dram_tensor("wpk2", [128, 4608], BF16,
                            kind="ExternalInput").ap()
    rows_d = nc.dram_tensor("rows", [1, 256], BF16, kind="ExternalInput").ap()
    prm_d = nc.dram_tensor("prm", [128, 18], F32, kind="ExternalInput").ap()
    if with_A:
        posT_d = nc.dram_tensor("posT", [D_MODEL, TPC], BF16,
                                kind="ExternalInput").ap()
        # wpk3: wv (2x256) | woa (2x384)
        wpk3_d = nc.dram_tensor("wpk3", [128, 1280], BF16,
                                kind="ExternalInput").ap()
        x1n_d = nc.dram_tensor("x1n", [128, NT * 256], BF16,
                               kind="ExternalOutput").ap()
        valT_d = nc.dram_tensor("valT", [256, TPC], BF16,
                                kind="ExternalOutput").ap()
        oaT_d = nc.dram_tensor("offawT", [384, TPC], BF16,
                               kind="ExternalOutput").ap()
    else:
        out_d = nc.dram_tensor("out", [128, NT * 256], BF16,
                               kind="ExternalOutput").ap()

    with tile.TileContext(nc) as tc, ExitStack() as ctx:
        sb = ctx.enter_context(tc.tile_pool(name="sb", bufs=1))
        ps = ctx.enter_context(tc.tile_pool(name="ps", bufs=1, space="PSUM"))
        ob = ctx.enter_context(tc.tile_pool(name="ob", bufs=1))

        _warmup(nc, sb, ps, "pb")

        # ---- input DMAs: critical ones up front, bulk deferred ----
        wpk1 = sb.tile([128, 640], BF16, tag="wpk1")
        nc.sync.dma_start(wpk1[:], wpk1_d[:])
        wo = wpk1[:, 0:512].rearrange("p (c n) -> p c n", c=2)
        idn = wpk1[:, 512:640]
        aT = sb.tile([128, 2, TPC], BF16, tag="aT")
        nc.sync.dma_start(aT[:, :, 0:512], _ccn(aT_d)[:, :, 0:512])
        nc.sync.dma_start(aT[:, :, 512:TPC], _ccn(aT_d)[:, :, 512:TPC])
        xbT = sb.tile([128, 2, TPC], BF16, tag="xbT")
        nc.scalar.dma_start(xbT[:, :, 0:512], _ccn(xbT_d)[:, :, 0:512])
        nc.scalar.dma_start(xbT[:, :, 512:TPC], _ccn(xbT_d)[:, :, 512:TPC])
        # wpk2/posT/wpk3 tiles declared now, DMAs emitted inside sweep 1
        # gated on PE progress so they don't steal HBM bandwidth from aT/xbT
        wpk2 = sb.tile([128, 4608], BF16, tag="wpk2")
        wl1 = wpk2[:, 0:2048].rearrange("p (c n) -> p c n", c=2)
        dg1 = wpk2[:, 2048:2560].rearrange("p (c n) -> p c n", c=2)
        wl2 = wpk2[:, 2560:4608].rearrange("p (c n) -> p c n", c=8)
        prm = sb.tile([128, 18], F32, tag="prm")
        nc.gpsimd.dma_start(prm[:], prm_d[:])
        rows = sb.tile([1, 256], BF16, tag="rows")
        nc.gpsimd.dma_start(rows[:], rows_d[:])
        if with_A:
            posT = sb.tile([128, 2, TPC], BF16, tag="posT")
            wpk3 = sb.tile([128, 1280], BF16, tag="wpk3")
            wv = wpk3[:, 0:512].rearrange("p (c n) -> p c n", c=2)
            woa = wpk3[:, 512:1280].rearrange("p (c n) -> p c n", c=2)
            valTs = sb.tile([128, 2, TPC], BF16, tag="valTs")
            oaTs = sb.tile([128, 3, TPC], BF16, tag="oaTs")
            q1Ts = sb.tile([128, 2, TPC], BF16, tag="q1Ts")
        ones = sb.tile([1, 512], BF16, tag="ones")
        nc.gpsimd.memset(ones[:], 1.0)

        # persistent intermediates
        r1a = sb.tile([128, NT, 256], F32, tag="r1a")
        xnTa = sb.tile([128, 2, TPC], BF16, tag="xnTa")
        hta = sb.tile([128, 8, TPC], BF16, tag="hta")
        xna = sb.tile([128, NT, 256], BF16, tag="xna")
        bst1 = sb.tile([128, NT, 6], F32, tag="bst1")
        mv1 = sb.tile([128, NT, 2], F32, tag="mv1")
        st1r = sb.tile([128, NT, 1], F32, tag="st1r", name="st1r")
        st1n = sb.tile([128, NT, 1], F32, tag="st1n", name="st1n")
        if with_A:
            r2a = sb.tile([128, NT, 256], F32, tag="r2a")
            xn2Ta = sb.tile([128, 2, TPC], BF16, tag="xn2Ta")
            xout = sb.tile([128, NT, 256], BF16, tag="xout")
            bst2 = sb.tile([128, NT, 6], F32, tag="bst2")
            mv2 = sb.tile([128, NT, 2], F32, tag="mv2")
            st2r = sb.tile([128, NT, 1], F32, tag="st2r", name="st2r")
            st2n = sb.tile([128, NT, 1], F32, tag="st2n", name="st2n")
        else:
            r2b = sb.tile([128, NT, 256], BF16, tag="r2b")

        def stats_chain(mv, str_, stn, h0, h1, eng):
            """mv[:, h0:h1] = (mean, var) -> str_=rstd, stn=-mean*rstd."""
            sd = ob.tile([128, NT, 1], F32, tag="sd", bufs=2)
            nc.scalar.activation(sd[:, h0:h1, :], mv[:, h0:h1, 1:2], AF.Sqrt,
                                 bias=prm[:, 0:1])
            nc.vector.reciprocal(str_[:, h0:h1, :], sd[:, h0:h1, :])
            eng.scalar_tensor_tensor(stn[:, h0:h1, :], mv[:, h0:h1, 0:1],
                                     -1.0, str_[:, h0:h1, :],
                                     op0=ALU.mult, op1=ALU.mult)

        # ---- sweep 1: B matmul + residual, paired drains + LN1 stats ----
        for pi, (t0i, np_) in enumerate(PAIRS):
            pbp = ps.tile([128, 2, 256], F32, tag="pb", bufs=2)
            first_mm = None
            for j in range(np_):
                ti = t0i + j
                sz = _tsz(ti)
                t0 = ti * 128
                for k in range(2):
                    mi = nc.tensor.matmul(pbp[:sz, j, :],
                                          aT[:, k, t0:t0 + sz],
                                          wo[:, k, :], start=(k == 0),
                                          stop=False)
                    if first_mm is None:
                        first_mm = mi
                for k in range(2):
                    nc.tensor.matmul(pbp[:sz, j, k * 128:k * 128 + 128],
                                     xbT[:, k, t0:t0 + sz], idn[:, :],
                                     start=False, stop=(k == 1),
                                     skip_group_check=True)
            if pi == 0:
                d = nc.sync.dma_start(wpk2[:], wpk2_d[:])
                add_dep_helper(d.ins, first_mm.ins, sync=True,
                               reason="defer wpk2 dma")
            elif pi == 2 and with_A:
                d = nc.sync.dma_start(posT[:], _ccn(posT_d))
                nc.sync.dma_start(wpk3[:], wpk3_d[:])
                add_dep_helper(d.ins, first_mm.ins, sync=True,
                               reason="defer posT/wpk3 dma")
            sz0 = _tsz(t0i + np_ - 1)
            if np_ == 2:
                if pi % 2 == 0:
                    nc.scalar.copy(r1a[:, t0i:t0i + 2, :], pbp[:, :, :])
                else:
                    nc.vector.tensor_copy(r1a[:, t0i:t0i + 2, :], pbp[:, :, :])
            else:
                nc.vector.tensor_copy(r1a[:sz0, t0i, :], pbp[:sz0, 0, :])
            for j in range(np_):
                ti = t0i + j
                sz = _tsz(ti)
                nc.vector.bn_stats(bst1[:sz, ti, :], r1a[:sz, ti, :])
                nc.vector.bn_aggr(mv1[:sz, ti, :], bst1[:sz, ti, :])
            if t0i + np_ == 6:
                stats_chain(mv1, st1r, st1n, 0, 6, nc.vector)
        stats_chain(mv1, st1r, st1n, 6, NT, nc.vector)

        # ---- sweep 2: LN1 apply, transpose, C, D (+ LN2 stats) ----
        for gi, (g0, gsz, tis) in enumerate(GROUPS):
            for t0i, np_ in PAIRS:
                if t0i not in tis:
                    continue
                pt = ps.tile([128, 2, 2, 128], BF16, tag="ptr", bufs=2)
                for j in range(np_):
                    ti = t0i + j
                    sz = _tsz(ti)
                    nc.scalar.activation(xna[:sz, ti, :], r1a[:sz, ti, :],
                                         AF.Identity,
                                         bias=st1n[:sz, ti, :],
                                         scale=st1r[:sz, ti, :])
                    for c in range(2):
                        nc.tensor.transpose(
                            pt[:, j, c, :sz],
                            xna[:sz, ti, c * 128:c * 128 + 128],
                            idn[:sz, :sz])
                t0 = t0i * 128
                tw = sum(_tsz(t0i + j) for j in range(np_))
                if np_ == 2:
                    src = pt.transpose((0, 2, 1, 3))
                    dst = xnTa[:, :, t0:t0 + 256].rearrange(
                        "p c (a b) -> p c a b", a=2)
                    if t0i % 4 == 0:
                        nc.vector.tensor_copy(dst, src)
                    else:
                        nc.scalar.copy(dst, src)
                else:
                    nc.vector.tensor_copy(xnTa[:, :, t0:t0 + tw],
                                          pt[:, 0, :, :tw])
            # C over the whole group: hT = relu(Wl1g.T @ xnT + bl1row)
            for m in range(8):
                pc = ps.tile([128, 512], F32, tag="pca", bufs=2)
                for k in range(2):
                    nc.tensor.matmul(pc[:, :gsz],
                                     wl1[:, k, m * 128:m * 128 + 128],
                                     xnTa[:, k, g0:g0 + gsz],
                                     start=(k == 0), stop=(k == 1))
                if m % 2 == 0:
                    nc.scalar.activation(hta[:, m, g0:g0 + gsz], pc[:, :gsz],
                                         AF.Relu, bias=prm[:, 5 + m:6 + m])
                else:
                    nc.vector.tensor_scalar(hta[:, m, g0:g0 + gsz],
                                            pc[:, :gsz], prm[:, 5 + m:6 + m],
                                            0.0, ALU.add, ALU.max)
            # D, paired into 2-slot PSUM tiles
            for t0i, np_ in PAIRS:
                if t0i not in tis:
                    continue
                pdp = ps.tile([128, 2, 256], F32, tag="pd", bufs=2)
                for j in range(np_):
                    ti = t0i + j
                    sz = _tsz(ti)
                    t0 = ti * 128
                    for k in range(8):
                        nc.tensor.matmul(pdp[:sz, j, :], hta[:, k, t0:t0 + sz],
                                         wl2[:, k, :],
                                         start=(k == 0), stop=False)
                    for k in range(2):
                        nc.tensor.matmul(pdp[:sz, j, :],
                                         xnTa[:, k, t0:t0 + sz],
                                         dg1[:, k, :], start=False, stop=False)
                    nc.tensor.matmul(pdp[:sz, j, :], ones[0:1, :sz],
                                     rows[:, :], start=False, stop=True)
                sz0 = _tsz(t0i + np_ - 1)
                if with_A:
                    if np_ == 2:
                        nc.vector.tensor_copy(r2a[:, t0i:t0i + 2, :],
                                              pdp[:, :, :])
                    else:
                        nc.vector.tensor_copy(r2a[:sz0, t0i, :],
                                              pdp[:sz0, 0, :])
                    for j in range(np_):
                        ti = t0i + j
                        sz = _tsz(ti)
                        nc.vector.bn_stats(bst2[:sz, ti, :], r2a[:sz, ti, :])
                        nc.vector.bn_aggr(mv2[:sz, ti, :], bst2[:sz, ti, :])
                else:
                    if np_ == 2:
                        if t0i % 4 == 0:
                            nc.vector.tensor_copy(r2b[:, t0i:t0i + 2, :],
                                                  pdp[:, :, :])
                        else:
                            nc.scalar.copy(r2b[:, t0i:t0i + 2, :],
                                           pdp[:, :, :])
                        dma_eng = nc.sync if t0i % 4 == 0 else nc.scalar
                        dma_eng.dma_start(
                            out_d[:, t0i * 256:(t0i + 2) * 256],
                            r2b[:, t0i:t0i + 2, :])
                    else:
                        nc.vector.tensor_copy(r2b[:sz0, t0i, :],
                                              pdp[:sz0, 0, :])
                        nc.sync.dma_start(
                            out_d[:, t0i * 256:(t0i + 1) * 256],
                            r2b[:, t0i, :])
            if with_A:
                stats_chain(mv2, st2r, st2n, tis[0], tis[-1] + 1, nc.vector)

        # ---- sweep 3 (with_A): LN2 apply + next-layer projections ----
        if with_A:
            for g0, gsz, tis in GROUPS:
                for t0i, np_ in PAIRS:
                    if t0i not in tis:
                        continue
                    pt2 = ps.tile([128, 2, 2, 128], BF16, tag="ptr", bufs=2)
                    for j in range(np_):
                        ti = t0i + j
                        sz = _tsz(ti)
                        nc.scalar.activation(xout[:sz, ti, :], r2a[:sz, ti, :],
                                             AF.Identity,
                                             bias=st2n[:sz, ti, :],
                                             scale=st2r[:sz, ti, :])
                        for c in range(2):
                            nc.tensor.transpose(
                                pt2[:, j, c, :sz],
                                xout[:sz, ti, c * 128:c * 128 + 128],
                                idn[:sz, :sz])
                    t0 = t0i * 128
                    tw = sum(_tsz(t0i + j) for j in range(np_))
                    if np_ == 2:
                        src = pt2.transpose((0, 2, 1, 3))
                        dst = xn2Ta[:, :, t0:t0 + 256].rearrange(
                            "p c (a b) -> p c a b", a=2)
                        nc.scalar.copy(dst, src)
                        # q1T = g2*xn2T + (be2+pos)T, fused into the drain
                        for c in range(2):
                            nc.vector.scalar_tensor_tensor(
                                q1Ts[:, c, t0:t0 + 256].rearrange(
                                    "p (a b) -> p a b", a=2),
                                pt2[:, :, c, :], prm[:, 1 + c:2 + c],
                                posT[:, c, t0:t0 + 256].rearrange(
                                    "p (a b) -> p a b", a=2),
                                op0=ALU.mult, op1=ALU.add)
                    else:
                        nc.scalar.copy(xn2Ta[:, :, t0:t0 + tw],
                                       pt2[:, 0, :, :tw])
                        for c in range(2):
                            nc.vector.scalar_tensor_tensor(
                                q1Ts[:, c, t0:t0 + tw],
                                pt2[:, 0, c, :tw], prm[:, 1 + c:2 + c],
                                posT[:, c, t0:t0 + tw],
                                op0=ALU.mult, op1=ALU.add)
                # val projections first (no q dependency)
                for m in range(2):
                    pa = ps.tile([128, 512], F32, tag="pca", bufs=2)
                    for k in range(2):
                        nc.tensor.matmul(pa[:, :gsz],
                                         wv[:, k, m * 128:m * 128 + 128],
                                         xn2Ta[:, k, g0:g0 + gsz],
                                         start=(k == 0), stop=(k == 1))
                    dst = valTs[:, m, g0:g0 + gsz]
                    if m % 2 == 0:
                        nc.scalar.activation(dst, pa[:, :gsz], AF.Identity,
                                             bias=prm[:, 13 + m:14 + m])
                    else:
                        nc.vector.tensor_scalar(dst, pa[:, :gsz],
                                                prm[:, 13 + m:14 + m],
                                                None, ALU.add)
                for m in range(3):
                    pa = ps.tile([128, 512], F32, tag="pca", bufs=2)
                    for k in range(2):
                        nc.tensor.matmul(pa[:, :gsz],
                                         woa[:, k, m * 128:m * 128 + 128],
                                         q1Ts[:, k, g0:g0 + gsz],
                                         start=(k == 0), stop=(k == 1))
                    dst = oaTs[:, m, g0:g0 + gsz]
                    if m % 2 == 1:
                        nc.scalar.activation(dst, pa[:, :gsz], AF.Identity,
                                             bias=prm[:, 15 + m:16 + m])
                    else:
                        nc.vector.tensor_scalar(dst, pa[:, :gsz],
                                                prm[:, 15 + m:16 + m],
                                                None, ALU.add)
                # output DMAs per group
                lo, hi = tis[0], tis[-1] + 1
                nc.scalar.dma_start(_ccn(valT_d)[:, :, g0:g0 + gsz],
                                    valTs[:, :, g0:g0 + gsz])
                nc.sync.dma_start(
                    oaT_d.rearrange("(c p) n -> p c n", p=128)[:, :,
                                                              g0:g0 + gsz],
                    oaTs[:, :, g0:g0 + gsz])
                nc.gpsimd.dma_start(
                    x1n_d[:, lo * 256:hi * 256], xout[:, lo:hi, :])
    nc.compile()
    return nc


def _run(prog, in_maps):
    trace = bool(os.environ.get("BASS_TRACE"))
    res = run_bass_kernel_spmd(prog, in_maps, core_ids=list(range(NCORE)),
                               trace=trace)
    if res.exec_time_ns:
        HW_EXEC_NS.append(res.exec_time_ns)
    if trace:
        LAST_RES.append(res)
    return res.results


def _bf(a):
    return np.ascontiguousarray(np.asarray(a, np.float32).astype(NPBF))


def _ccn_host(w):
    """[K, M] -> [128, K//128 * M] channel-major pack block."""
    w = np.asarray(w, np.float32)
    k, m = w.shape
    return w.reshape(k // 128, 128, m).transpose(1, 0, 2).reshape(128, -1)


def _chunked(v, nch):
    v = np.asarray(v, np.float32)
    return np.ascontiguousarray(v.reshape(nch, 128).T.astype(np.float32))


def _ref_points(valid_ratios):
    refs = []
    for lvl, (H, W) in enumerate(SHAPES):
        gy, gx = np.meshgrid(np.arange(H, dtype=np.float32) + 0.5,
                             np.arange(W, dtype=np.float32) + 0.5,
                             indexing="ij")
        ry = gy.reshape(-1)[None] / (valid_ratios[:, lvl, 1][:, None] * H)
        rx = gx.reshape(-1)[None] / (valid_ratios[:, lvl, 0][:, None] * W)
        refs.append(np.stack([rx, ry], -1))
    ref = np.concatenate(refs, 1)
    return ref[:, :, None, :] * valid_ratios[:, None]


def _host_sample(value, off, aw, ref_pts):
    N, Lq = off.shape[:2]
    off = off.reshape(N, Lq, N_HEADS, N_LEVELS, N_POINTS, 2)
    aw = aw.reshape(N, Lq, N_HEADS, N_LEVELS, N_POINTS)
    normalizer = np.array([[w, h] for h, w in SHAPES], np.float32)
    loc = (ref_pts[:, :, None, :, None, :]
           + off / normalizer[None, None, None, :, None, :])
    acc = np.zeros((N, N_HEADS, Lq, HEAD_DIM), np.float32)
    for lvl, (H, W) in enumerate(SHAPES):
        s = LEVEL_STARTS[lvl]
        val = value[:, s:s + H * W].transpose(0, 2, 1, 3)
        x = loc[:, :, :, lvl, :, 0] * W - 0.5
        y = loc[:, :, :, lvl, :, 1] * H - 0.5
        x0 = np.floor(x)
        y0 = np.floor(y)
        wx1 = x - x0
        wy1 = y - y0
        ix0 = x0.astype(np.int64)
        iy0 = y0.astype(np.int64)

        def corner(ix, iy, w):
            valid = (ix >= 0) & (ix < W) & (iy >= 0) & (iy < H)
            idx = np.clip(iy, 0, H - 1) * W + np.clip(ix, 0, W - 1)
            idx = idx.transpose(0, 2, 1, 3).reshape(N, N_HEADS, Lq * N_POINTS)
            g = np.take_along_axis(val, idx[..., None], axis=2)
            g = g.reshape(N, N_HEADS, Lq, N_POINTS, HEAD_DIM)
            w = np.where(valid, w, 0.0).transpose(0, 2, 1, 3)
            return g * w[..., None].astype(np.float32)

        sampled = (corner(ix0, iy0, (1 - wx1) * (1 - wy1))
                   + corner(ix0 + 1, iy0, wx1 * (1 - wy1))
                   + corner(ix0, iy0 + 1, (1 - wx1) * wy1)
                   + corner(ix0 + 1, iy0 + 1, wx1 * wy1))
        acc += (sampled * aw[:, :, :, lvl].transpose(0, 2, 1, 3)[..., None]
                ).sum(3)
    return acc.transpose(0, 2, 1, 3).reshape(N, Lq, D_MODEL)


def _shardT(fullT):
    return [np.ascontiguousarray(fullT[c // 4, :, (c % 4) * TPC:
                                       (c % 4 + 1) * TPC])
            for c in range(NCORE)]


def _unshardT(parts):
    F = parts[0].shape[0]
    out = np.empty((BATCH, LEN_IN, F), np.float32)
    for c in range(NCORE):
        out[c // 4, (c % 4) * TPC:(c % 4 + 1) * TPC] = \
            np.asarray(parts[c], np.float32).T
    return out


def _unshard_pm(parts):  # partition-major parts [128, NT*256]
    out = np.empty((BATCH, LEN_IN, 256), np.float32)
    for c in range(NCORE):
        a = np.asarray(parts[c], np.float32).reshape(128, NT, 256)
        a = a.transpose(1, 0, 2).reshape(NT * 128, 256)[:TPC]
        out[c // 4, (c % 4) * TPC:(c % 4 + 1) * TPC] = a
    return out


_IDENT = np.eye(128, dtype=np.float32)


def kernel(src, pos, valid_ratios, Wv, bv, Woff, boff, Wa, ba, Wo, bo,
           g1, be1, Wl1, bl1, Wl2, bl2, g2, be2):
    src = np.asarray(src, np.float32)
    pos = np.asarray(pos, np.float32)
    valid_ratios = np.asarray(valid_ratios, np.float32)
    asf = lambda a: np.asarray(a, np.float32)
    HW_EXEC_NS.clear()
    LAST_RES.clear()

    if "A" not in _PROGS:
        _PROGS["A"] = _build_A()
        _PROGS["BCDA"] = _build_BCDA(with_A=True)
        _PROGS["BCD"] = _build_BCDA(with_A=False)

    ref_pts = _ref_points(valid_ratios)

    Woa = [np.concatenate([asf(Woff[l]), asf(Wa[l])], axis=1)
           for l in range(2)]
    bva = [np.concatenate([asf(bv[l]), asf(boff[l]), asf(ba[l])])
           for l in range(2)]
    Wl1g = [asf(g1[l])[:, None] * asf(Wl1[l]) for l in range(2)]
    bl1f = [asf(bl1[l]) + asf(be1[l]) @ asf(Wl1[l]) for l in range(2)]
    # layer-1 value-proj with layer-0 g2/be2 folded in (q-path keeps
    # plain Woa; q is built on device as g2*xn2 + be2 + pos)
    Wv1f = asf(g2[0])[:, None] * asf(Wv[1])
    bva1f = np.concatenate([asf(bv[1]) + asf(be2[0]) @ asf(Wv[1]),
                            bva[1][256:]])
    cr = [asf(be1[l]) + asf(bl2[l]) for l in range(2)]
    rows = [np.ascontiguousarray(cr[l][None, :].astype(NPBF))
            for l in range(2)]
    prm = [np.concatenate([np.full((128, 1), 1e-5, np.float32),
                           _chunked(g2[l], 2), _chunked(be2[l], 2),
                           _chunked(bl1f[l], 8),
                           _chunked(bva1f if l == 0 else np.zeros(640), 5)],
                          axis=1) for l in range(2)]
    dg1 = [np.diag(asf(g1[l])) for l in range(2)]

    # packed weight blobs
    wpkA = np.concatenate([_ccn_host(Wv[0]), _ccn_host(Woa[0])],
                          axis=1).astype(NPBF)
    wpk1 = [np.concatenate([_ccn_host(Wo[l]), _IDENT],
                           axis=1).astype(NPBF) for l in range(2)]
    wpk2 = [np.concatenate([_ccn_host(Wl1g[l]), _ccn_host(dg1[l]),
                            _ccn_host(Wl2[l])], axis=1).astype(NPBF)
            for l in range(2)]
    wpk3 = np.concatenate([_ccn_host(Wv1f), _ccn_host(Woa[1])],
                          axis=1).astype(NPBF)

    xT = np.ascontiguousarray(src.transpose(0, 2, 1))
    qT = np.ascontiguousarray((src + pos).transpose(0, 2, 1))
    # q for layer-1 projections is g2*xn2 + (be2 + pos); fold be2 into pos
    posbT = np.ascontiguousarray(
        (pos + asf(be2[0])[None, None, :]).transpose(0, 2, 1))
    xTs = _shardT(xT.astype(NPBF))
    qTs = _shardT(qT.astype(NPBF))
    posTs = _shardT(posbT.astype(NPBF))

    # ---- launch 1: layer-0 projections ----
    in_maps = [{
        "xT": xTs[c], "qT": qTs[c],
        "wpk": wpkA, "prm": _chunked(bva[0], 5),
    } for c in range(NCORE)]
    resA = _run(_PROGS["A"], in_maps)

    def gather_attn(value, offaw, layer, x_full):
        aw = offaw[:, :, 256:].reshape(BATCH, LEN_IN, N_HEADS, 16)
        aw = aw - aw.max(-1, keepdims=True)
        e = np.exp(aw)
        aw = (e / e.sum(-1, keepdims=True)).reshape(BATCH, LEN_IN, 128)
        attn = _host_sample(value.reshape(BATCH, LEN_IN, N_HEADS, HEAD_DIM),
                            offaw[:, :, :256], aw, ref_pts)
        attnT = np.ascontiguousarray(attn.transpose(0, 2, 1))
        xbf = (x_full + asf(bo[layer])[None, None, :]).transpose(0, 2, 1)
        return (_shardT(attnT.astype(NPBF)),
                _shardT(np.ascontiguousarray(xbf).astype(NPBF)))

    # ---- launch 2: layer-0 BCD + layer-1 projections ----
    value = _unshardT([resA[c]["valT"] for c in range(NCORE)])
    offaw = _unshardT([resA[c]["offawT"] for c in range(NCORE)])
    attnTs, xbs = gather_attn(value, offaw, 0, src)
    in_maps = [{
        "attnT": attnTs[c], "xbT": xbs[c],
        "wpk1": wpk1[0], "wpk2": wpk2[0], "wpk3": wpk3,
        "rows": rows[0], "prm": prm[0], "posT": posTs[c],
    } for c in range(NCORE)]
    resB = _run(_PROGS["BCDA"], in_maps)

    # x1 = g2*xn2 + be2 (host applies the folded affine)
    xn2 = _unshard_pm([resB[c]["x1n"] for c in range(NCORE)])
    x1 = xn2 * asf(g2[0])[None, None, :] + asf(be2[0])[None, None, :]

    # ---- launch 3: layer-1 BCD -> final ----
    val1 = _unshardT([resB[c]["valT"] for c in range(NCORE)])
    oa1 = _unshardT([resB[c]["offawT"] for c in range(NCORE)])
    attnTs, xbs = gather_attn(val1, oa1, 1, x1)
    in_maps = [{
        "attnT": attnTs[c], "xbT": xbs[c],
        "wpk1": wpk1[1], "wpk2": wpk2[1],
        "rows": rows[1], "prm": prm[1],
    } for c in range(NCORE)]
    resC = _run(_PROGS["BCD"], in_maps)

    # host LN2 + affine for the final layer
    r2 = _unshard_pm([resC[c]["out"] for c in range(NCORE)])
    m = r2.mean(-1, keepdims=True)
    v = np.square(r2 - m).mean(-1, keepdims=True)
    xn = (r2 - m) / np.sqrt(v + 1e-5)
    return (xn * asf(g2[1])[None, None, :]
            + asf(be2[1])[None, None, :]).astype(np.float32)


# revision 24
# speedup vs baseline: 1.1220x; 1.0284x over previous
"""Deformable-Transformer encoder on 8 trn2 NeuronCores — v4.

v3 + latency restructuring driven by NTFF traces:
  - Scratch-tile PE warmup at body start (no DMA dependency) so the HAM
    clock-gate is at 8/8 by the time real matmuls start; removes the
    wo-dependent warmup and the dummy transpose fillers.
  - Input DMAs packed (weights into 1-2 blobs) and spread across
    sync/scalar/vector/gpsimd queues so issue serialization (~0.65us per
    dma instr on one engine) stops gating the first matmul.
  - Drains paired: two 128-token tiles share one 2-slot PSUM tile, so
    PSUM->SBUF evacuation + bn_stats run at half the op count.
  - LN stats via bn_aggr (1 op) instead of a 9-op manual combine.
  - Final launch (BCD) skips LN2 entirely: it streams out the pre-LN2
    residual r2 per tile-pair and the host applies LN2+affine.
"""
import os
import sys
import types
import contextlib
import ctypes
import numpy as np

sys.path.insert(0, "/opt/trn_rl_repo")


def _install_ntff_hook():
    try:
        import antenv

        if hasattr(antenv, "axon_hooks"):
            return
        so_path = "/opt/axon/libaxon_pjrt.so"
        lib = ctypes.CDLL(so_path)
        if not hasattr(lib, "axon_start_nrt_profile"):
            hook = None
        else:
            lib.axon_start_nrt_profile.argtypes = [
                ctypes.POINTER(ctypes.c_int64), ctypes.c_size_t]
            lib.axon_start_nrt_profile.restype = ctypes.c_int64
            lib.axon_stop_nrt_profile.argtypes = [ctypes.c_char_p]
            lib.axon_stop_nrt_profile.restype = ctypes.c_int64

            @contextlib.contextmanager
            def hook(output_dir, device_ids):
                import jax
                jax.devices()
                if device_ids:
                    ids = (ctypes.c_int64 * len(device_ids))(*device_ids)
                    rc = lib.axon_start_nrt_profile(ids, len(device_ids))
                else:
                    rc = lib.axon_start_nrt_profile(None, 0)
                if rc != 0:
                    raise RuntimeError(f"start_nrt_profile rc={rc}")
                try:
                    yield
                finally:
                    lib.axon_stop_nrt_profile(str(output_dir).encode())

        m = types.ModuleType("antenv.axon_hooks")
        m.get_axon_ntff_profile_hook = lambda: hook
        m.set_axon_ntff_profile_hook = lambda h: None
        sys.modules["antenv.axon_hooks"] = m
        antenv.axon_hooks = m
    except Exception:
        pass


_install_ntff_hook()

import ml_dtypes  # noqa: E402
from concourse import bacc, tile, mybir, bass  # noqa: E402
from concourse.tile import add_dep_helper  # noqa: E402
from concourse.bass_utils import run_bass_kernel_spmd  # noqa: E402
from contextlib import ExitStack  # noqa: E402

F32 = mybir.dt.float32
BF16 = mybir.dt.bfloat16
NPBF = ml_dtypes.bfloat16
AF = mybir.ActivationFunctionType
ALU = mybir.AluOpType

SHAPES = ((64, 64), (32, 32), (16, 16), (8, 8))
LEVEL_STARTS = [0, 4096, 5120, 5376, 5440]
N_LEVELS, N_HEADS, N_POINTS = 4, 8, 4
D_MODEL, HEAD_DIM, D_FFN = 256, 32, 1024
LEN_IN, BATCH, NCORE = 5440, 2, 8
TPC = LEN_IN * BATCH // NCORE  # 1360 tokens per core
NT = 11                        # 128-token tiles per core
GROUPS = [(0, 512, range(0, 4)), (512, 512, range(4, 8)),
          (1024, 336, range(8, 11))]
PAIRS = [(0, 2), (2, 2), (4, 2), (6, 2), (8, 2), (10, 1)]
WARMUP_MM = 16

HW_EXEC_NS = []
LAST_RES = []
_PROGS = {}


def _nc():
    return bacc.Bacc("TRN2", target_bir_lowering=False, debug=False,
                     num_devices=NCORE)


def _tsz(ti):
    return min(128, TPC - ti * 128)


def _ccn(d):
    return d.rearrange("(c p) n -> p c n", p=128)


def _tchunks(step):
    out = []
    t0 = 0
    while t0 < TPC:
        out.append((t0, min(step, TPC - t0)))
        t0 += step
    return out


def _warmup(nc, sb, ps, ps_tag, bufs=2):
    """HAM warmup: dense matmuls on a memset scratch tile, no DMA deps.

    Returns the matmul handles so input DMAs can be staged against
    warmup progress (issue later ones only once earlier transfers have
    had the HBM bandwidth to themselves for a while)."""
    wsc = sb.tile([128, 256], BF16, tag="wsc")
    nc.gpsimd.memset(wsc[:], 0.25)
    mms = []
    for _ in range(WARMUP_MM):
        pw = ps.tile([128, 256], F32, tag=ps_tag, bufs=bufs)
        mms.append(nc.tensor.matmul(pw[:], wsc[:, 0:128], wsc[:],
                                    start=True, stop=True))
    return mms


def _build_A():
    """Layer-0 projections, channel-major world."""
    nc = _nc()
    xT_d = nc.dram_tensor("xT", [D_MODEL, TPC], BF16, kind="ExternalInput").ap()
    qT_d = nc.dram_tensor("qT", [D_MODEL, TPC], BF16, kind="ExternalInput").ap()
    wpk_d = nc.dram_tensor("wpk", [128, 1280], BF16, kind="ExternalInput").ap()
    prm_d = nc.dram_tensor("prm", [128, 5], F32, kind="ExternalInput").ap()
    valT_d = nc.dram_tensor("valT", [256, TPC], BF16,
                            kind="ExternalOutput").ap()
    oaT_d = nc.dram_tensor("offawT", [384, TPC], BF16,
                           kind="ExternalOutput").ap()

    with tile.TileContext(nc) as tc, ExitStack() as ctx:
        sb = ctx.enter_context(tc.tile_pool(name="sb", bufs=1))
        ps = ctx.enter_context(tc.tile_pool(name="ps", bufs=1, space="PSUM"))
        ob = ctx.enter_context(tc.tile_pool(name="ob", bufs=1))

        wmms = _warmup(nc, sb, ps, "p", bufs=3)

        wpk = sb.tile([128, 1280], BF16, tag="wpk")
        nc.sync.dma_start(wpk[:], wpk_d[:])
        wv = wpk[:, 0:512].rearrange("p (c n) -> p c n", c=2)
        woa = wpk[:, 512:1280].rearrange("p (c n) -> p c n", c=2)
        prm = sb.tile([128, 5], F32, tag="prm")
        nc.gpsimd.dma_start(prm[:], prm_d[:])

        chunks = _tchunks(512)
        xcs, qcs = [], []
        # chunk-0 input DMAs up front; chunk 1 staged on mid-warmup so
        # chunk 0 has the HBM bandwidth to itself first
        for ci in range(2):
            xc = ob.tile([128, 2, 512], BF16, tag="xc", bufs=2)
            qc = ob.tile([128, 2, 512], BF16, tag="qc", bufs=2)
            t0, tsz = chunks[ci]
            d1 = nc.sync.dma_start(xc[:, :, :tsz],
                                   _ccn(xT_d)[:, :, t0:t0 + tsz])
            nc.sync.dma_start(qc[:, :, :tsz], _ccn(qT_d)[:, :, t0:t0 + tsz])
            if ci == 1:
                add_dep_helper(d1.ins, wmms[6].ins, sync=True,
                               reason="stage chunk-1 dma")
            xcs.append(xc)
            qcs.append(qc)

        for ci, (t0, tsz) in enumerate(chunks):
            xc, qc = xcs[ci], qcs[ci]
            vsb = ob.tile([128, 2, 512], BF16, tag="vsb", bufs=2)
            osb = ob.tile([128, 3, 512], BF16, tag="osb", bufs=2)
            first_mm = None
            for m in range(5):  # 0-1: val (from x), 2-4: offaw (from q)
                src = xc if m < 2 else qc
                w = wv if m < 2 else woa
                mm = m if m < 2 else m - 2
                p = ps.tile([128, 512], F32, tag="p", bufs=3)
                for k in range(2):
                    mi = nc.tensor.matmul(
                        p[:, :tsz], w[:, k, mm * 128:mm * 128 + 128],
                        src[:, k, :tsz], start=(k == 0), stop=(k == 1))
                    if first_mm is None:
                        first_mm = mi
                dst = (vsb if m < 2 else osb)[:, mm, :tsz]
                if m % 2 == 0:
                    nc.scalar.activation(dst, p[:, :tsz], AF.Identity,
                                         bias=prm[:, m:m + 1])
                else:
                    nc.vector.tensor_scalar(dst, p[:, :tsz], prm[:, m:m + 1],
                                            None, ALU.add)
            if ci + 2 < len(chunks):
                t1, tsz1 = chunks[ci + 2]
                xn = ob.tile([128, 2, 512], BF16, tag="xc", bufs=2)
                qn = ob.tile([128, 2, 512], BF16, tag="qc", bufs=2)
                d1 = nc.sync.dma_start(xn[:, :, :tsz1],
                                       _ccn(xT_d)[:, :, t1:t1 + tsz1])
                nc.sync.dma_start(qn[:, :, :tsz1],
                                  _ccn(qT_d)[:, :, t1:t1 + tsz1])
                add_dep_helper(d1.ins, first_mm.ins, sync=True,
                               reason="defer chunk dma")
                xcs.append(xn)
                qcs.append(qn)
            nc.scalar.dma_start(_ccn(valT_d)[:, :, t0:t0 + tsz],
                                vsb[:, :, :tsz])
            nc.sync.dma_start(
                oaT_d.rearrange("(c p) n -> p c n", p=128)[:, :, t0:t0 + tsz],
                osb[:, :, :tsz])
    nc.compile()
    return nc


def _build_BCDA(with_A):
    """Fused out-proj + LN1 + FFN (+ LN2 + next-layer projections).

    with_A=True (layer 0): outputs x1n (pre-affine LN2), valT, offawT.
    with_A=False (layer 1): outputs out = r2 (pre-LN2 residual); the host
    applies LN2 + g2/be2.
    """
    nc = _nc()
    aT_d = nc.dram_tensor("attnT", [D_MODEL, TPC], BF16,
                          kind="ExternalInput").ap()
    xbT_d = nc.dram_tensor("xbT", [D_MODEL, TPC], BF16,
                           kind="ExternalInput").ap()
    # wpk1: wo (2x256) | ident (128)
    wpk1_d = nc.dram_tensor("wpk1", [128, 640], BF16,
                            kind="ExternalInput").ap()
    # wpk2: wl1 (2x1024) | dg1 (2x256) | wl2 (8x256)
    wpk2_d = nc.dram_tensor("wpk2", [128, 4608], BF16,
                            kind="ExternalInput").ap()
    rows_d = nc.dram_tensor("rows", [1, 256], BF16, kind="ExternalInput").ap()
    prm_d = nc.dram_tensor("prm", [128, 18], F32, kind="ExternalInput").ap()
    if with_A:
        posT_d = nc.dram_tensor("posT", [D_MODEL, TPC], BF16,
                                kind="ExternalInput").ap()
        # wpk3: wv (2x256) | woa (2x384)
        wpk3_d = nc.dram_tensor("wpk3", [128, 1280], BF16,
                                kind="ExternalInput").ap()
        x1n_d = nc.dram_tensor("x1n", [128, NT * 256], BF16,
                               kind="ExternalOutput").ap()
        valT_d = nc.dram_tensor("valT", [256, TPC], BF16,
                                kind="ExternalOutput").ap()
        oaT_d = nc.dram_tensor("offawT", [384, TPC], BF16,
                               kind="ExternalOutput").ap()
    else:
        out_d = nc.dram_tensor("out", [128, NT * 256], BF16,
                               kind="ExternalOutput").ap()

    with tile.TileContext(nc) as tc, ExitStack() as ctx:
        sb = ctx.enter_context(tc.tile_pool(name="sb", bufs=1))
        ps = ctx.enter_context(tc.tile_pool(name="ps", bufs=1, space="PSUM"))
        ob = ctx.enter_context(tc.tile_pool(name="ob", bufs=1))

        wmms = _warmup(nc, sb, ps, "pb")

        # ---- input DMAs: critical ones up front, bulk deferred ----
        wpk1 = sb.tile([128, 640], BF16, tag="wpk1")
        nc.sync.dma_start(wpk1[:], wpk1_d[:])
        wo = wpk1[:, 0:512].rearrange("p (c n) -> p c n", c=2)
        idn = wpk1[:, 512:640]
        aT = sb.tile([128, 2, TPC], BF16, tag="aT")
        nc.sync.dma_start(aT[:, :, 0:512], _ccn(aT_d)[:, :, 0:512])
        d = nc.sync.dma_start(aT[:, :, 512:TPC], _ccn(aT_d)[:, :, 512:TPC])
        add_dep_helper(d.ins, wmms[5].ins, sync=True, reason="stage aT1")
        xbT = sb.tile([128, 2, TPC], BF16, tag="xbT")
        nc.scalar.dma_start(xbT[:, :, 0:512], _ccn(xbT_d)[:, :, 0:512])
        d = nc.scalar.dma_start(xbT[:, :, 512:TPC], _ccn(xbT_d)[:, :, 512:TPC])
        add_dep_helper(d.ins, wmms[5].ins, sync=True, reason="stage xbT1")
        # force the sqrt ACT table set resident before the LN stats chain
        sqd = sb.tile([128, 1], F32, tag="sqd")
        nc.gpsimd.memset(sqd[:], 1.0)
        nc.scalar.activation(sqd[:, 0:1], sqd[:, 0:1], AF.Sqrt)
        # wpk2/posT/wpk3 tiles declared now, DMAs emitted inside sweep 1
        # gated on PE progress so they don't steal HBM bandwidth from aT/xbT
        wpk2 = sb.tile([128, 4608], BF16, tag="wpk2")
        wl1 = wpk2[:, 0:2048].rearrange("p (c n) -> p c n", c=2)
        dg1 = wpk2[:, 2048:2560].rearrange("p (c n) -> p c n", c=2)
        wl2 = wpk2[:, 2560:4608].rearrange("p (c n) -> p c n", c=8)
        prm = sb.tile([128, 18], F32, tag="prm")
        nc.gpsimd.dma_start(prm[:], prm_d[:])
        rows = sb.tile([1, 256], BF16, tag="rows")
        nc.gpsimd.dma_start(rows[:], rows_d[:])
        if with_A:
            posT = sb.tile([128, 2, TPC], BF16, tag="posT")
            wpk3 = sb.tile([128, 1280], BF16, tag="wpk3")
            wv = wpk3[:, 0:512].rearrange("p (c n) -> p c n", c=2)
            woa = wpk3[:, 512:1280].rearrange("p (c n) -> p c n", c=2)
            valTs = sb.tile([128, 2, TPC], BF16, tag="valTs")
            oaTs = sb.tile([128, 3, TPC], BF16, tag="oaTs")
            q1Ts = sb.tile([128, 2, TPC], BF16, tag="q1Ts")
        ones = sb.tile([1, 512], BF16, tag="ones")
        nc.gpsimd.memset(ones[:], 1.0)

        # persistent intermediates
        r1a = sb.tile([128, NT, 256], F32, tag="r1a")
        xnTa = sb.tile([128, 2, TPC], BF16, tag="xnTa")
        hta = sb.tile([128, 8, TPC], BF16, tag="hta")
        xna = sb.tile([128, NT, 256], BF16, tag="xna")
        bst1 = sb.tile([128, NT, 6], F32, tag="bst1")
        mv1 = sb.tile([128, NT, 2], F32, tag="mv1")
        st1r = sb.tile([128, NT, 1], F32, tag="st1r", name="st1r")
        st1n = sb.tile([128, NT, 1], F32, tag="st1n", name="st1n")
        if with_A:
            r2a = sb.tile([128, NT, 256], F32, tag="r2a")
            xn2Ta = sb.tile([128, 2, TPC], BF16, tag="xn2Ta")
            xout = sb.tile([128, NT, 256], BF16, tag="xout")
            bst2 = sb.tile([128, NT, 6], F32, tag="bst2")
            mv2 = sb.tile([128, NT, 2], F32, tag="mv2")
            st2r = sb.tile([128, NT, 1], F32, tag="st2r", name="st2r")
            st2n = sb.tile([128, NT, 1], F32, tag="st2n", name="st2n")
        else:
            r2b = sb.tile([128, NT, 256], BF16, tag="r2b")

        def stats_chain(mv, str_, stn, h0, h1, eng):
            """mv[:, h0:h1] = (mean, var) -> str_=rstd, stn=-mean*rstd."""
            sd = ob.tile([128, NT, 1], F32, tag="sd", bufs=2)
            nc.scalar.activation(sd[:, h0:h1, :], mv[:, h0:h1, 1:2], AF.Sqrt,
                                 bias=prm[:, 0:1])
            nc.vector.reciprocal(str_[:, h0:h1, :], sd[:, h0:h1, :])
            eng.scalar_tensor_tensor(stn[:, h0:h1, :], mv[:, h0:h1, 0:1],
                                     -1.0, str_[:, h0:h1, :],
                                     op0=ALU.mult, op1=ALU.mult)

        # ---- sweep 1: B matmul + residual, paired drains + LN1 stats ----
        for pi, (t0i, np_) in enumerate(PAIRS):
            pbp = ps.tile([128, 2, 256], F32, tag="pb", bufs=2)
            first_mm = None
            for j in range(np_):
                ti = t0i + j
                sz = _tsz(ti)
                t0 = ti * 128
                for k in range(2):
                    mi = nc.tensor.matmul(pbp[:sz, j, :],
                                          aT[:, k, t0:t0 + sz],
                                          wo[:, k, :], start=(k == 0),
                                          stop=False)
                    if first_mm is None:
                        first_mm = mi
                for k in range(2):
                    nc.tensor.matmul(pbp[:sz, j, k * 128:k * 128 + 128],
                                     xbT[:, k, t0:t0 + sz], idn[:, :],
                                     start=False, stop=(k == 1),
                                     skip_group_check=True)
            if pi == 0:
                d = nc.sync.dma_start(wpk2[:], wpk2_d[:])
                add_dep_helper(d.ins, first_mm.ins, sync=True,
                               reason="defer wpk2 dma")
            elif pi == 2 and with_A:
                d = nc.sync.dma_start(posT[:], _ccn(posT_d))
                nc.sync.dma_start(wpk3[:], wpk3_d[:])
                add_dep_helper(d.ins, first_mm.ins, sync=True,
                               reason="defer posT/wpk3 dma")
            sz0 = _tsz(t0i + np_ - 1)
            if np_ == 2:
                if pi % 2 == 0:
                    nc.scalar.copy(r1a[:, t0i:t0i + 2, :], pbp[:, :, :])
                else:
                    nc.vector.tensor_copy(r1a[:, t0i:t0i + 2, :], pbp[:, :, :])
            else:
                nc.vector.tensor_copy(r1a[:sz0, t0i, :], pbp[:sz0, 0, :])
            for j in range(np_):
                ti = t0i + j
                sz = _tsz(ti)
                nc.vector.bn_stats(bst1[:sz, ti, :], r1a[:sz, ti, :])
                nc.vector.bn_aggr(mv1[:sz, ti, :], bst1[:sz, ti, :])
            stats_chain(mv1, st1r, st1n, t0i, t0i + np_, nc.vector)

        # ---- sweep 2: LN1 apply, transpose, C, D (+ LN2 stats) ----
        for gi, (g0, gsz, tis) in enumerate(GROUPS):
            for t0i, np_ in PAIRS:
                if t0i not in tis:
                    continue
                pt = ps.tile([128, 2, 2, 128], BF16, tag="ptr", bufs=2)
                for j in range(np_):
                    ti = t0i + j
                    sz = _tsz(ti)
                    nc.scalar.activation(xna[:sz, ti, :], r1a[:sz, ti, :],
                                         AF.Identity,
                                         bias=st1n[:sz, ti, :],
                                         scale=st1r[:sz, ti, :])
                    for c in range(2):
                        nc.tensor.transpose(
                            pt[:, j, c, :sz],
                            xna[:sz, ti, c * 128:c * 128 + 128],
                            idn[:sz, :sz])
                t0 = t0i * 128
                tw = sum(_tsz(t0i + j) for j in range(np_))
                if np_ == 2:
                    src = pt.transpose((0, 2, 1, 3))
                    dst = xnTa[:, :, t0:t0 + 256].rearrange(
                        "p c (a b) -> p c a b", a=2)
                    if t0i % 4 == 0:
                        nc.vector.tensor_copy(dst, src)
                    else:
                        nc.scalar.copy(dst, src)
                else:
                    nc.vector.tensor_copy(xnTa[:, :, t0:t0 + tw],
                                          pt[:, 0, :, :tw])
            # C over the whole group: hT = relu(Wl1g.T @ xnT + bl1row)
            for m in range(8):
                pc = ps.tile([128, 512], F32, tag="pca", bufs=2)
                for k in range(2):
                    nc.tensor.matmul(pc[:, :gsz],
                                     wl1[:, k, m * 128:m * 128 + 128],
                                     xnTa[:, k, g0:g0 + gsz],
                                     start=(k == 0), stop=(k == 1))
                if m % 2 == 0:
                    nc.scalar.activation(hta[:, m, g0:g0 + gsz], pc[:, :gsz],
                                         AF.Relu, bias=prm[:, 5 + m:6 + m])
                else:
                    nc.vector.tensor_scalar(hta[:, m, g0:g0 + gsz],
                                            pc[:, :gsz], prm[:, 5 + m:6 + m],
                                            0.0, ALU.add, ALU.max)
            # D, paired into 2-slot PSUM tiles
            for t0i, np_ in PAIRS:
                if t0i not in tis:
                    continue
                pdp = ps.tile([128, 2, 256], F32, tag="pd", bufs=2)
                for j in range(np_):
                    ti = t0i + j
                    sz = _tsz(ti)
                    t0 = ti * 128
                    for k in range(8):
                        nc.tensor.matmul(pdp[:sz, j, :], hta[:, k, t0:t0 + sz],
                                         wl2[:, k, :],
                                         start=(k == 0), stop=False)
                    for k in range(2):
                        nc.tensor.matmul(pdp[:sz, j, :],
                                         xnTa[:, k, t0:t0 + sz],
                                         dg1[:, k, :], start=False, stop=False)
                    nc.tensor.matmul(pdp[:sz, j, :], ones[0:1, :sz],
                                     rows[:, :], start=False, stop=True)
                sz0 = _tsz(t0i + np_ - 1)
                if with_A:
                    if np_ == 2:
                        nc.vector.tensor_copy(r2a[:, t0i:t0i + 2, :],
                                              pdp[:, :, :])
                    else:
                        nc.vector.tensor_copy(r2a[:sz0, t0i, :],
                                              pdp[:sz0, 0, :])
                    for j in range(np_):
                        ti = t0i + j
                        sz = _tsz(ti)
                        nc.vector.bn_stats(bst2[:sz, ti, :], r2a[:sz, ti, :])
                        nc.vector.bn_aggr(mv2[:sz, ti, :], bst2[:sz, ti, :])
                    stats_chain(mv2, st2r, st2n, t0i, t0i + np_, nc.vector)
                else:
                    if np_ == 2:
                        if t0i % 4 == 0:
                            nc.vector.tensor_copy(r2b[:, t0i:t0i + 2, :],
                                                  pdp[:, :, :])
                        else:
                            nc.scalar.copy(r2b[:, t0i:t0i + 2, :],
                                           pdp[:, :, :])
                        dma_eng = nc.sync if t0i % 4 == 0 else nc.scalar
                        dma_eng.dma_start(
                            out_d[:, t0i * 256:(t0i + 2) * 256],
                            r2b[:, t0i:t0i + 2, :])
                    else:
                        nc.vector.tensor_copy(r2b[:sz0, t0i, :],
                                              pdp[:sz0, 0, :])
                        nc.sync.dma_start(
                            out_d[:, t0i * 256:(t0i + 1) * 256],
                            r2b[:, t0i, :])


        # ---- sweep 3 (with_A): LN2 apply + next-layer projections ----
        if with_A:
            for g0, gsz, tis in GROUPS:
                for t0i, np_ in PAIRS:
                    if t0i not in tis:
                        continue
                    pt2 = ps.tile([128, 2, 2, 128], BF16, tag="ptr", bufs=2)
                    for j in range(np_):
                        ti = t0i + j
                        sz = _tsz(ti)
                        nc.scalar.activation(xout[:sz, ti, :], r2a[:sz, ti, :],
                                             AF.Identity,
                                             bias=st2n[:sz, ti, :],
                                             scale=st2r[:sz, ti, :])
                        for c in range(2):
                            nc.tensor.transpose(
                                pt2[:, j, c, :sz],
                                xout[:sz, ti, c * 128:c * 128 + 128],
                                idn[:sz, :sz])
                    t0 = t0i * 128
                    tw = sum(_tsz(t0i + j) for j in range(np_))
                    if np_ == 2:
                        src = pt2.transpose((0, 2, 1, 3))
                        dst = xn2Ta[:, :, t0:t0 + 256].rearrange(
                            "p c (a b) -> p c a b", a=2)
                        nc.scalar.copy(dst, src)
                        # q1T = g2*xn2T + (be2+pos)T, fused into the drain
                        for c in range(2):
                            nc.vector.scalar_tensor_tensor(
                                q1Ts[:, c, t0:t0 + 256].rearrange(
                                    "p (a b) -> p a b", a=2),
                                pt2[:, :, c, :], prm[:, 1 + c:2 + c],
                                posT[:, c, t0:t0 + 256].rearrange(
                                    "p (a b) -> p a b", a=2),
                                op0=ALU.mult, op1=ALU.add)
                    else:
                        nc.scalar.copy(xn2Ta[:, :, t0:t0 + tw],
                                       pt2[:, 0, :, :tw])
                        for c in range(2):
                            nc.vector.scalar_tensor_tensor(
                                q1Ts[:, c, t0:t0 + tw],
                                pt2[:, 0, c, :tw], prm[:, 1 + c:2 + c],
                                posT[:, c, t0:t0 + tw],
                                op0=ALU.mult, op1=ALU.add)
                # val projections first (no q dependency)
                for m in range(2):
                    pa = ps.tile([128, 512], F32, tag="pca", bufs=2)
                    for k in range(2):
                        nc.tensor.matmul(pa[:, :gsz],
                                         wv[:, k, m * 128:m * 128 + 128],
                                         xn2Ta[:, k, g0:g0 + gsz],
                                         start=(k == 0), stop=(k == 1))
                    dst = valTs[:, m, g0:g0 + gsz]
                    if m % 2 == 0:
                        nc.scalar.activation(dst, pa[:, :gsz], AF.Identity,
                                             bias=prm[:, 13 + m:14 + m])
                    else:
                        nc.vector.tensor_scalar(dst, pa[:, :gsz],
                                                prm[:, 13 + m:14 + m],
                                                None, ALU.add)
                for m in range(3):
                    pa = ps.tile([128, 512], F32, tag="pca", bufs=2)
                    for k in range(2):
                        nc.tensor.matmul(pa[:, :gsz],
                                         woa[:, k, m * 128:m * 128 + 128],
                                         q1Ts[:, k, g0:g0 + gsz],
                                         start=(k == 0), stop=(k == 1))
                    dst = oaTs[:, m, g0:g0 + gsz]
                    if m % 2 == 1:
                        nc.scalar.activation(dst, pa[:, :gsz], AF.Identity,
                                             bias=prm[:, 15 + m:16 + m])
                    else:
                        nc.vector.tensor_scalar(dst, pa[:, :gsz],
                                                prm[:, 15 + m:16 + m],
                                                None, ALU.add)
                # output DMAs per group
                lo, hi = tis[0], tis[-1] + 1
                nc.scalar.dma_start(_ccn(valT_d)[:, :, g0:g0 + gsz],
                                    valTs[:, :, g0:g0 + gsz])
                nc.sync.dma_start(
                    oaT_d.rearrange("(c p) n -> p c n", p=128)[:, :,
                                                              g0:g0 + gsz],
                    oaTs[:, :, g0:g0 + gsz])
                nc.gpsimd.dma_start(
                    x1n_d[:, lo * 256:hi * 256], xout[:, lo:hi, :])
    nc.compile()
    return nc


def _run(prog, in_maps):
    trace = bool(os.environ.get("BASS_TRACE"))
    res = run_bass_kernel_spmd(prog, in_maps, core_ids=list(range(NCORE)),
                               trace=trace)
    if res.exec_time_ns:
        HW_EXEC_NS.append(res.exec_time_ns)
    if trace:
        LAST_RES.append(res)
    return res.results


def _bf(a):
    return np.ascontiguousarray(np.asarray(a, np.float32).astype(NPBF))


def _ccn_host(w):
    """[K, M] -> [128, K//128 * M] channel-major pack block."""
    w = np.asarray(w, np.float32)
    k, m = w.shape
    return w.reshape(k // 128, 128, m).transpose(1, 0, 2).reshape(128, -1)


def _chunked(v, nch):
    v = np.asarray(v, np.float32)
    return np.ascontiguousarray(v.reshape(nch, 128).T.astype(np.float32))


def _ref_points(valid_ratios):
    refs = []
    for lvl, (H, W) in enumerate(SHAPES):
        gy, gx = np.meshgrid(np.arange(H, dtype=np.float32) + 0.5,
                             np.arange(W, dtype=np.float32) + 0.5,
                             indexing="ij")
        ry = gy.reshape(-1)[None] / (valid_ratios[:, lvl, 1][:, None] * H)
        rx = gx.reshape(-1)[None] / (valid_ratios[:, lvl, 0][:, None] * W)
        refs.append(np.stack([rx, ry], -1))
    ref = np.concatenate(refs, 1)
    return ref[:, :, None, :] * valid_ratios[:, None]


def _host_sample(value, off, aw, ref_pts):
    N, Lq = off.shape[:2]
    off = off.reshape(N, Lq, N_HEADS, N_LEVELS, N_POINTS, 2)
    aw = aw.reshape(N, Lq, N_HEADS, N_LEVELS, N_POINTS)
    normalizer = np.array([[w, h] for h, w in SHAPES], np.float32)
    loc = (ref_pts[:, :, None, :, None, :]
           + off / normalizer[None, None, None, :, None, :])
    acc = np.zeros((N, N_HEADS, Lq, HEAD_DIM), np.float32)
    for lvl, (H, W) in enumerate(SHAPES):
        s = LEVEL_STARTS[lvl]
        val = value[:, s:s + H * W].transpose(0, 2, 1, 3)
        x = loc[:, :, :, lvl, :, 0] * W - 0.5
        y = loc[:, :, :, lvl, :, 1] * H - 0.5
        x0 = np.floor(x)
        y0 = np.floor(y)
        wx1 = x - x0
        wy1 = y - y0
        ix0 = x0.astype(np.int64)
        iy0 = y0.astype(np.int64)

        def corner(ix, iy, w):
            valid = (ix >= 0) & (ix < W) & (iy >= 0) & (iy < H)
            idx = np.clip(iy, 0, H - 1) * W + np.clip(ix, 0, W - 1)
            idx = idx.transpose(0, 2, 1, 3).reshape(N, N_HEADS, Lq * N_POINTS)
            g = np.take_along_axis(val, idx[..., None], axis=2)
            g = g.reshape(N, N_HEADS, Lq, N_POINTS, HEAD_DIM)
            w = np.where(valid, w, 0.0).transpose(0, 2, 1, 3)
            return g * w[..., None].astype(np.float32)

        sampled = (corner(ix0, iy0, (1 - wx1) * (1 - wy1))
                   + corner(ix0 + 1, iy0, wx1 * (1 - wy1))
                   + corner(ix0, iy0 + 1, (1 - wx1) * wy1)
                   + corner(ix0 + 1, iy0 + 1, wx1 * wy1))
        acc += (sampled * aw[:, :, :, lvl].transpose(0, 2, 1, 3)[..., None]
                ).sum(3)
    return acc.transpose(0, 2, 1, 3).reshape(N, Lq, D_MODEL)


def _shardT(fullT):
    return [np.ascontiguousarray(fullT[c // 4, :, (c % 4) * TPC:
                                       (c % 4 + 1) * TPC])
            for c in range(NCORE)]


def _unshardT(parts):
    F = parts[0].shape[0]
    out = np.empty((BATCH, LEN_IN, F), np.float32)
    for c in range(NCORE):
        out[c // 4, (c % 4) * TPC:(c % 4 + 1) * TPC] = \
            np.asarray(parts[c], np.float32).T
    return out


def _unshard_pm(parts):  # partition-major parts [128, NT*256]
    out = np.empty((BATCH, LEN_IN, 256), np.float32)
    for c in range(NCORE):
        a = np.asarray(parts[c], np.float32).reshape(128, NT, 256)
        a = a.transpose(1, 0, 2).reshape(NT * 128, 256)[:TPC]
        out[c // 4, (c % 4) * TPC:(c % 4 + 1) * TPC] = a
    return out


_IDENT = np.eye(128, dtype=np.float32)


def kernel(src, pos, valid_ratios, Wv, bv, Woff, boff, Wa, ba, Wo, bo,
           g1, be1, Wl1, bl1, Wl2, bl2, g2, be2):
    src = np.asarray(src, np.float32)
    pos = np.asarray(pos, np.float32)
    valid_ratios = np.asarray(valid_ratios, np.float32)
    asf = lambda a: np.asarray(a, np.float32)
    HW_EXEC_NS.clear()
    LAST_RES.clear()

    if "A" not in _PROGS:
        _PROGS["A"] = _build_A()
        _PROGS["BCDA"] = _build_BCDA(with_A=True)
        _PROGS["BCD"] = _build_BCDA(with_A=False)

    ref_pts = _ref_points(valid_ratios)

    Woa = [np.concatenate([asf(Woff[l]), asf(Wa[l])], axis=1)
           for l in range(2)]
    bva = [np.concatenate([asf(bv[l]), asf(boff[l]), asf(ba[l])])
           for l in range(2)]
    Wl1g = [asf(g1[l])[:, None] * asf(Wl1[l]) for l in range(2)]
    bl1f = [asf(bl1[l]) + asf(be1[l]) @ asf(Wl1[l]) for l in range(2)]
    # layer-1 value-proj with layer-0 g2/be2 folded in (q-path keeps
    # plain Woa; q is built on device as g2*xn2 + be2 + pos)
    Wv1f = asf(g2[0])[:, None] * asf(Wv[1])
    bva1f = np.concatenate([asf(bv[1]) + asf(be2[0]) @ asf(Wv[1]),
                            bva[1][256:]])
    cr = [asf(be1[l]) + asf(bl2[l]) for l in range(2)]
    rows = [np.ascontiguousarray(cr[l][None, :].astype(NPBF))
            for l in range(2)]
    prm = [np.concatenate([np.full((128, 1), 1e-5, np.float32),
                           _chunked(g2[l], 2), _chunked(be2[l], 2),
                           _chunked(bl1f[l], 8),
                           _chunked(bva1f if l == 0 else np.zeros(640), 5)],
                          axis=1) for l in range(2)]
    dg1 = [np.diag(asf(g1[l])) for l in range(2)]

    # packed weight blobs
    wpkA = np.concatenate([_ccn_host(Wv[0]), _ccn_host(Woa[0])],
                          axis=1).astype(NPBF)
    wpk1 = [np.concatenate([_ccn_host(Wo[l]), _IDENT],
                           axis=1).astype(NPBF) for l in range(2)]
    wpk2 = [np.concatenate([_ccn_host(Wl1g[l]), _ccn_host(dg1[l]),
                            _ccn_host(Wl2[l])], axis=1).astype(NPBF)
            for l in range(2)]
    wpk3 = np.concatenate([_ccn_host(Wv1f), _ccn_host(Woa[1])],
                          axis=1).astype(NPBF)

    xT = np.ascontiguousarray(src.transpose(0, 2, 1))
    qT = np.ascontiguousarray((src + pos).transpose(0, 2, 1))
    # q for layer-1 projections is g2*xn2 + (be2 + pos); fold be2 into pos
    posbT = np.ascontiguousarray(
        (pos + asf(be2[0])[None, None, :]).transpose(0, 2, 1))
    xTs = _shardT(xT.astype(NPBF))
    qTs = _shardT(qT.astype(NPBF))
    posTs = _shardT(posbT.astype(NPBF))

    # ---- launch 1: layer-0 projections ----
    in_maps = [{
        "xT": xTs[c], "qT": qTs[c],
        "wpk": wpkA, "prm": _chunked(bva[0], 5),
    } for c in range(NCORE)]
    resA = _run(_PROGS["A"], in_maps)

    def gather_attn(value, offaw, layer, x_full):
        aw = offaw[:, :, 256:].reshape(BATCH, LEN_IN, N_HEADS, 16)
        aw = aw - aw.max(-1, keepdims=True)
        e = np.exp(aw)
        aw = (e / e.sum(-1, keepdims=True)).reshape(BATCH, LEN_IN, 128)
        attn = _host_sample(value.reshape(BATCH, LEN_IN, N_HEADS, HEAD_DIM),
                            offaw[:, :, :256], aw, ref_pts)
        attnT = np.ascontiguousarray(attn.transpose(0, 2, 1))
        xbf = (x_full + asf(bo[layer])[None, None, :]).transpose(0, 2, 1)
        return (_shardT(attnT.astype(NPBF)),
                _shardT(np.ascontiguousarray(xbf).astype(NPBF)))

    # ---- launch 2: layer-0 BCD + layer-1 projections ----
    value = _unshardT([resA[c]["valT"] for c in range(NCORE)])
    offaw = _unshardT([resA[c]["offawT"] for c in range(NCORE)])
    attnTs, xbs = gather_attn(value, offaw, 0, src)
    in_maps = [{
        "attnT": attnTs[c], "xbT": xbs[c],
        "wpk1": wpk1[0], "wpk2": wpk2[0], "wpk3": wpk3,
        "rows": rows[0], "prm": prm[0], "posT": posTs[c],
    } for c in range(NCORE)]
    resB = _run(_PROGS["BCDA"], in_maps)

    # x1 = g2*xn2 + be2 (host applies the folded affine)
    xn2 = _unshard_pm([resB[c]["x1n"] for c in range(NCORE)])
    x1 = xn2 * asf(g2[0])[None, None, :] + asf(be2[0])[None, None, :]

    # ---- launch 3: layer-1 BCD -> final ----
    val1 = _unshardT([resB[c]["valT"] for c in range(NCORE)])
    oa1 = _unshardT([resB[c]["offawT"] for c in range(NCORE)])
    attnTs, xbs = gather_attn(val1, oa1, 1, x1)
    in_maps = [{
        "attnT": attnTs[c], "xbT": xbs[c],
        "wpk1": wpk1[1], "wpk2": wpk2[1],
        "rows": rows[1], "prm": prm[1],
    } for c in range(NCORE)]
    resC = _run(_PROGS["BCD"], in_maps)

    # host LN2 + affine for the final layer
    r2 = _unshard_pm([resC[c]["out"] for c in range(NCORE)])
    m = r2.mean(-1, keepdims=True)
    v = np.square(r2 - m).mean(-1, keepdims=True)
    xn = (r2 - m) / np.sqrt(v + 1e-5)
    return (xn * asf(g2[1])[None, None, :]
            + asf(be2[1])[None, None, :]).astype(np.float32)


# revision 25
# speedup vs baseline: 1.1541x; 1.0286x over previous
"""Deformable-Transformer encoder on 8 trn2 NeuronCores — v4.

v3 + latency restructuring driven by NTFF traces:
  - Scratch-tile PE warmup at body start (no DMA dependency) so the HAM
    clock-gate is at 8/8 by the time real matmuls start; removes the
    wo-dependent warmup and the dummy transpose fillers.
  - Input DMAs packed (weights into 1-2 blobs) and spread across
    sync/scalar/vector/gpsimd queues so issue serialization (~0.65us per
    dma instr on one engine) stops gating the first matmul.
  - Drains paired: two 128-token tiles share one 2-slot PSUM tile, so
    PSUM->SBUF evacuation + bn_stats run at half the op count.
  - LN stats via bn_aggr (1 op) instead of a 9-op manual combine.
  - Final launch (BCD) skips LN2 entirely: it streams out the pre-LN2
    residual r2 per tile-pair and the host applies LN2+affine.
"""
import os
import sys
import types
import contextlib
import ctypes
import numpy as np

sys.path.insert(0, "/opt/trn_rl_repo")


def _install_ntff_hook():
    try:
        import antenv

        if hasattr(antenv, "axon_hooks"):
            return
        so_path = "/opt/axon/libaxon_pjrt.so"
        lib = ctypes.CDLL(so_path)
        if not hasattr(lib, "axon_start_nrt_profile"):
            hook = None
        else:
            lib.axon_start_nrt_profile.argtypes = [
                ctypes.POINTER(ctypes.c_int64), ctypes.c_size_t]
            lib.axon_start_nrt_profile.restype = ctypes.c_int64
            lib.axon_stop_nrt_profile.argtypes = [ctypes.c_char_p]
            lib.axon_stop_nrt_profile.restype = ctypes.c_int64

            @contextlib.contextmanager
            def hook(output_dir, device_ids):
                import jax
                jax.devices()
                if device_ids:
                    ids = (ctypes.c_int64 * len(device_ids))(*device_ids)
                    rc = lib.axon_start_nrt_profile(ids, len(device_ids))
                else:
                    rc = lib.axon_start_nrt_profile(None, 0)
                if rc != 0:
                    raise RuntimeError(f"start_nrt_profile rc={rc}")
                try:
                    yield
                finally:
                    lib.axon_stop_nrt_profile(str(output_dir).encode())

        m = types.ModuleType("antenv.axon_hooks")
        m.get_axon_ntff_profile_hook = lambda: hook
        m.set_axon_ntff_profile_hook = lambda h: None
        sys.modules["antenv.axon_hooks"] = m
        antenv.axon_hooks = m
    except Exception:
        pass


_install_ntff_hook()

import ml_dtypes  # noqa: E402
from concourse import bacc, tile, mybir, bass  # noqa: E402
from concourse.tile import add_dep_helper  # noqa: E402
from concourse.bass_utils import run_bass_kernel_spmd  # noqa: E402
from contextlib import ExitStack  # noqa: E402

F32 = mybir.dt.float32
BF16 = mybir.dt.bfloat16
NPBF = ml_dtypes.bfloat16
AF = mybir.ActivationFunctionType
ALU = mybir.AluOpType

SHAPES = ((64, 64), (32, 32), (16, 16), (8, 8))
LEVEL_STARTS = [0, 4096, 5120, 5376, 5440]
N_LEVELS, N_HEADS, N_POINTS = 4, 8, 4
D_MODEL, HEAD_DIM, D_FFN = 256, 32, 1024
LEN_IN, BATCH, NCORE = 5440, 2, 8
TPC = LEN_IN * BATCH // NCORE  # 1360 tokens per core
NT = 11                        # 128-token tiles per core
GROUPS = [(0, 512, range(0, 4)), (512, 512, range(4, 8)),
          (1024, 336, range(8, 11))]
PAIRS = [(0, 2), (2, 2), (4, 2), (6, 2), (8, 2), (10, 1)]
WARMUP_MM = 16

HW_EXEC_NS = []
LAST_RES = []
_PROGS = {}


def _nc():
    return bacc.Bacc("TRN2", target_bir_lowering=False, debug=False,
                     num_devices=NCORE)


def _tsz(ti):
    return min(128, TPC - ti * 128)


def _ccn(d):
    return d.rearrange("(c p) n -> p c n", p=128)


def _tchunks(step):
    out = []
    t0 = 0
    while t0 < TPC:
        out.append((t0, min(step, TPC - t0)))
        t0 += step
    return out


def _warmup(nc, sb, ps, ps_tag, bufs=2):
    """HAM warmup: dense matmuls on a memset scratch tile, no DMA deps.

    Returns the matmul handles so input DMAs can be staged against
    warmup progress (issue later ones only once earlier transfers have
    had the HBM bandwidth to themselves for a while)."""
    wsc = sb.tile([128, 256], BF16, tag="wsc")
    nc.gpsimd.memset(wsc[:], 0.25)
    mms = []
    for _ in range(WARMUP_MM):
        pw = ps.tile([128, 256], F32, tag=ps_tag, bufs=bufs)
        mms.append(nc.tensor.matmul(pw[:], wsc[:, 0:128], wsc[:],
                                    start=True, stop=True))
    return mms


def _build_A():
    """Layer-0 projections, channel-major world."""
    nc = _nc()
    xT_d = nc.dram_tensor("xT", [D_MODEL, TPC], BF16, kind="ExternalInput").ap()
    qT_d = nc.dram_tensor("qT", [D_MODEL, TPC], BF16, kind="ExternalInput").ap()
    wpk_d = nc.dram_tensor("wpk", [128, 1280], BF16, kind="ExternalInput").ap()
    prm_d = nc.dram_tensor("prm", [128, 5], F32, kind="ExternalInput").ap()
    valT_d = nc.dram_tensor("valT", [256, TPC], BF16,
                            kind="ExternalOutput").ap()
    oaT_d = nc.dram_tensor("offawT", [384, TPC], BF16,
                           kind="ExternalOutput").ap()

    with tile.TileContext(nc) as tc, ExitStack() as ctx:
        sb = ctx.enter_context(tc.tile_pool(name="sb", bufs=1))
        ps = ctx.enter_context(tc.tile_pool(name="ps", bufs=1, space="PSUM"))
        ob = ctx.enter_context(tc.tile_pool(name="ob", bufs=1))

        wmms = _warmup(nc, sb, ps, "p", bufs=3)

        wpk = sb.tile([128, 1280], BF16, tag="wpk")
        nc.sync.dma_start(wpk[:], wpk_d[:])
        wv = wpk[:, 0:512].rearrange("p (c n) -> p c n", c=2)
        woa = wpk[:, 512:1280].rearrange("p (c n) -> p c n", c=2)
        prm = sb.tile([128, 5], F32, tag="prm")
        nc.gpsimd.dma_start(prm[:], prm_d[:])

        chunks = _tchunks(512)
        xcs, qcs = [], []
        # chunk-0 input DMAs up front; chunk 1 staged on mid-warmup so
        # chunk 0 has the HBM bandwidth to itself first
        for ci in range(2):
            xc = ob.tile([128, 2, 512], BF16, tag="xc", bufs=2)
            qc = ob.tile([128, 2, 512], BF16, tag="qc", bufs=2)
            t0, tsz = chunks[ci]
            d1 = nc.sync.dma_start(xc[:, :, :tsz],
                                   _ccn(xT_d)[:, :, t0:t0 + tsz])
            nc.sync.dma_start(qc[:, :, :tsz], _ccn(qT_d)[:, :, t0:t0 + tsz])
            if ci == 1:
                add_dep_helper(d1.ins, wmms[6].ins, sync=True,
                               reason="stage chunk-1 dma")
            xcs.append(xc)
            qcs.append(qc)

        for ci, (t0, tsz) in enumerate(chunks):
            xc, qc = xcs[ci], qcs[ci]
            vsb = ob.tile([128, 2, 512], BF16, tag="vsb", bufs=2)
            osb = ob.tile([128, 3, 512], BF16, tag="osb", bufs=2)
            first_mm = None
            for m in range(5):  # 0-1: val (from x), 2-4: offaw (from q)
                src = xc if m < 2 else qc
                w = wv if m < 2 else woa
                mm = m if m < 2 else m - 2
                p = ps.tile([128, 512], F32, tag="p", bufs=3)
                for k in range(2):
                    mi = nc.tensor.matmul(
                        p[:, :tsz], w[:, k, mm * 128:mm * 128 + 128],
                        src[:, k, :tsz], start=(k == 0), stop=(k == 1))
                    if first_mm is None:
                        first_mm = mi
                dst = (vsb if m < 2 else osb)[:, mm, :tsz]
                if m % 2 == 0:
                    nc.scalar.activation(dst, p[:, :tsz], AF.Identity,
                                         bias=prm[:, m:m + 1])
                else:
                    nc.vector.tensor_scalar(dst, p[:, :tsz], prm[:, m:m + 1],
                                            None, ALU.add)
            if ci + 2 < len(chunks):
                t1, tsz1 = chunks[ci + 2]
                xn = ob.tile([128, 2, 512], BF16, tag="xc", bufs=2)
                qn = ob.tile([128, 2, 512], BF16, tag="qc", bufs=2)
                d1 = nc.sync.dma_start(xn[:, :, :tsz1],
                                       _ccn(xT_d)[:, :, t1:t1 + tsz1])
                nc.sync.dma_start(qn[:, :, :tsz1],
                                  _ccn(qT_d)[:, :, t1:t1 + tsz1])
                add_dep_helper(d1.ins, first_mm.ins, sync=True,
                               reason="defer chunk dma")
                xcs.append(xn)
                qcs.append(qn)
            nc.scalar.dma_start(_ccn(valT_d)[:, :, t0:t0 + tsz],
                                vsb[:, :, :tsz])
            nc.sync.dma_start(
                oaT_d.rearrange("(c p) n -> p c n", p=128)[:, :, t0:t0 + tsz],
                osb[:, :, :tsz])
    nc.compile()
    return nc


def _build_BCDA(with_A):
    """Fused out-proj + LN1 + FFN (+ LN2 + next-layer projections).

    with_A=True (layer 0): outputs x1n (pre-affine LN2), valT, offawT.
    with_A=False (layer 1): outputs out = r2 (pre-LN2 residual); the host
    applies LN2 + g2/be2.
    """
    nc = _nc()
    aT_d = nc.dram_tensor("attnT", [D_MODEL, TPC], BF16,
                          kind="ExternalInput").ap()
    xbT_d = nc.dram_tensor("xbT", [D_MODEL, TPC], BF16,
                           kind="ExternalInput").ap()
    # wpk1: wo (2x256) | ident (128)
    wpk1_d = nc.dram_tensor("wpk1", [128, 640], BF16,
                            kind="ExternalInput").ap()
    # wpk2: wl1 (2x1024) | dg1 (2x256) | wl2 (8x256)
    wpk2_d = nc.dram_tensor("wpk2", [128, 4608], BF16,
                            kind="ExternalInput").ap()
    rows_d = nc.dram_tensor("rows", [1, 256], BF16, kind="ExternalInput").ap()
    prm_d = nc.dram_tensor("prm", [128, 18], F32, kind="ExternalInput").ap()
    if with_A:
        posT_d = nc.dram_tensor("posT", [D_MODEL, TPC], BF16,
                                kind="ExternalInput").ap()
        # wpk3: wv (2x256) | woa (2x384)
        wpk3_d = nc.dram_tensor("wpk3", [128, 1280], BF16,
                                kind="ExternalInput").ap()
        x1n_d = nc.dram_tensor("x1n", [128, NT * 256], BF16,
                               kind="ExternalOutput").ap()
        valT_d = nc.dram_tensor("valT", [256, TPC], BF16,
                                kind="ExternalOutput").ap()
        oaT_d = nc.dram_tensor("offawT", [384, TPC], BF16,
                               kind="ExternalOutput").ap()
    else:
        out_d = nc.dram_tensor("out", [128, NT * 256], BF16,
                               kind="ExternalOutput").ap()

    with tile.TileContext(nc) as tc, ExitStack() as ctx:
        sb = ctx.enter_context(tc.tile_pool(name="sb", bufs=1))
        ps = ctx.enter_context(tc.tile_pool(name="ps", bufs=1, space="PSUM"))
        ob = ctx.enter_context(tc.tile_pool(name="ob", bufs=1))

        wmms = _warmup(nc, sb, ps, "pb")

        # ---- input DMAs: critical ones up front, bulk deferred ----
        wpk1 = sb.tile([128, 640], BF16, tag="wpk1")
        nc.sync.dma_start(wpk1[:], wpk1_d[:])
        wo = wpk1[:, 0:512].rearrange("p (c n) -> p c n", c=2)
        idn = wpk1[:, 512:640]
        aT = sb.tile([128, 2, TPC], BF16, tag="aT")
        nc.sync.dma_start(aT[:, :, 0:512], _ccn(aT_d)[:, :, 0:512])
        d = nc.sync.dma_start(aT[:, :, 512:TPC], _ccn(aT_d)[:, :, 512:TPC])
        add_dep_helper(d.ins, wmms[5].ins, sync=True, reason="stage aT1")
        xbT = sb.tile([128, 2, TPC], BF16, tag="xbT")
        nc.scalar.dma_start(xbT[:, :, 0:512], _ccn(xbT_d)[:, :, 0:512])
        d = nc.scalar.dma_start(xbT[:, :, 512:TPC], _ccn(xbT_d)[:, :, 512:TPC])
        add_dep_helper(d.ins, wmms[5].ins, sync=True, reason="stage xbT1")
        # force the sqrt ACT table set resident before the LN stats chain
        sqd = sb.tile([128, 1], F32, tag="sqd")
        nc.gpsimd.memset(sqd[:], 1.0)
        nc.scalar.activation(sqd[:, 0:1], sqd[:, 0:1], AF.Sqrt)
        # wpk2/posT/wpk3 tiles declared now, DMAs emitted inside sweep 1
        # gated on PE progress so they don't steal HBM bandwidth from aT/xbT
        wpk2 = sb.tile([128, 4608], BF16, tag="wpk2")
        wl1 = wpk2[:, 0:2048].rearrange("p (c n) -> p c n", c=2)
        dg1 = wpk2[:, 2048:2560].rearrange("p (c n) -> p c n", c=2)
        wl2 = wpk2[:, 2560:4608].rearrange("p (c n) -> p c n", c=8)
        prm = sb.tile([128, 18], F32, tag="prm")
        nc.gpsimd.dma_start(prm[:], prm_d[:])
        rows = sb.tile([1, 256], BF16, tag="rows")
        nc.gpsimd.dma_start(rows[:], rows_d[:])
        if with_A:
            posT = sb.tile([128, 2, TPC], BF16, tag="posT")
            wpk3 = sb.tile([128, 1280], BF16, tag="wpk3")
            wv = wpk3[:, 0:512].rearrange("p (c n) -> p c n", c=2)
            woa = wpk3[:, 512:1280].rearrange("p (c n) -> p c n", c=2)
            valTs = sb.tile([128, 2, TPC], BF16, tag="valTs")
            oaTs = sb.tile([128, 3, TPC], BF16, tag="oaTs")
            q1Ts = sb.tile([128, 2, TPC], BF16, tag="q1Ts")
        ones = sb.tile([1, 512], BF16, tag="ones")
        nc.gpsimd.memset(ones[:], 1.0)

        # persistent intermediates
        r1a = sb.tile([128, NT, 256], BF16, tag="r1a")
        xnTa = sb.tile([128, 2, TPC], BF16, tag="xnTa")
        hta = sb.tile([128, 8, TPC], BF16, tag="hta")
        xna = sb.tile([128, NT, 256], BF16, tag="xna")
        bst1 = sb.tile([128, NT, 6], F32, tag="bst1")
        mv1 = sb.tile([128, NT, 2], F32, tag="mv1")
        st1r = sb.tile([128, NT, 1], F32, tag="st1r", name="st1r")
        st1n = sb.tile([128, NT, 1], F32, tag="st1n", name="st1n")
        if with_A:
            r2a = sb.tile([128, NT, 256], BF16, tag="r2a")
            xn2Ta = sb.tile([128, 2, TPC], BF16, tag="xn2Ta")
            xout = sb.tile([128, NT, 256], BF16, tag="xout")
            bst2 = sb.tile([128, NT, 6], F32, tag="bst2")
            mv2 = sb.tile([128, NT, 2], F32, tag="mv2")
            st2r = sb.tile([128, NT, 1], F32, tag="st2r", name="st2r")
            st2n = sb.tile([128, NT, 1], F32, tag="st2n", name="st2n")
        else:
            r2b = sb.tile([128, NT, 256], BF16, tag="r2b")

        def stats_chain(mv, str_, stn, h0, h1, eng):
            """mv[:, h0:h1] = (mean, var) -> str_=rstd, stn=-mean*rstd."""
            sd = ob.tile([128, NT, 1], F32, tag="sd", bufs=2)
            nc.scalar.activation(sd[:, h0:h1, :], mv[:, h0:h1, 1:2], AF.Sqrt,
                                 bias=prm[:, 0:1])
            nc.vector.reciprocal(str_[:, h0:h1, :], sd[:, h0:h1, :])
            eng.scalar_tensor_tensor(stn[:, h0:h1, :], mv[:, h0:h1, 0:1],
                                     -1.0, str_[:, h0:h1, :],
                                     op0=ALU.mult, op1=ALU.mult)

        # ---- sweep 1: B matmul + residual, paired drains + LN1 stats ----
        for pi, (t0i, np_) in enumerate(PAIRS):
            pbp = ps.tile([128, 2, 256], F32, tag="pb", bufs=2)
            first_mm = None
            for j in range(np_):
                ti = t0i + j
                sz = _tsz(ti)
                t0 = ti * 128
                for k in range(2):
                    mi = nc.tensor.matmul(pbp[:sz, j, :],
                                          aT[:, k, t0:t0 + sz],
                                          wo[:, k, :], start=(k == 0),
                                          stop=False)
                    if first_mm is None:
                        first_mm = mi
                for k in range(2):
                    nc.tensor.matmul(pbp[:sz, j, k * 128:k * 128 + 128],
                                     xbT[:, k, t0:t0 + sz], idn[:, :],
                                     start=False, stop=(k == 1),
                                     skip_group_check=True)
            if pi == 0:
                d = nc.sync.dma_start(wpk2[:], wpk2_d[:])
                add_dep_helper(d.ins, first_mm.ins, sync=True,
                               reason="defer wpk2 dma")
            elif pi == 2 and with_A:
                d = nc.sync.dma_start(posT[:], _ccn(posT_d))
                nc.sync.dma_start(wpk3[:], wpk3_d[:])
                add_dep_helper(d.ins, first_mm.ins, sync=True,
                               reason="defer posT/wpk3 dma")
            sz0 = _tsz(t0i + np_ - 1)
            if np_ == 2:
                if pi % 2 == 0:
                    nc.scalar.copy(r1a[:, t0i:t0i + 2, :], pbp[:, :, :])
                else:
                    nc.vector.tensor_copy(r1a[:, t0i:t0i + 2, :], pbp[:, :, :])
            else:
                nc.vector.tensor_copy(r1a[:sz0, t0i, :], pbp[:sz0, 0, :])
            for j in range(np_):
                ti = t0i + j
                sz = _tsz(ti)
                nc.vector.bn_stats(bst1[:sz, ti, :], r1a[:sz, ti, :])
                nc.vector.bn_aggr(mv1[:sz, ti, :], bst1[:sz, ti, :])
            stats_chain(mv1, st1r, st1n, t0i, t0i + np_, nc.vector)

        # ---- sweep 2: LN1 apply, transpose, C, D (+ LN2 stats) ----
        for gi, (g0, gsz, tis) in enumerate(GROUPS):
            for t0i, np_ in PAIRS:
                if t0i not in tis:
                    continue
                pt = ps.tile([128, 2, 2, 128], BF16, tag="ptr", bufs=2)
                for j in range(np_):
                    ti = t0i + j
                    sz = _tsz(ti)
                    nc.scalar.activation(xna[:sz, ti, :], r1a[:sz, ti, :],
                                         AF.Identity,
                                         bias=st1n[:sz, ti, :],
                                         scale=st1r[:sz, ti, :])
                    for c in range(2):
                        nc.tensor.transpose(
                            pt[:, j, c, :sz],
                            xna[:sz, ti, c * 128:c * 128 + 128],
                            idn[:sz, :sz])
                t0 = t0i * 128
                tw = sum(_tsz(t0i + j) for j in range(np_))
                if np_ == 2:
                    src = pt.transpose((0, 2, 1, 3))
                    dst = xnTa[:, :, t0:t0 + 256].rearrange(
                        "p c (a b) -> p c a b", a=2)
                    if t0i % 4 == 0:
                        nc.vector.tensor_copy(dst, src)
                    else:
                        nc.scalar.copy(dst, src)
                else:
                    nc.vector.tensor_copy(xnTa[:, :, t0:t0 + tw],
                                          pt[:, 0, :, :tw])
            # C over the whole group: hT = relu(Wl1g.T @ xnT + bl1row)
            for m in range(8):
                pc = ps.tile([128, 512], F32, tag="pca", bufs=2)
                for k in range(2):
                    nc.tensor.matmul(pc[:, :gsz],
                                     wl1[:, k, m * 128:m * 128 + 128],
                                     xnTa[:, k, g0:g0 + gsz],
                                     start=(k == 0), stop=(k == 1))
                if m % 2 == 0:
                    nc.scalar.activation(hta[:, m, g0:g0 + gsz], pc[:, :gsz],
                                         AF.Relu, bias=prm[:, 5 + m:6 + m])
                else:
                    nc.vector.tensor_scalar(hta[:, m, g0:g0 + gsz],
                                            pc[:, :gsz], prm[:, 5 + m:6 + m],
                                            0.0, ALU.add, ALU.max)
            # D, paired into 2-slot PSUM tiles
            for t0i, np_ in PAIRS:
                if t0i not in tis:
                    continue
                pdp = ps.tile([128, 2, 256], F32, tag="pd", bufs=2)
                for j in range(np_):
                    ti = t0i + j
                    sz = _tsz(ti)
                    t0 = ti * 128
                    for k in range(8):
                        nc.tensor.matmul(pdp[:sz, j, :], hta[:, k, t0:t0 + sz],
                                         wl2[:, k, :],
                                         start=(k == 0), stop=False)
                    for k in range(2):
                        nc.tensor.matmul(pdp[:sz, j, :],
                                         xnTa[:, k, t0:t0 + sz],
                                         dg1[:, k, :], start=False, stop=False)
                    nc.tensor.matmul(pdp[:sz, j, :], ones[0:1, :sz],
                                     rows[:, :], start=False, stop=True)
                sz0 = _tsz(t0i + np_ - 1)
                if with_A:
                    if np_ == 2:
                        nc.vector.tensor_copy(r2a[:, t0i:t0i + 2, :],
                                              pdp[:, :, :])
                    else:
                        nc.vector.tensor_copy(r2a[:sz0, t0i, :],
                                              pdp[:sz0, 0, :])
                    for j in range(np_):
                        ti = t0i + j
                        sz = _tsz(ti)
                        nc.vector.bn_stats(bst2[:sz, ti, :], r2a[:sz, ti, :])
                        nc.vector.bn_aggr(mv2[:sz, ti, :], bst2[:sz, ti, :])
                    stats_chain(mv2, st2r, st2n, t0i, t0i + np_, nc.vector)
                else:
                    if np_ == 2:
                        if t0i % 4 == 0:
                            nc.vector.tensor_copy(r2b[:, t0i:t0i + 2, :],
                                                  pdp[:, :, :])
                        else:
                            nc.scalar.copy(r2b[:, t0i:t0i + 2, :],
                                           pdp[:, :, :])
                        dma_eng = nc.sync if t0i % 4 == 0 else nc.scalar
                        dma_eng.dma_start(
                            out_d[:, t0i * 256:(t0i + 2) * 256],
                            r2b[:, t0i:t0i + 2, :])
                    else:
                        nc.vector.tensor_copy(r2b[:sz0, t0i, :],
                                              pdp[:sz0, 0, :])
                        nc.sync.dma_start(
                            out_d[:, t0i * 256:(t0i + 1) * 256],
                            r2b[:, t0i, :])


        # ---- sweep 3 (with_A): LN2 apply + next-layer projections ----
        if with_A:
            for g0, gsz, tis in GROUPS:
                for t0i, np_ in PAIRS:
                    if t0i not in tis:
                        continue
                    pt2 = ps.tile([128, 2, 2, 128], BF16, tag="ptr", bufs=2)
                    for j in range(np_):
                        ti = t0i + j
                        sz = _tsz(ti)
                        nc.scalar.activation(xout[:sz, ti, :], r2a[:sz, ti, :],
                                             AF.Identity,
                                             bias=st2n[:sz, ti, :],
                                             scale=st2r[:sz, ti, :])
                        for c in range(2):
                            nc.tensor.transpose(
                                pt2[:, j, c, :sz],
                                xout[:sz, ti, c * 128:c * 128 + 128],
                                idn[:sz, :sz])
                    t0 = t0i * 128
                    tw = sum(_tsz(t0i + j) for j in range(np_))
                    if np_ == 2:
                        src = pt2.transpose((0, 2, 1, 3))
                        dst = xn2Ta[:, :, t0:t0 + 256].rearrange(
                            "p c (a b) -> p c a b", a=2)
                        nc.scalar.copy(dst, src)
                        # q1T = g2*xn2T + (be2+pos)T, fused into the drain
                        for c in range(2):
                            nc.vector.scalar_tensor_tensor(
                                q1Ts[:, c, t0:t0 + 256].rearrange(
                                    "p (a b) -> p a b", a=2),
                                pt2[:, :, c, :], prm[:, 1 + c:2 + c],
                                posT[:, c, t0:t0 + 256].rearrange(
                                    "p (a b) -> p a b", a=2),
                                op0=ALU.mult, op1=ALU.add)
                    else:
                        nc.scalar.copy(xn2Ta[:, :, t0:t0 + tw],
                                       pt2[:, 0, :, :tw])
                        for c in range(2):
                            nc.vector.scalar_tensor_tensor(
                                q1Ts[:, c, t0:t0 + tw],
                                pt2[:, 0, c, :tw], prm[:, 1 + c:2 + c],
                                posT[:, c, t0:t0 + tw],
                                op0=ALU.mult, op1=ALU.add)
                # val projections first (no q dependency)
                for m in range(2):
                    pa = ps.tile([128, 512], F32, tag="pca", bufs=2)
                    for k in range(2):
                        nc.tensor.matmul(pa[:, :gsz],
                                         wv[:, k, m * 128:m * 128 + 128],
                                         xn2Ta[:, k, g0:g0 + gsz],
                                         start=(k == 0), stop=(k == 1))
                    dst = valTs[:, m, g0:g0 + gsz]
                    if m % 2 == 0:
                        nc.scalar.activation(dst, pa[:, :gsz], AF.Identity,
                                             bias=prm[:, 13 + m:14 + m])
                    else:
                        nc.vector.tensor_scalar(dst, pa[:, :gsz],
                                                prm[:, 13 + m:14 + m],
                                                None, ALU.add)
                for m in range(3):
                    pa = ps.tile([128, 512], F32, tag="pca", bufs=2)
                    for k in range(2):
                        nc.tensor.matmul(pa[:, :gsz],
                                         woa[:, k, m * 128:m * 128 + 128],
                                         q1Ts[:, k, g0:g0 + gsz],
                                         start=(k == 0), stop=(k == 1))
                    dst = oaTs[:, m, g0:g0 + gsz]
                    if m % 2 == 1:
                        nc.scalar.activation(dst, pa[:, :gsz], AF.Identity,
                                             bias=prm[:, 15 + m:16 + m])
                    else:
                        nc.vector.tensor_scalar(dst, pa[:, :gsz],
                                                prm[:, 15 + m:16 + m],
                                                None, ALU.add)
                # output DMAs per group
                lo, hi = tis[0], tis[-1] + 1
                nc.scalar.dma_start(_ccn(valT_d)[:, :, g0:g0 + gsz],
                                    valTs[:, :, g0:g0 + gsz])
                nc.sync.dma_start(
                    oaT_d.rearrange("(c p) n -> p c n", p=128)[:, :,
                                                              g0:g0 + gsz],
                    oaTs[:, :, g0:g0 + gsz])
                nc.gpsimd.dma_start(
                    x1n_d[:, lo * 256:hi * 256], xout[:, lo:hi, :])
    nc.compile()
    return nc


def _run(prog, in_maps):
    trace = bool(os.environ.get("BASS_TRACE"))
    res = run_bass_kernel_spmd(prog, in_maps, core_ids=list(range(NCORE)),
                               trace=trace)
    if res.exec_time_ns:
        HW_EXEC_NS.append(res.exec_time_ns)
    if trace:
        LAST_RES.append(res)
    return res.results


def _bf(a):
    return np.ascontiguousarray(np.asarray(a, np.float32).astype(NPBF))


def _ccn_host(w):
    """[K, M] -> [128, K//128 * M] channel-major pack block."""
    w = np.asarray(w, np.float32)
    k, m = w.shape
    return w.reshape(k // 128, 128, m).transpose(1, 0, 2).reshape(128, -1)


def _chunked(v, nch):
    v = np.asarray(v, np.float32)
    return np.ascontiguousarray(v.reshape(nch, 128).T.astype(np.float32))


def _ref_points(valid_ratios):
    refs = []
    for lvl, (H, W) in enumerate(SHAPES):
        gy, gx = np.meshgrid(np.arange(H, dtype=np.float32) + 0.5,
                             np.arange(W, dtype=np.float32) + 0.5,
                             indexing="ij")
        ry = gy.reshape(-1)[None] / (valid_ratios[:, lvl, 1][:, None] * H)
        rx = gx.reshape(-1)[None] / (valid_ratios[:, lvl, 0][:, None] * W)
        refs.append(np.stack([rx, ry], -1))
    ref = np.concatenate(refs, 1)
    return ref[:, :, None, :] * valid_ratios[:, None]


def _host_sample(value, off, aw, ref_pts):
    N, Lq = off.shape[:2]
    off = off.reshape(N, Lq, N_HEADS, N_LEVELS, N_POINTS, 2)
    aw = aw.reshape(N, Lq, N_HEADS, N_LEVELS, N_POINTS)
    normalizer = np.array([[w, h] for h, w in SHAPES], np.float32)
    loc = (ref_pts[:, :, None, :, None, :]
           + off / normalizer[None, None, None, :, None, :])
    acc = np.zeros((N, N_HEADS, Lq, HEAD_DIM), np.float32)
    for lvl, (H, W) in enumerate(SHAPES):
        s = LEVEL_STARTS[lvl]
        val = value[:, s:s + H * W].transpose(0, 2, 1, 3)
        x = loc[:, :, :, lvl, :, 0] * W - 0.5
        y = loc[:, :, :, lvl, :, 1] * H - 0.5
        x0 = np.floor(x)
        y0 = np.floor(y)
        wx1 = x - x0
        wy1 = y - y0
        ix0 = x0.astype(np.int64)
        iy0 = y0.astype(np.int64)

        def corner(ix, iy, w):
            valid = (ix >= 0) & (ix < W) & (iy >= 0) & (iy < H)
            idx = np.clip(iy, 0, H - 1) * W + np.clip(ix, 0, W - 1)
            idx = idx.transpose(0, 2, 1, 3).reshape(N, N_HEADS, Lq * N_POINTS)
            g = np.take_along_axis(val, idx[..., None], axis=2)
            g = g.reshape(N, N_HEADS, Lq, N_POINTS, HEAD_DIM)
            w = np.where(valid, w, 0.0).transpose(0, 2, 1, 3)
            return g * w[..., None].astype(np.float32)

        sampled = (corner(ix0, iy0, (1 - wx1) * (1 - wy1))
                   + corner(ix0 + 1, iy0, wx1 * (1 - wy1))
                   + corner(ix0, iy0 + 1, (1 - wx1) * wy1)
                   + corner(ix0 + 1, iy0 + 1, wx1 * wy1))
        acc += (sampled * aw[:, :, :, lvl].transpose(0, 2, 1, 3)[..., None]
                ).sum(3)
    return acc.transpose(0, 2, 1, 3).reshape(N, Lq, D_MODEL)


def _shardT(fullT):
    return [np.ascontiguousarray(fullT[c // 4, :, (c % 4) * TPC:
                                       (c % 4 + 1) * TPC])
            for c in range(NCORE)]


def _unshardT(parts):
    F = parts[0].shape[0]
    out = np.empty((BATCH, LEN_IN, F), np.float32)
    for c in range(NCORE):
        out[c // 4, (c % 4) * TPC:(c % 4 + 1) * TPC] = \
            np.asarray(parts[c], np.float32).T
    return out


def _unshard_pm(parts):  # partition-major parts [128, NT*256]
    out = np.empty((BATCH, LEN_IN, 256), np.float32)
    for c in range(NCORE):
        a = np.asarray(parts[c], np.float32).reshape(128, NT, 256)
        a = a.transpose(1, 0, 2).reshape(NT * 128, 256)[:TPC]
        out[c // 4, (c % 4) * TPC:(c % 4 + 1) * TPC] = a
    return out


_IDENT = np.eye(128, dtype=np.float32)


def kernel(src, pos, valid_ratios, Wv, bv, Woff, boff, Wa, ba, Wo, bo,
           g1, be1, Wl1, bl1, Wl2, bl2, g2, be2):
    src = np.asarray(src, np.float32)
    pos = np.asarray(pos, np.float32)
    valid_ratios = np.asarray(valid_ratios, np.float32)
    asf = lambda a: np.asarray(a, np.float32)
    HW_EXEC_NS.clear()
    LAST_RES.clear()

    if "A" not in _PROGS:
        _PROGS["A"] = _build_A()
        _PROGS["BCDA"] = _build_BCDA(with_A=True)
        _PROGS["BCD"] = _build_BCDA(with_A=False)

    ref_pts = _ref_points(valid_ratios)

    Woa = [np.concatenate([asf(Woff[l]), asf(Wa[l])], axis=1)
           for l in range(2)]
    bva = [np.concatenate([asf(bv[l]), asf(boff[l]), asf(ba[l])])
           for l in range(2)]
    Wl1g = [asf(g1[l])[:, None] * asf(Wl1[l]) for l in range(2)]
    bl1f = [asf(bl1[l]) + asf(be1[l]) @ asf(Wl1[l]) for l in range(2)]
    # layer-1 value-proj with layer-0 g2/be2 folded in (q-path keeps
    # plain Woa; q is built on device as g2*xn2 + be2 + pos)
    Wv1f = asf(g2[0])[:, None] * asf(Wv[1])
    bva1f = np.concatenate([asf(bv[1]) + asf(be2[0]) @ asf(Wv[1]),
                            bva[1][256:]])
    cr = [asf(be1[l]) + asf(bl2[l]) for l in range(2)]
    rows = [np.ascontiguousarray(cr[l][None, :].astype(NPBF))
            for l in range(2)]
    prm = [np.concatenate([np.full((128, 1), 1e-5, np.float32),
                           _chunked(g2[l], 2), _chunked(be2[l], 2),
                           _chunked(bl1f[l], 8),
                           _chunked(bva1f if l == 0 else np.zeros(640), 5)],
                          axis=1) for l in range(2)]
    dg1 = [np.diag(asf(g1[l])) for l in range(2)]

    # packed weight blobs
    wpkA = np.concatenate([_ccn_host(Wv[0]), _ccn_host(Woa[0])],
                          axis=1).astype(NPBF)
    wpk1 = [np.concatenate([_ccn_host(Wo[l]), _IDENT],
                           axis=1).astype(NPBF) for l in range(2)]
    wpk2 = [np.concatenate([_ccn_host(Wl1g[l]), _ccn_host(dg1[l]),
                            _ccn_host(Wl2[l])], axis=1).astype(NPBF)
            for l in range(2)]
    wpk3 = np.concatenate([_ccn_host(Wv1f), _ccn_host(Woa[1])],
                          axis=1).astype(NPBF)

    xT = np.ascontiguousarray(src.transpose(0, 2, 1))
    qT = np.ascontiguousarray((src + pos).transpose(0, 2, 1))
    # q for layer-1 projections is g2*xn2 + (be2 + pos); fold be2 into pos
    posbT = np.ascontiguousarray(
        (pos + asf(be2[0])[None, None, :]).transpose(0, 2, 1))
    xTs = _shardT(xT.astype(NPBF))
    qTs = _shardT(qT.astype(NPBF))
    posTs = _shardT(posbT.astype(NPBF))

    # ---- launch 1: layer-0 projections ----
    in_maps = [{
        "xT": xTs[c], "qT": qTs[c],
        "wpk": wpkA, "prm": _chunked(bva[0], 5),
    } for c in range(NCORE)]
    resA = _run(_PROGS["A"], in_maps)

    def gather_attn(value, offaw, layer, x_full):
        aw = offaw[:, :, 256:].reshape(BATCH, LEN_IN, N_HEADS, 16)
        aw = aw - aw.max(-1, keepdims=True)
        e = np.exp(aw)
        aw = (e / e.sum(-1, keepdims=True)).reshape(BATCH, LEN_IN, 128)
        attn = _host_sample(value.reshape(BATCH, LEN_IN, N_HEADS, HEAD_DIM),
                            offaw[:, :, :256], aw, ref_pts)
        attnT = np.ascontiguousarray(attn.transpose(0, 2, 1))
        xbf = (x_full + asf(bo[layer])[None, None, :]).transpose(0, 2, 1)
        return (_shardT(attnT.astype(NPBF)),
                _shardT(np.ascontiguousarray(xbf).astype(NPBF)))

    # ---- launch 2: layer-0 BCD + layer-1 projections ----
    value = _unshardT([resA[c]["valT"] for c in range(NCORE)])
    offaw = _unshardT([resA[c]["offawT"] for c in range(NCORE)])
    attnTs, xbs = gather_attn(value, offaw, 0, src)
    in_maps = [{
        "attnT": attnTs[c], "xbT": xbs[c],
        "wpk1": wpk1[0], "wpk2": wpk2[0], "wpk3": wpk3,
        "rows": rows[0], "prm": prm[0], "posT": posTs[c],
    } for c in range(NCORE)]
    resB = _run(_PROGS["BCDA"], in_maps)

    # x1 = g2*xn2 + be2 (host applies the folded affine)
    xn2 = _unshard_pm([resB[c]["x1n"] for c in range(NCORE)])
    x1 = xn2 * asf(g2[0])[None, None, :] + asf(be2[0])[None, None, :]

    # ---- launch 3: layer-1 BCD -> final ----
    val1 = _unshardT([resB[c]["valT"] for c in range(NCORE)])
    oa1 = _unshardT([resB[c]["offawT"] for c in range(NCORE)])
    attnTs, xbs = gather_attn(val1, oa1, 1, x1)
    in_maps = [{
        "attnT": attnTs[c], "xbT": xbs[c],
        "wpk1": wpk1[1], "wpk2": wpk2[1],
        "rows": rows[1], "prm": prm[1],
    } for c in range(NCORE)]
    resC = _run(_PROGS["BCD"], in_maps)

    # host LN2 + affine for the final layer
    r2 = _unshard_pm([resC[c]["out"] for c in range(NCORE)])
    m = r2.mean(-1, keepdims=True)
    v = np.square(r2 - m).mean(-1, keepdims=True)
    xn = (r2 - m) / np.sqrt(v + 1e-5)
    return (xn * asf(g2[1])[None, None, :]
            + asf(be2[1])[None, None, :]).astype(np.float32)


# revision 27
# speedup vs baseline: 1.1553x; 1.0011x over previous
"""Deformable-Transformer encoder on 8 trn2 NeuronCores — v4.

v3 + latency restructuring driven by NTFF traces:
  - Scratch-tile PE warmup at body start (no DMA dependency) so the HAM
    clock-gate is at 8/8 by the time real matmuls start; removes the
    wo-dependent warmup and the dummy transpose fillers.
  - Input DMAs packed (weights into 1-2 blobs) and spread across
    sync/scalar/vector/gpsimd queues so issue serialization (~0.65us per
    dma instr on one engine) stops gating the first matmul.
  - Drains paired: two 128-token tiles share one 2-slot PSUM tile, so
    PSUM->SBUF evacuation + bn_stats run at half the op count.
  - LN stats via bn_aggr (1 op) instead of a 9-op manual combine.
  - Final launch (BCD) skips LN2 entirely: it streams out the pre-LN2
    residual r2 per tile-pair and the host applies LN2+affine.
"""
import os
import sys
import types
import contextlib
import ctypes
import numpy as np

sys.path.insert(0, "/opt/trn_rl_repo")


def _install_ntff_hook():
    try:
        import antenv

        if hasattr(antenv, "axon_hooks"):
            return
        so_path = "/opt/axon/libaxon_pjrt.so"
        lib = ctypes.CDLL(so_path)
        if not hasattr(lib, "axon_start_nrt_profile"):
            hook = None
        else:
            lib.axon_start_nrt_profile.argtypes = [
                ctypes.POINTER(ctypes.c_int64), ctypes.c_size_t]
            lib.axon_start_nrt_profile.restype = ctypes.c_int64
            lib.axon_stop_nrt_profile.argtypes = [ctypes.c_char_p]
            lib.axon_stop_nrt_profile.restype = ctypes.c_int64

            @contextlib.contextmanager
            def hook(output_dir, device_ids):
                import jax
                jax.devices()
                if device_ids:
                    ids = (ctypes.c_int64 * len(device_ids))(*device_ids)
                    rc = lib.axon_start_nrt_profile(ids, len(device_ids))
                else:
                    rc = lib.axon_start_nrt_profile(None, 0)
                if rc != 0:
                    raise RuntimeError(f"start_nrt_profile rc={rc}")
                try:
                    yield
                finally:
                    lib.axon_stop_nrt_profile(str(output_dir).encode())

        m = types.ModuleType("antenv.axon_hooks")
        m.get_axon_ntff_profile_hook = lambda: hook
        m.set_axon_ntff_profile_hook = lambda h: None
        sys.modules["antenv.axon_hooks"] = m
        antenv.axon_hooks = m
    except Exception:
        pass


_install_ntff_hook()

import ml_dtypes  # noqa: E402
from concourse import bacc, tile, mybir, bass  # noqa: E402
from concourse.tile import add_dep_helper  # noqa: E402
from concourse.bass_utils import run_bass_kernel_spmd  # noqa: E402
from contextlib import ExitStack  # noqa: E402

F32 = mybir.dt.float32
BF16 = mybir.dt.bfloat16
NPBF = ml_dtypes.bfloat16
AF = mybir.ActivationFunctionType
ALU = mybir.AluOpType

SHAPES = ((64, 64), (32, 32), (16, 16), (8, 8))
LEVEL_STARTS = [0, 4096, 5120, 5376, 5440]
N_LEVELS, N_HEADS, N_POINTS = 4, 8, 4
D_MODEL, HEAD_DIM, D_FFN = 256, 32, 1024
LEN_IN, BATCH, NCORE = 5440, 2, 8
TPC = LEN_IN * BATCH // NCORE  # 1360 tokens per core
NT = 11                        # 128-token tiles per core
GROUPS = [(0, 512, range(0, 4)), (512, 512, range(4, 8)),
          (1024, 336, range(8, 11))]
PAIRS = [(0, 2), (2, 2), (4, 2), (6, 2), (8, 2), (10, 1)]
WARMUP_MM = 14

HW_EXEC_NS = []
LAST_RES = []
_PROGS = {}


def _nc():
    return bacc.Bacc("TRN2", target_bir_lowering=False, debug=False,
                     num_devices=NCORE)


def _tsz(ti):
    return min(128, TPC - ti * 128)


def _ccn(d):
    return d.rearrange("(c p) n -> p c n", p=128)


def _tchunks(step):
    out = []
    t0 = 0
    while t0 < TPC:
        out.append((t0, min(step, TPC - t0)))
        t0 += step
    return out


def _warmup(nc, sb, ps, ps_tag, bufs=2):
    """HAM warmup: dense matmuls on a memset scratch tile, no DMA deps.

    Returns the matmul handles so input DMAs can be staged against
    warmup progress (issue later ones only once earlier transfers have
    had the HBM bandwidth to themselves for a while)."""
    wsc = sb.tile([128, 256], BF16, tag="wsc")
    nc.gpsimd.memset(wsc[:], 0.25)
    mms = []
    for _ in range(WARMUP_MM):
        pw = ps.tile([128, 256], F32, tag=ps_tag, bufs=bufs)
        mms.append(nc.tensor.matmul(pw[:], wsc[:, 0:128], wsc[:],
                                    start=True, stop=True))
    return mms


def _build_A():
    """Layer-0 projections, channel-major world."""
    nc = _nc()
    xT_d = nc.dram_tensor("xT", [D_MODEL, TPC], BF16, kind="ExternalInput").ap()
    qT_d = nc.dram_tensor("qT", [D_MODEL, TPC], BF16, kind="ExternalInput").ap()
    wpk_d = nc.dram_tensor("wpk", [128, 1280], BF16, kind="ExternalInput").ap()
    prm_d = nc.dram_tensor("prm", [128, 5], F32, kind="ExternalInput").ap()
    valT_d = nc.dram_tensor("valT", [256, TPC], BF16,
                            kind="ExternalOutput").ap()
    oaT_d = nc.dram_tensor("offawT", [384, TPC], BF16,
                           kind="ExternalOutput").ap()

    with tile.TileContext(nc) as tc, ExitStack() as ctx:
        sb = ctx.enter_context(tc.tile_pool(name="sb", bufs=1))
        ps = ctx.enter_context(tc.tile_pool(name="ps", bufs=1, space="PSUM"))
        ob = ctx.enter_context(tc.tile_pool(name="ob", bufs=1))

        wmms = _warmup(nc, sb, ps, "p", bufs=3)

        wpk = sb.tile([128, 1280], BF16, tag="wpk")
        nc.sync.dma_start(wpk[:], wpk_d[:])
        wv = wpk[:, 0:512].rearrange("p (c n) -> p c n", c=2)
        woa = wpk[:, 512:1280].rearrange("p (c n) -> p c n", c=2)
        prm = sb.tile([128, 5], F32, tag="prm")
        nc.gpsimd.dma_start(prm[:], prm_d[:])

        chunks = _tchunks(512)
        xcs, qcs = [], []
        # chunk-0 input DMAs up front; chunk 1 staged on mid-warmup so
        # chunk 0 has the HBM bandwidth to itself first
        for ci in range(2):
            xc = ob.tile([128, 2, 512], BF16, tag="xc", bufs=2)
            qc = ob.tile([128, 2, 512], BF16, tag="qc", bufs=2)
            t0, tsz = chunks[ci]
            d1 = nc.sync.dma_start(xc[:, :, :tsz],
                                   _ccn(xT_d)[:, :, t0:t0 + tsz])
            nc.sync.dma_start(qc[:, :, :tsz], _ccn(qT_d)[:, :, t0:t0 + tsz])
            if ci == 1:
                add_dep_helper(d1.ins, wmms[6].ins, sync=True,
                               reason="stage chunk-1 dma")
            xcs.append(xc)
            qcs.append(qc)

        for ci, (t0, tsz) in enumerate(chunks):
            xc, qc = xcs[ci], qcs[ci]
            vsb = ob.tile([128, 2, 512], BF16, tag="vsb", bufs=2)
            osb = ob.tile([128, 3, 512], BF16, tag="osb", bufs=2)
            first_mm = None
            for m in range(5):  # 0-1: val (from x), 2-4: offaw (from q)
                src = xc if m < 2 else qc
                w = wv if m < 2 else woa
                mm = m if m < 2 else m - 2
                p = ps.tile([128, 512], F32, tag="p", bufs=3)
                for k in range(2):
                    mi = nc.tensor.matmul(
                        p[:, :tsz], w[:, k, mm * 128:mm * 128 + 128],
                        src[:, k, :tsz], start=(k == 0), stop=(k == 1))
                    if first_mm is None:
                        first_mm = mi
                dst = (vsb if m < 2 else osb)[:, mm, :tsz]
                if m % 2 == 0:
                    nc.scalar.activation(dst, p[:, :tsz], AF.Identity,
                                         bias=prm[:, m:m + 1])
                else:
                    nc.vector.tensor_scalar(dst, p[:, :tsz], prm[:, m:m + 1],
                                            None, ALU.add)
            if ci + 2 < len(chunks):
                t1, tsz1 = chunks[ci + 2]
                xn = ob.tile([128, 2, 512], BF16, tag="xc", bufs=2)
                qn = ob.tile([128, 2, 512], BF16, tag="qc", bufs=2)
                d1 = nc.sync.dma_start(xn[:, :, :tsz1],
                                       _ccn(xT_d)[:, :, t1:t1 + tsz1])
                nc.sync.dma_start(qn[:, :, :tsz1],
                                  _ccn(qT_d)[:, :, t1:t1 + tsz1])
                add_dep_helper(d1.ins, first_mm.ins, sync=True,
                               reason="defer chunk dma")
                xcs.append(xn)
                qcs.append(qn)
            nc.scalar.dma_start(_ccn(valT_d)[:, :, t0:t0 + tsz],
                                vsb[:, :, :tsz])
            nc.sync.dma_start(
                oaT_d.rearrange("(c p) n -> p c n", p=128)[:, :, t0:t0 + tsz],
                osb[:, :, :tsz])
    nc.compile()
    return nc


def _build_BCDA(with_A):
    """Fused out-proj + LN1 + FFN (+ LN2 + next-layer projections).

    with_A=True (layer 0): outputs x1n (pre-affine LN2), valT, offawT.
    with_A=False (layer 1): outputs out = r2 (pre-LN2 residual); the host
    applies LN2 + g2/be2.
    """
    nc = _nc()
    aT_d = nc.dram_tensor("attnT", [D_MODEL, TPC], BF16,
                          kind="ExternalInput").ap()
    xbT_d = nc.dram_tensor("xbT", [D_MODEL, TPC], BF16,
                           kind="ExternalInput").ap()
    # wpk1: wo (2x256) | ident (128)
    wpk1_d = nc.dram_tensor("wpk1", [128, 640], BF16,
                            kind="ExternalInput").ap()
    # wpk2: wl1 (2x1024) | dg1 (2x256) | wl2 (8x256)
    wpk2_d = nc.dram_tensor("wpk2", [128, 4608], BF16,
                            kind="ExternalInput").ap()
    rows_d = nc.dram_tensor("rows", [1, 256], BF16, kind="ExternalInput").ap()
    prm_d = nc.dram_tensor("prm", [128, 18], F32, kind="ExternalInput").ap()
    if with_A:
        posT_d = nc.dram_tensor("posT", [D_MODEL, TPC], BF16,
                                kind="ExternalInput").ap()
        # wpk3: wv (2x256) | woa (2x384)
        wpk3_d = nc.dram_tensor("wpk3", [128, 1280], BF16,
                                kind="ExternalInput").ap()
        x1n_d = nc.dram_tensor("x1n", [128, NT * 256], BF16,
                               kind="ExternalOutput").ap()
        valT_d = nc.dram_tensor("valT", [256, TPC], BF16,
                                kind="ExternalOutput").ap()
        oaT_d = nc.dram_tensor("offawT", [384, TPC], BF16,
                               kind="ExternalOutput").ap()
    else:
        out_d = nc.dram_tensor("out", [128, NT * 256], BF16,
                               kind="ExternalOutput").ap()

    with tile.TileContext(nc) as tc, ExitStack() as ctx:
        sb = ctx.enter_context(tc.tile_pool(name="sb", bufs=1))
        ps = ctx.enter_context(tc.tile_pool(name="ps", bufs=1, space="PSUM"))
        ob = ctx.enter_context(tc.tile_pool(name="ob", bufs=1))

        wmms = _warmup(nc, sb, ps, "pb")

        # ---- input DMAs: critical ones up front, bulk deferred ----
        wpk1 = sb.tile([128, 640], BF16, tag="wpk1")
        nc.sync.dma_start(wpk1[:], wpk1_d[:])
        wo = wpk1[:, 0:512].rearrange("p (c n) -> p c n", c=2)
        idn = wpk1[:, 512:640]
        aT = sb.tile([128, 2, TPC], BF16, tag="aT")
        nc.sync.dma_start(aT[:, :, 0:256], _ccn(aT_d)[:, :, 0:256])
        d = nc.sync.dma_start(aT[:, :, 256:512], _ccn(aT_d)[:, :, 256:512])
        add_dep_helper(d.ins, wmms[2].ins, sync=True, reason="stage aT0b")
        d = nc.sync.dma_start(aT[:, :, 512:TPC], _ccn(aT_d)[:, :, 512:TPC])
        add_dep_helper(d.ins, wmms[6].ins, sync=True, reason="stage aT1")
        xbT = sb.tile([128, 2, TPC], BF16, tag="xbT")
        nc.scalar.dma_start(xbT[:, :, 0:256], _ccn(xbT_d)[:, :, 0:256])
        d = nc.scalar.dma_start(xbT[:, :, 256:512], _ccn(xbT_d)[:, :, 256:512])
        add_dep_helper(d.ins, wmms[2].ins, sync=True, reason="stage xbT0b")
        d = nc.scalar.dma_start(xbT[:, :, 512:TPC], _ccn(xbT_d)[:, :, 512:TPC])
        add_dep_helper(d.ins, wmms[6].ins, sync=True, reason="stage xbT1")
        # force the sqrt ACT table set resident before the LN stats chain
        sqd = sb.tile([128, 1], F32, tag="sqd")
        nc.gpsimd.memset(sqd[:], 1.0)
        nc.scalar.activation(sqd[:, 0:1], sqd[:, 0:1], AF.Sqrt)
        # wpk2/posT/wpk3 tiles declared now, DMAs emitted inside sweep 1
        # gated on PE progress so they don't steal HBM bandwidth from aT/xbT
        wpk2 = sb.tile([128, 4608], BF16, tag="wpk2")
        wl1 = wpk2[:, 0:2048].rearrange("p (c n) -> p c n", c=2)
        dg1 = wpk2[:, 2048:2560].rearrange("p (c n) -> p c n", c=2)
        wl2 = wpk2[:, 2560:4608].rearrange("p (c n) -> p c n", c=8)
        prm = sb.tile([128, 18], F32, tag="prm")
        nc.gpsimd.dma_start(prm[:], prm_d[:])
        rows = sb.tile([1, 256], BF16, tag="rows")
        nc.gpsimd.dma_start(rows[:], rows_d[:])
        if with_A:
            posT = sb.tile([128, 2, TPC], BF16, tag="posT")
            wpk3 = sb.tile([128, 1280], BF16, tag="wpk3")
            wv = wpk3[:, 0:512].rearrange("p (c n) -> p c n", c=2)
            woa = wpk3[:, 512:1280].rearrange("p (c n) -> p c n", c=2)
            valTs = sb.tile([128, 2, TPC], BF16, tag="valTs")
            oaTs = sb.tile([128, 3, TPC], BF16, tag="oaTs")
            q1Ts = sb.tile([128, 2, TPC], BF16, tag="q1Ts")
        ones = sb.tile([1, 512], BF16, tag="ones")
        nc.gpsimd.memset(ones[:], 1.0)

        # persistent intermediates
        r1a = sb.tile([128, NT, 256], BF16, tag="r1a")
        xnTa = sb.tile([128, 2, TPC], BF16, tag="xnTa")
        hta = sb.tile([128, 8, TPC], BF16, tag="hta")
        xna = sb.tile([128, NT, 256], BF16, tag="xna")
        bst1 = sb.tile([128, NT, 6], F32, tag="bst1")
        mv1 = sb.tile([128, NT, 2], F32, tag="mv1")
        st1r = sb.tile([128, NT, 1], F32, tag="st1r", name="st1r")
        st1n = sb.tile([128, NT, 1], F32, tag="st1n", name="st1n")
        if with_A:
            r2a = sb.tile([128, NT, 256], BF16, tag="r2a")
            xn2Ta = sb.tile([128, 2, TPC], BF16, tag="xn2Ta")
            xout = sb.tile([128, NT, 256], BF16, tag="xout")
            bst2 = sb.tile([128, NT, 6], F32, tag="bst2")
            mv2 = sb.tile([128, NT, 2], F32, tag="mv2")
            st2r = sb.tile([128, NT, 1], F32, tag="st2r", name="st2r")
            st2n = sb.tile([128, NT, 1], F32, tag="st2n", name="st2n")
        else:
            r2b = sb.tile([128, NT, 256], BF16, tag="r2b")

        def stats_chain(mv, str_, stn, h0, h1, eng):
            """mv[:, h0:h1] = (mean, var) -> str_=rstd, stn=-mean*rstd."""
            sd = ob.tile([128, NT, 1], F32, tag="sd", bufs=2)
            nc.scalar.activation(sd[:, h0:h1, :], mv[:, h0:h1, 1:2], AF.Sqrt,
                                 bias=prm[:, 0:1])
            nc.vector.reciprocal(str_[:, h0:h1, :], sd[:, h0:h1, :])
            eng.scalar_tensor_tensor(stn[:, h0:h1, :], mv[:, h0:h1, 0:1],
                                     -1.0, str_[:, h0:h1, :],
                                     op0=ALU.mult, op1=ALU.mult)

        # ---- sweep 1: B matmul + residual, paired drains + LN1 stats ----
        for pi, (t0i, np_) in enumerate(PAIRS):
            pbp = ps.tile([128, 2, 256], F32, tag="pb", bufs=2)
            first_mm = None
            for j in range(np_):
                ti = t0i + j
                sz = _tsz(ti)
                t0 = ti * 128
                for k in range(2):
                    mi = nc.tensor.matmul(pbp[:sz, j, :],
                                          aT[:, k, t0:t0 + sz],
                                          wo[:, k, :], start=(k == 0),
                                          stop=False)
                    if first_mm is None:
                        first_mm = mi
                for k in range(2):
                    nc.tensor.matmul(pbp[:sz, j, k * 128:k * 128 + 128],
                                     xbT[:, k, t0:t0 + sz], idn[:, :],
                                     start=False, stop=(k == 1),
                                     skip_group_check=True)
            if pi == 0:
                d = nc.sync.dma_start(wpk2[:], wpk2_d[:])
                add_dep_helper(d.ins, first_mm.ins, sync=True,
                               reason="defer wpk2 dma")
            elif pi == 2 and with_A:
                d = nc.sync.dma_start(posT[:], _ccn(posT_d))
                nc.sync.dma_start(wpk3[:], wpk3_d[:])
                add_dep_helper(d.ins, first_mm.ins, sync=True,
                               reason="defer posT/wpk3 dma")
            sz0 = _tsz(t0i + np_ - 1)
            if np_ == 2:
                if pi % 2 == 0:
                    nc.scalar.copy(r1a[:, t0i:t0i + 2, :], pbp[:, :, :])
                else:
                    nc.vector.tensor_copy(r1a[:, t0i:t0i + 2, :], pbp[:, :, :])
            else:
                nc.vector.tensor_copy(r1a[:sz0, t0i, :], pbp[:sz0, 0, :])
            for j in range(np_):
                ti = t0i + j
                sz = _tsz(ti)
                nc.vector.bn_stats(bst1[:sz, ti, :], r1a[:sz, ti, :])
                nc.vector.bn_aggr(mv1[:sz, ti, :], bst1[:sz, ti, :])
            stats_chain(mv1, st1r, st1n, t0i, t0i + np_, nc.vector)

        # ---- sweep 2: LN1 apply, transpose, C, D (+ LN2 stats) ----
        for gi, (g0, gsz, tis) in enumerate(GROUPS):
            for t0i, np_ in PAIRS:
                if t0i not in tis:
                    continue
                pt = ps.tile([128, 2, 2, 128], BF16, tag="ptr", bufs=2)
                for j in range(np_):
                    ti = t0i + j
                    sz = _tsz(ti)
                    nc.scalar.activation(xna[:sz, ti, :], r1a[:sz, ti, :],
                                         AF.Identity,
                                         bias=st1n[:sz, ti, :],
                                         scale=st1r[:sz, ti, :])
                    for c in range(2):
                        nc.tensor.transpose(
                            pt[:, j, c, :sz],
                            xna[:sz, ti, c * 128:c * 128 + 128],
                            idn[:sz, :sz])
                t0 = t0i * 128
                tw = sum(_tsz(t0i + j) for j in range(np_))
                if np_ == 2:
                    src = pt.transpose((0, 2, 1, 3))
                    dst = xnTa[:, :, t0:t0 + 256].rearrange(
                        "p c (a b) -> p c a b", a=2)
                    if t0i % 4 == 0:
                        nc.vector.tensor_copy(dst, src)
                    else:
                        nc.scalar.copy(dst, src)
                else:
                    nc.vector.tensor_copy(xnTa[:, :, t0:t0 + tw],
                                          pt[:, 0, :, :tw])
            # C over the whole group: hT = relu(Wl1g.T @ xnT + bl1row)
            for m in range(8):
                pc = ps.tile([128, 512], F32, tag="pca", bufs=2)
                for k in range(2):
                    nc.tensor.matmul(pc[:, :gsz],
                                     wl1[:, k, m * 128:m * 128 + 128],
                                     xnTa[:, k, g0:g0 + gsz],
                                     start=(k == 0), stop=(k == 1))
                if m % 2 == 0:
                    nc.scalar.activation(hta[:, m, g0:g0 + gsz], pc[:, :gsz],
                                         AF.Relu, bias=prm[:, 5 + m:6 + m])
                else:
                    nc.vector.tensor_scalar(hta[:, m, g0:g0 + gsz],
                                            pc[:, :gsz], prm[:, 5 + m:6 + m],
                                            0.0, ALU.add, ALU.max)
            # D, paired into 2-slot PSUM tiles
            for t0i, np_ in PAIRS:
                if t0i not in tis:
                    continue
                pdp = ps.tile([128, 2, 256], F32, tag="pd", bufs=2)
                for j in range(np_):
                    ti = t0i + j
                    sz = _tsz(ti)
                    t0 = ti * 128
                    for k in range(8):
                        nc.tensor.matmul(pdp[:sz, j, :], hta[:, k, t0:t0 + sz],
                                         wl2[:, k, :],
                                         start=(k == 0), stop=False)
                    for k in range(2):
                        nc.tensor.matmul(pdp[:sz, j, :],
                                         xnTa[:, k, t0:t0 + sz],
                                         dg1[:, k, :], start=False, stop=False)
                    nc.tensor.matmul(pdp[:sz, j, :], ones[0:1, :sz],
                                     rows[:, :], start=False, stop=True)
                sz0 = _tsz(t0i + np_ - 1)
                if with_A:
                    if np_ == 2:
                        nc.vector.tensor_copy(r2a[:, t0i:t0i + 2, :],
                                              pdp[:, :, :])
                    else:
                        nc.vector.tensor_copy(r2a[:sz0, t0i, :],
                                              pdp[:sz0, 0, :])
                    for j in range(np_):
                        ti = t0i + j
                        sz = _tsz(ti)
                        nc.vector.bn_stats(bst2[:sz, ti, :], r2a[:sz, ti, :])
                        nc.vector.bn_aggr(mv2[:sz, ti, :], bst2[:sz, ti, :])
                    stats_chain(mv2, st2r, st2n, t0i, t0i + np_, nc.vector)
                else:
                    if np_ == 2:
                        if t0i % 4 == 0:
                            nc.vector.tensor_copy(r2b[:, t0i:t0i + 2, :],
                                                  pdp[:, :, :])
                        else:
                            nc.scalar.copy(r2b[:, t0i:t0i + 2, :],
                                           pdp[:, :, :])
                        dma_eng = nc.sync if t0i % 4 == 0 else nc.scalar
                        dma_eng.dma_start(
                            out_d[:, t0i * 256:(t0i + 2) * 256],
                            r2b[:, t0i:t0i + 2, :])
                    else:
                        nc.vector.tensor_copy(r2b[:sz0, t0i, :],
                                              pdp[:sz0, 0, :])
                        nc.sync.dma_start(
                            out_d[:, t0i * 256:(t0i + 1) * 256],
                            r2b[:, t0i, :])


        # ---- sweep 3 (with_A): LN2 apply + next-layer projections ----
        if with_A:
            for g0, gsz, tis in GROUPS:
                for t0i, np_ in PAIRS:
                    if t0i not in tis:
                        continue
                    pt2 = ps.tile([128, 2, 2, 128], BF16, tag="ptr", bufs=2)
                    for j in range(np_):
                        ti = t0i + j
                        sz = _tsz(ti)
                        nc.scalar.activation(xout[:sz, ti, :], r2a[:sz, ti, :],
                                             AF.Identity,
                                             bias=st2n[:sz, ti, :],
                                             scale=st2r[:sz, ti, :])
                        for c in range(2):
                            nc.tensor.transpose(
                                pt2[:, j, c, :sz],
                                xout[:sz, ti, c * 128:c * 128 + 128],
                                idn[:sz, :sz])
                    t0 = t0i * 128
                    tw = sum(_tsz(t0i + j) for j in range(np_))
                    if np_ == 2:
                        src = pt2.transpose((0, 2, 1, 3))
                        dst = xn2Ta[:, :, t0:t0 + 256].rearrange(
                            "p c (a b) -> p c a b", a=2)
                        nc.scalar.copy(dst, src)
                        # q1T = g2*xn2T + (be2+pos)T, fused into the drain
                        for c in range(2):
                            nc.vector.scalar_tensor_tensor(
                                q1Ts[:, c, t0:t0 + 256].rearrange(
                                    "p (a b) -> p a b", a=2),
                                pt2[:, :, c, :], prm[:, 1 + c:2 + c],
                                posT[:, c, t0:t0 + 256].rearrange(
                                    "p (a b) -> p a b", a=2),
                                op0=ALU.mult, op1=ALU.add)
                    else:
                        nc.scalar.copy(xn2Ta[:, :, t0:t0 + tw],
                                       pt2[:, 0, :, :tw])
                        for c in range(2):
                            nc.vector.scalar_tensor_tensor(
                                q1Ts[:, c, t0:t0 + tw],
                                pt2[:, 0, c, :tw], prm[:, 1 + c:2 + c],
                                posT[:, c, t0:t0 + tw],
                                op0=ALU.mult, op1=ALU.add)
                # val projections first (no q dependency)
                for m in range(2):
                    pa = ps.tile([128, 512], F32, tag="pca", bufs=2)
                    for k in range(2):
                        nc.tensor.matmul(pa[:, :gsz],
                                         wv[:, k, m * 128:m * 128 + 128],
                                         xn2Ta[:, k, g0:g0 + gsz],
                                         start=(k == 0), stop=(k == 1))
                    dst = valTs[:, m, g0:g0 + gsz]
                    if m % 2 == 0:
                        nc.scalar.activation(dst, pa[:, :gsz], AF.Identity,
                                             bias=prm[:, 13 + m:14 + m])
                    else:
                        nc.vector.tensor_scalar(dst, pa[:, :gsz],
                                                prm[:, 13 + m:14 + m],
                                                None, ALU.add)
                for m in range(3):
                    pa = ps.tile([128, 512], F32, tag="pca", bufs=2)
                    for k in range(2):
                        nc.tensor.matmul(pa[:, :gsz],
                                         woa[:, k, m * 128:m * 128 + 128],
                                         q1Ts[:, k, g0:g0 + gsz],
                                         start=(k == 0), stop=(k == 1))
                    dst = oaTs[:, m, g0:g0 + gsz]
                    if m % 2 == 1:
                        nc.scalar.activation(dst, pa[:, :gsz], AF.Identity,
                                             bias=prm[:, 15 + m:16 + m])
                    else:
                        nc.vector.tensor_scalar(dst, pa[:, :gsz],
                                                prm[:, 15 + m:16 + m],
                                                None, ALU.add)
                # output DMAs per group
                lo, hi = tis[0], tis[-1] + 1
                nc.scalar.dma_start(_ccn(valT_d)[:, :, g0:g0 + gsz],
                                    valTs[:, :, g0:g0 + gsz])
                nc.sync.dma_start(
                    oaT_d.rearrange("(c p) n -> p c n", p=128)[:, :,
                                                              g0:g0 + gsz],
                    oaTs[:, :, g0:g0 + gsz])
                nc.gpsimd.dma_start(
                    x1n_d[:, lo * 256:hi * 256], xout[:, lo:hi, :])
    nc.compile()
    return nc


def _run(prog, in_maps):
    trace = bool(os.environ.get("BASS_TRACE"))
    res = run_bass_kernel_spmd(prog, in_maps, core_ids=list(range(NCORE)),
                               trace=trace)
    if res.exec_time_ns:
        HW_EXEC_NS.append(res.exec_time_ns)
    if trace:
        LAST_RES.append(res)
    return res.results


def _bf(a):
    return np.ascontiguousarray(np.asarray(a, np.float32).astype(NPBF))


def _ccn_host(w):
    """[K, M] -> [128, K//128 * M] channel-major pack block."""
    w = np.asarray(w, np.float32)
    k, m = w.shape
    return w.reshape(k // 128, 128, m).transpose(1, 0, 2).reshape(128, -1)


def _chunked(v, nch):
    v = np.asarray(v, np.float32)
    return np.ascontiguousarray(v.reshape(nch, 128).T.astype(np.float32))


def _ref_points(valid_ratios):
    refs = []
    for lvl, (H, W) in enumerate(SHAPES):
        gy, gx = np.meshgrid(np.arange(H, dtype=np.float32) + 0.5,
                             np.arange(W, dtype=np.float32) + 0.5,
                             indexing="ij")
        ry = gy.reshape(-1)[None] / (valid_ratios[:, lvl, 1][:, None] * H)
        rx = gx.reshape(-1)[None] / (valid_ratios[:, lvl, 0][:, None] * W)
        refs.append(np.stack([rx, ry], -1))
    ref = np.concatenate(refs, 1)
    return ref[:, :, None, :] * valid_ratios[:, None]


def _host_sample(value, off, aw, ref_pts):
    N, Lq = off.shape[:2]
    off = off.reshape(N, Lq, N_HEADS, N_LEVELS, N_POINTS, 2)
    aw = aw.reshape(N, Lq, N_HEADS, N_LEVELS, N_POINTS)
    normalizer = np.array([[w, h] for h, w in SHAPES], np.float32)
    loc = (ref_pts[:, :, None, :, None, :]
           + off / normalizer[None, None, None, :, None, :])
    acc = np.zeros((N, N_HEADS, Lq, HEAD_DIM), np.float32)
    for lvl, (H, W) in enumerate(SHAPES):
        s = LEVEL_STARTS[lvl]
        val = value[:, s:s + H * W].transpose(0, 2, 1, 3)
        x = loc[:, :, :, lvl, :, 0] * W - 0.5
        y = loc[:, :, :, lvl, :, 1] * H - 0.5
        x0 = np.floor(x)
        y0 = np.floor(y)
        wx1 = x - x0
        wy1 = y - y0
        ix0 = x0.astype(np.int64)
        iy0 = y0.astype(np.int64)

        def corner(ix, iy, w):
            valid = (ix >= 0) & (ix < W) & (iy >= 0) & (iy < H)
            idx = np.clip(iy, 0, H - 1) * W + np.clip(ix, 0, W - 1)
            idx = idx.transpose(0, 2, 1, 3).reshape(N, N_HEADS, Lq * N_POINTS)
            g = np.take_along_axis(val, idx[..., None], axis=2)
            g = g.reshape(N, N_HEADS, Lq, N_POINTS, HEAD_DIM)
            w = np.where(valid, w, 0.0).transpose(0, 2, 1, 3)
            return g * w[..., None].astype(np.float32)

        sampled = (corner(ix0, iy0, (1 - wx1) * (1 - wy1))
                   + corner(ix0 + 1, iy0, wx1 * (1 - wy1))
                   + corner(ix0, iy0 + 1, (1 - wx1) * wy1)
                   + corner(ix0 + 1, iy0 + 1, wx1 * wy1))
        acc += (sampled * aw[:, :, :, lvl].transpose(0, 2, 1, 3)[..., None]
                ).sum(3)
    return acc.transpose(0, 2, 1, 3).reshape(N, Lq, D_MODEL)


def _shardT(fullT):
    return [np.ascontiguousarray(fullT[c // 4, :, (c % 4) * TPC:
                                       (c % 4 + 1) * TPC])
            for c in range(NCORE)]


def _unshardT(parts):
    F = parts[0].shape[0]
    out = np.empty((BATCH, LEN_IN, F), np.float32)
    for c in range(NCORE):
        out[c // 4, (c % 4) * TPC:(c % 4 + 1) * TPC] = \
            np.asarray(parts[c], np.float32).T
    return out


def _unshard_pm(parts):  # partition-major parts [128, NT*256]
    out = np.empty((BATCH, LEN_IN, 256), np.float32)
    for c in range(NCORE):
        a = np.asarray(parts[c], np.float32).reshape(128, NT, 256)
        a = a.transpose(1, 0, 2).reshape(NT * 128, 256)[:TPC]
        out[c // 4, (c % 4) * TPC:(c % 4 + 1) * TPC] = a
    return out


_IDENT = np.eye(128, dtype=np.float32)


def kernel(src, pos, valid_ratios, Wv, bv, Woff, boff, Wa, ba, Wo, bo,
           g1, be1, Wl1, bl1, Wl2, bl2, g2, be2):
    src = np.asarray(src, np.float32)
    pos = np.asarray(pos, np.float32)
    valid_ratios = np.asarray(valid_ratios, np.float32)
    asf = lambda a: np.asarray(a, np.float32)
    HW_EXEC_NS.clear()
    LAST_RES.clear()

    if "A" not in _PROGS:
        _PROGS["A"] = _build_A()
        _PROGS["BCDA"] = _build_BCDA(with_A=True)
        _PROGS["BCD"] = _build_BCDA(with_A=False)

    ref_pts = _ref_points(valid_ratios)

    Woa = [np.concatenate([asf(Woff[l]), asf(Wa[l])], axis=1)
           for l in range(2)]
    bva = [np.concatenate([asf(bv[l]), asf(boff[l]), asf(ba[l])])
           for l in range(2)]
    Wl1g = [asf(g1[l])[:, None] * asf(Wl1[l]) for l in range(2)]
    bl1f = [asf(bl1[l]) + asf(be1[l]) @ asf(Wl1[l]) for l in range(2)]
    # layer-1 value-proj with layer-0 g2/be2 folded in (q-path keeps
    # plain Woa; q is built on device as g2*xn2 + be2 + pos)
    Wv1f = asf(g2[0])[:, None] * asf(Wv[1])
    bva1f = np.concatenate([asf(bv[1]) + asf(be2[0]) @ asf(Wv[1]),
                            bva[1][256:]])
    cr = [asf(be1[l]) + asf(bl2[l]) for l in range(2)]
    rows = [np.ascontiguousarray(cr[l][None, :].astype(NPBF))
            for l in range(2)]
    prm = [np.concatenate([np.full((128, 1), 1e-5, np.float32),
                           _chunked(g2[l], 2), _chunked(be2[l], 2),
                           _chunked(bl1f[l], 8),
                           _chunked(bva1f if l == 0 else np.zeros(640), 5)],
                          axis=1) for l in range(2)]
    dg1 = [np.diag(asf(g1[l])) for l in range(2)]

    # packed weight blobs
    wpkA = np.concatenate([_ccn_host(Wv[0]), _ccn_host(Woa[0])],
                          axis=1).astype(NPBF)
    wpk1 = [np.concatenate([_ccn_host(Wo[l]), _IDENT],
                           axis=1).astype(NPBF) for l in range(2)]
    wpk2 = [np.concatenate([_ccn_host(Wl1g[l]), _ccn_host(dg1[l]),
                            _ccn_host(Wl2[l])], axis=1).astype(NPBF)
            for l in range(2)]
    wpk3 = np.concatenate([_ccn_host(Wv1f), _ccn_host(Woa[1])],
                          axis=1).astype(NPBF)

    xT = np.ascontiguousarray(src.transpose(0, 2, 1))
    qT = np.ascontiguousarray((src + pos).transpose(0, 2, 1))
    # q for layer-1 projections is g2*xn2 + (be2 + pos); fold be2 into pos
    posbT = np.ascontiguousarray(
        (pos + asf(be2[0])[None, None, :]).transpose(0, 2, 1))
    xTs = _shardT(xT.astype(NPBF))
    qTs = _shardT(qT.astype(NPBF))
    posTs = _shardT(posbT.astype(NPBF))

    # ---- launch 1: layer-0 projections ----
    in_maps = [{
        "xT": xTs[c], "qT": qTs[c],
        "wpk": wpkA, "prm": _chunked(bva[0], 5),
    } for c in range(NCORE)]
    resA = _run(_PROGS["A"], in_maps)

    def gather_attn(value, offaw, layer, x_full):
        aw = offaw[:, :, 256:].reshape(BATCH, LEN_IN, N_HEADS, 16)
        aw = aw - aw.max(-1, keepdims=True)
        e = np.exp(aw)
        aw = (e / e.sum(-1, keepdims=True)).reshape(BATCH, LEN_IN, 128)
        attn = _host_sample(value.reshape(BATCH, LEN_IN, N_HEADS, HEAD_DIM),
                            offaw[:, :, :256], aw, ref_pts)
        attnT = np.ascontiguousarray(attn.transpose(0, 2, 1))
        xbf = (x_full + asf(bo[layer])[None, None, :]).transpose(0, 2, 1)
        return (_shardT(attnT.astype(NPBF)),
                _shardT(np.ascontiguousarray(xbf).astype(NPBF)))

    # ---- launch 2: layer-0 BCD + layer-1 projections ----
    value = _unshardT([resA[c]["valT"] for c in range(NCORE)])
    offaw = _unshardT([resA[c]["offawT"] for c in range(NCORE)])
    attnTs, xbs = gather_attn(value, offaw, 0, src)
    in_maps = [{
        "attnT": attnTs[c], "xbT": xbs[c],
        "wpk1": wpk1[0], "wpk2": wpk2[0], "wpk3": wpk3,
        "rows": rows[0], "prm": prm[0], "posT": posTs[c],
    } for c in range(NCORE)]
    resB = _run(_PROGS["BCDA"], in_maps)

    # x1 = g2*xn2 + be2 (host applies the folded affine)
    xn2 = _unshard_pm([resB[c]["x1n"] for c in range(NCORE)])
    x1 = xn2 * asf(g2[0])[None, None, :] + asf(be2[0])[None, None, :]

    # ---- launch 3: layer-1 BCD -> final ----
    val1 = _unshardT([resB[c]["valT"] for c in range(NCORE)])
    oa1 = _unshardT([resB[c]["offawT"] for c in range(NCORE)])
    attnTs, xbs = gather_attn(val1, oa1, 1, x1)
    in_maps = [{
        "attnT": attnTs[c], "xbT": xbs[c],
        "wpk1": wpk1[1], "wpk2": wpk2[1],
        "rows": rows[1], "prm": prm[1],
    } for c in range(NCORE)]
    resC = _run(_PROGS["BCD"], in_maps)

    # host LN2 + affine for the final layer
    r2 = _unshard_pm([resC[c]["out"] for c in range(NCORE)])
    m = r2.mean(-1, keepdims=True)
    v = np.square(r2 - m).mean(-1, keepdims=True)
    xn = (r2 - m) / np.sqrt(v + 1e-5)
    return (xn * asf(g2[1])[None, None, :]
            + asf(be2[1])[None, None, :]).astype(np.float32)


# revision 28
# speedup vs baseline: 1.1599x; 1.0040x over previous
"""Deformable-Transformer encoder on 8 trn2 NeuronCores — v8.

v3 + latency restructuring driven by NTFF traces (178.6us -> 154.7us):
  - Scratch-tile PE warmup at body start (no DMA dependency) so the HAM
    clock-gate is at 8/8 by the time real matmuls start; removes the
    wo-dependent warmup and the dummy transpose fillers.
  - Input DMAs packed (weights into 1-3 blobs) and STAGED: critical
    first-tile chunks issue up front, bulk transfers (wl1/wl2, posT,
    next-layer packs) are gated on PE warmup/sweep progress via
    add_dep_helper so they don't share HBM bandwidth with the
    first-needed bytes (SDMA round-robins all queued transfers).
  - Drains paired: two 128-token tiles share one 2-slot PSUM tile, so
    PSUM->SBUF evacuation runs at half the op count; r1/r2 kept bf16
    for 2x DVE throughput.
  - LN stats per pair: bn_stats + bn_aggr (1 op) + 3-op rstd chain,
    replacing the 9-op manual combine per tile-group; a dummy sqrt at
    kernel start preloads the ACT sqrt table outside the critical path.
  - q-path for next-layer projections fused into the transpose drain
    (scalar_tensor_tensor: g2*xn2T + (be2+pos)T), removing the separate
    scale ACTs and add.
  - Final launch (BCD) skips LN2 entirely: it streams out the pre-LN2
    residual r2 per tile-pair (with per-pair output DMAs) and the host
    applies LN2+affine.
"""
import os
import sys
import types
import contextlib
import ctypes
import numpy as np

sys.path.insert(0, "/opt/trn_rl_repo")


def _install_ntff_hook():
    try:
        import antenv

        if hasattr(antenv, "axon_hooks"):
            return
        so_path = "/opt/axon/libaxon_pjrt.so"
        lib = ctypes.CDLL(so_path)
        if not hasattr(lib, "axon_start_nrt_profile"):
            hook = None
        else:
            lib.axon_start_nrt_profile.argtypes = [
                ctypes.POINTER(ctypes.c_int64), ctypes.c_size_t]
            lib.axon_start_nrt_profile.restype = ctypes.c_int64
            lib.axon_stop_nrt_profile.argtypes = [ctypes.c_char_p]
            lib.axon_stop_nrt_profile.restype = ctypes.c_int64

            @contextlib.contextmanager
            def hook(output_dir, device_ids):
                import jax
                jax.devices()
                if device_ids:
                    ids = (ctypes.c_int64 * len(device_ids))(*device_ids)
                    rc = lib.axon_start_nrt_profile(ids, len(device_ids))
                else:
                    rc = lib.axon_start_nrt_profile(None, 0)
                if rc != 0:
                    raise RuntimeError(f"start_nrt_profile rc={rc}")
                try:
                    yield
                finally:
                    lib.axon_stop_nrt_profile(str(output_dir).encode())

        m = types.ModuleType("antenv.axon_hooks")
        m.get_axon_ntff_profile_hook = lambda: hook
        m.set_axon_ntff_profile_hook = lambda h: None
        sys.modules["antenv.axon_hooks"] = m
        antenv.axon_hooks = m
    except Exception:
        pass


_install_ntff_hook()

import ml_dtypes  # noqa: E402
from concourse import bacc, tile, mybir, bass  # noqa: E402
from concourse.tile import add_dep_helper  # noqa: E402
from concourse.bass_utils import run_bass_kernel_spmd  # noqa: E402
from contextlib import ExitStack  # noqa: E402

F32 = mybir.dt.float32
BF16 = mybir.dt.bfloat16
NPBF = ml_dtypes.bfloat16
AF = mybir.ActivationFunctionType
ALU = mybir.AluOpType

SHAPES = ((64, 64), (32, 32), (16, 16), (8, 8))
LEVEL_STARTS = [0, 4096, 5120, 5376, 5440]
N_LEVELS, N_HEADS, N_POINTS = 4, 8, 4
D_MODEL, HEAD_DIM, D_FFN = 256, 32, 1024
LEN_IN, BATCH, NCORE = 5440, 2, 8
TPC = LEN_IN * BATCH // NCORE  # 1360 tokens per core
NT = 11                        # 128-token tiles per core
GROUPS = [(0, 512, range(0, 4)), (512, 512, range(4, 8)),
          (1024, 336, range(8, 11))]
PAIRS = [(0, 2), (2, 2), (4, 2), (6, 2), (8, 2), (10, 1)]
WARMUP_MM = 14

HW_EXEC_NS = []
LAST_RES = []
_PROGS = {}


def _nc():
    return bacc.Bacc("TRN2", target_bir_lowering=False, debug=False,
                     num_devices=NCORE)


def _tsz(ti):
    return min(128, TPC - ti * 128)


def _ccn(d):
    return d.rearrange("(c p) n -> p c n", p=128)


def _tchunks(step):
    out = []
    t0 = 0
    while t0 < TPC:
        out.append((t0, min(step, TPC - t0)))
        t0 += step
    return out


def _warmup(nc, sb, ps, ps_tag, bufs=2):
    """HAM warmup: dense matmuls on a memset scratch tile, no DMA deps.

    Returns the matmul handles so input DMAs can be staged against
    warmup progress (issue later ones only once earlier transfers have
    had the HBM bandwidth to themselves for a while)."""
    wsc = sb.tile([128, 256], BF16, tag="wsc")
    nc.gpsimd.memset(wsc[:], 0.25)
    mms = []
    for _ in range(WARMUP_MM):
        pw = ps.tile([128, 256], F32, tag=ps_tag, bufs=bufs)
        mms.append(nc.tensor.matmul(pw[:], wsc[:, 0:128], wsc[:],
                                    start=True, stop=True))
    return mms


def _build_A():
    """Layer-0 projections, channel-major world."""
    nc = _nc()
    xT_d = nc.dram_tensor("xT", [D_MODEL, TPC], BF16, kind="ExternalInput").ap()
    qT_d = nc.dram_tensor("qT", [D_MODEL, TPC], BF16, kind="ExternalInput").ap()
    wpk_d = nc.dram_tensor("wpk", [128, 1280], BF16, kind="ExternalInput").ap()
    prm_d = nc.dram_tensor("prm", [128, 5], F32, kind="ExternalInput").ap()
    valT_d = nc.dram_tensor("valT", [256, TPC], BF16,
                            kind="ExternalOutput").ap()
    oaT_d = nc.dram_tensor("offawT", [384, TPC], BF16,
                           kind="ExternalOutput").ap()

    with tile.TileContext(nc) as tc, ExitStack() as ctx:
        sb = ctx.enter_context(tc.tile_pool(name="sb", bufs=1))
        ps = ctx.enter_context(tc.tile_pool(name="ps", bufs=1, space="PSUM"))
        ob = ctx.enter_context(tc.tile_pool(name="ob", bufs=1))

        wmms = _warmup(nc, sb, ps, "p", bufs=3)

        wpk = sb.tile([128, 1280], BF16, tag="wpk")
        nc.sync.dma_start(wpk[:], wpk_d[:])
        wv = wpk[:, 0:512].rearrange("p (c n) -> p c n", c=2)
        woa = wpk[:, 512:1280].rearrange("p (c n) -> p c n", c=2)
        prm = sb.tile([128, 5], F32, tag="prm")
        nc.gpsimd.dma_start(prm[:], prm_d[:])

        chunks = _tchunks(512)
        xcs, qcs = [], []
        # chunk-0 input DMAs up front; chunk 1 staged on mid-warmup so
        # chunk 0 has the HBM bandwidth to itself first
        for ci in range(2):
            xc = ob.tile([128, 2, 512], BF16, tag="xc", bufs=2)
            qc = ob.tile([128, 2, 512], BF16, tag="qc", bufs=2)
            t0, tsz = chunks[ci]
            d1 = nc.sync.dma_start(xc[:, :, :tsz],
                                   _ccn(xT_d)[:, :, t0:t0 + tsz])
            nc.sync.dma_start(qc[:, :, :tsz], _ccn(qT_d)[:, :, t0:t0 + tsz])
            if ci == 1:
                add_dep_helper(d1.ins, wmms[6].ins, sync=True,
                               reason="stage chunk-1 dma")
            xcs.append(xc)
            qcs.append(qc)

        for ci, (t0, tsz) in enumerate(chunks):
            xc, qc = xcs[ci], qcs[ci]
            vsb = ob.tile([128, 2, 512], BF16, tag="vsb", bufs=2)
            osb = ob.tile([128, 3, 512], BF16, tag="osb", bufs=2)
            first_mm = None
            for m in range(5):  # 0-1: val (from x), 2-4: offaw (from q)
                src = xc if m < 2 else qc
                w = wv if m < 2 else woa
                mm = m if m < 2 else m - 2
                p = ps.tile([128, 512], F32, tag="p", bufs=3)
                for k in range(2):
                    mi = nc.tensor.matmul(
                        p[:, :tsz], w[:, k, mm * 128:mm * 128 + 128],
                        src[:, k, :tsz], start=(k == 0), stop=(k == 1))
                    if first_mm is None:
                        first_mm = mi
                dst = (vsb if m < 2 else osb)[:, mm, :tsz]
                if m % 2 == 0:
                    nc.scalar.activation(dst, p[:, :tsz], AF.Identity,
                                         bias=prm[:, m:m + 1])
                else:
                    nc.vector.tensor_scalar(dst, p[:, :tsz], prm[:, m:m + 1],
                                            None, ALU.add)
            if ci + 2 < len(chunks):
                t1, tsz1 = chunks[ci + 2]
                xn = ob.tile([128, 2, 512], BF16, tag="xc", bufs=2)
                qn = ob.tile([128, 2, 512], BF16, tag="qc", bufs=2)
                d1 = nc.sync.dma_start(xn[:, :, :tsz1],
                                       _ccn(xT_d)[:, :, t1:t1 + tsz1])
                nc.sync.dma_start(qn[:, :, :tsz1],
                                  _ccn(qT_d)[:, :, t1:t1 + tsz1])
                add_dep_helper(d1.ins, first_mm.ins, sync=True,
                               reason="defer chunk dma")
                xcs.append(xn)
                qcs.append(qn)
            nc.scalar.dma_start(_ccn(valT_d)[:, :, t0:t0 + tsz],
                                vsb[:, :, :tsz])
            nc.sync.dma_start(
                oaT_d.rearrange("(c p) n -> p c n", p=128)[:, :, t0:t0 + tsz],
                osb[:, :, :tsz])
    nc.compile()
    return nc


def _build_BCDA(with_A):
    """Fused out-proj + LN1 + FFN (+ LN2 + next-layer projections).

    with_A=True (layer 0): outputs x1n (pre-affine LN2), valT, offawT.
    with_A=False (layer 1): outputs out = r2 (pre-LN2 residual); the host
    applies LN2 + g2/be2.
    """
    nc = _nc()
    aT_d = nc.dram_tensor("attnT", [D_MODEL, TPC], BF16,
                          kind="ExternalInput").ap()
    xbT_d = nc.dram_tensor("xbT", [D_MODEL, TPC], BF16,
                           kind="ExternalInput").ap()
    # wpk1: wo (2x256) | ident (128)
    wpk1_d = nc.dram_tensor("wpk1", [128, 640], BF16,
                            kind="ExternalInput").ap()
    # wpk2: wl1 (2x1024) | dg1 (2x256) | wl2 (8x256)
    wpk2_d = nc.dram_tensor("wpk2", [128, 4608], BF16,
                            kind="ExternalInput").ap()
    rows_d = nc.dram_tensor("rows", [1, 256], BF16, kind="ExternalInput").ap()
    prm_d = nc.dram_tensor("prm", [128, 18], F32, kind="ExternalInput").ap()
    if with_A:
        posT_d = nc.dram_tensor("posT", [D_MODEL, TPC], BF16,
                                kind="ExternalInput").ap()
        # wpk3: wv (2x256) | woa (2x384)
        wpk3_d = nc.dram_tensor("wpk3", [128, 1280], BF16,
                                kind="ExternalInput").ap()
        x1n_d = nc.dram_tensor("x1n", [128, NT * 256], BF16,
                               kind="ExternalOutput").ap()
        valT_d = nc.dram_tensor("valT", [256, TPC], BF16,
                                kind="ExternalOutput").ap()
        oaT_d = nc.dram_tensor("offawT", [384, TPC], BF16,
                               kind="ExternalOutput").ap()
    else:
        out_d = nc.dram_tensor("out", [128, NT * 256], BF16,
                               kind="ExternalOutput").ap()

    with tile.TileContext(nc) as tc, ExitStack() as ctx:
        sb = ctx.enter_context(tc.tile_pool(name="sb", bufs=1))
        ps = ctx.enter_context(tc.tile_pool(name="ps", bufs=1, space="PSUM"))
        ob = ctx.enter_context(tc.tile_pool(name="ob", bufs=1))

        wmms = _warmup(nc, sb, ps, "pb")

        # ---- input DMAs: critical ones up front, bulk deferred ----
        wpk1 = sb.tile([128, 640], BF16, tag="wpk1")
        nc.sync.dma_start(wpk1[:], wpk1_d[:])
        wo = wpk1[:, 0:512].rearrange("p (c n) -> p c n", c=2)
        idn = wpk1[:, 512:640]
        aT = sb.tile([128, 2, TPC], BF16, tag="aT")
        nc.sync.dma_start(aT[:, :, 0:256], _ccn(aT_d)[:, :, 0:256])
        d = nc.sync.dma_start(aT[:, :, 256:512], _ccn(aT_d)[:, :, 256:512])
        add_dep_helper(d.ins, wmms[2].ins, sync=True, reason="stage aT0b")
        d = nc.sync.dma_start(aT[:, :, 512:TPC], _ccn(aT_d)[:, :, 512:TPC])
        add_dep_helper(d.ins, wmms[6].ins, sync=True, reason="stage aT1")
        xbT = sb.tile([128, 2, TPC], BF16, tag="xbT")
        nc.scalar.dma_start(xbT[:, :, 0:256], _ccn(xbT_d)[:, :, 0:256])
        d = nc.scalar.dma_start(xbT[:, :, 256:512], _ccn(xbT_d)[:, :, 256:512])
        add_dep_helper(d.ins, wmms[2].ins, sync=True, reason="stage xbT0b")
        d = nc.scalar.dma_start(xbT[:, :, 512:TPC], _ccn(xbT_d)[:, :, 512:TPC])
        add_dep_helper(d.ins, wmms[6].ins, sync=True, reason="stage xbT1")
        # force the sqrt ACT table set resident before the LN stats chain
        sqd = sb.tile([128, 1], F32, tag="sqd")
        nc.gpsimd.memset(sqd[:], 1.0)
        nc.scalar.activation(sqd[:, 0:1], sqd[:, 0:1], AF.Sqrt)
        # wpk2/posT/wpk3 tiles declared now, DMAs emitted inside sweep 1
        # gated on PE progress so they don't steal HBM bandwidth from aT/xbT
        wpk2 = sb.tile([128, 4608], BF16, tag="wpk2")
        wl1 = wpk2[:, 0:2048].rearrange("p (c n) -> p c n", c=2)
        dg1 = wpk2[:, 2048:2560].rearrange("p (c n) -> p c n", c=2)
        wl2 = wpk2[:, 2560:4608].rearrange("p (c n) -> p c n", c=8)
        prm = sb.tile([128, 18], F32, tag="prm")
        nc.gpsimd.dma_start(prm[:], prm_d[:])
        rows = sb.tile([1, 256], BF16, tag="rows")
        nc.gpsimd.dma_start(rows[:], rows_d[:])
        if with_A:
            posT = sb.tile([128, 2, TPC], BF16, tag="posT")
            wpk3 = sb.tile([128, 1280], BF16, tag="wpk3")
            wv = wpk3[:, 0:512].rearrange("p (c n) -> p c n", c=2)
            woa = wpk3[:, 512:1280].rearrange("p (c n) -> p c n", c=2)
            valTs = sb.tile([128, 2, TPC], BF16, tag="valTs")
            oaTs = sb.tile([128, 3, TPC], BF16, tag="oaTs")
            q1Ts = sb.tile([128, 2, TPC], BF16, tag="q1Ts")
        ones = sb.tile([1, 512], BF16, tag="ones")
        nc.gpsimd.memset(ones[:], 1.0)

        # persistent intermediates
        r1a = sb.tile([128, NT, 256], BF16, tag="r1a")
        xnTa = sb.tile([128, 2, TPC], BF16, tag="xnTa")
        hta = sb.tile([128, 8, TPC], BF16, tag="hta")
        xna = sb.tile([128, NT, 256], BF16, tag="xna")
        bst1 = sb.tile([128, NT, 6], F32, tag="bst1")
        mv1 = sb.tile([128, NT, 2], F32, tag="mv1")
        st1r = sb.tile([128, NT, 1], F32, tag="st1r", name="st1r")
        st1n = sb.tile([128, NT, 1], F32, tag="st1n", name="st1n")
        if with_A:
            r2a = sb.tile([128, NT, 256], BF16, tag="r2a")
            xn2Ta = sb.tile([128, 2, TPC], BF16, tag="xn2Ta")
            xout = sb.tile([128, NT, 256], BF16, tag="xout")
            bst2 = sb.tile([128, NT, 6], F32, tag="bst2")
            mv2 = sb.tile([128, NT, 2], F32, tag="mv2")
            st2r = sb.tile([128, NT, 1], F32, tag="st2r", name="st2r")
            st2n = sb.tile([128, NT, 1], F32, tag="st2n", name="st2n")
        else:
            r2b = sb.tile([128, NT, 256], BF16, tag="r2b")

        def stats_chain(mv, str_, stn, h0, h1, eng):
            """mv[:, h0:h1] = (mean, var) -> str_=rstd, stn=-mean*rstd."""
            sd = ob.tile([128, NT, 1], F32, tag="sd", bufs=2)
            nc.scalar.activation(sd[:, h0:h1, :], mv[:, h0:h1, 1:2], AF.Sqrt,
                                 bias=prm[:, 0:1])
            nc.vector.reciprocal(str_[:, h0:h1, :], sd[:, h0:h1, :])
            eng.scalar_tensor_tensor(stn[:, h0:h1, :], mv[:, h0:h1, 0:1],
                                     -1.0, str_[:, h0:h1, :],
                                     op0=ALU.mult, op1=ALU.mult)

        # ---- sweep 1: B matmul + residual, paired drains + LN1 stats ----
        for pi, (t0i, np_) in enumerate(PAIRS):
            pbp = ps.tile([128, 2, 256], F32, tag="pb", bufs=2)
            first_mm = None
            for j in range(np_):
                ti = t0i + j
                sz = _tsz(ti)
                t0 = ti * 128
                for k in range(2):
                    mi = nc.tensor.matmul(pbp[:sz, j, :],
                                          aT[:, k, t0:t0 + sz],
                                          wo[:, k, :], start=(k == 0),
                                          stop=False)
                    if first_mm is None:
                        first_mm = mi
                for k in range(2):
                    nc.tensor.matmul(pbp[:sz, j, k * 128:k * 128 + 128],
                                     xbT[:, k, t0:t0 + sz], idn[:, :],
                                     start=False, stop=(k == 1),
                                     skip_group_check=True)
            if pi == 0:
                d = nc.sync.dma_start(wpk2[:], wpk2_d[:])
                add_dep_helper(d.ins, first_mm.ins, sync=True,
                               reason="defer wpk2 dma")
            elif pi == 2 and with_A:
                d = nc.sync.dma_start(posT[:], _ccn(posT_d))
                nc.sync.dma_start(wpk3[:], wpk3_d[:])
                add_dep_helper(d.ins, first_mm.ins, sync=True,
                               reason="defer posT/wpk3 dma")
            sz0 = _tsz(t0i + np_ - 1)
            if np_ == 2:
                if pi % 2 == 0:
                    nc.scalar.copy(r1a[:, t0i:t0i + 2, :], pbp[:, :, :])
                else:
                    nc.vector.tensor_copy(r1a[:, t0i:t0i + 2, :], pbp[:, :, :])
            else:
                nc.vector.tensor_copy(r1a[:sz0, t0i, :], pbp[:sz0, 0, :])
            for j in range(np_):
                ti = t0i + j
                sz = _tsz(ti)
                nc.vector.bn_stats(bst1[:sz, ti, :], r1a[:sz, ti, :])
                nc.vector.bn_aggr(mv1[:sz, ti, :], bst1[:sz, ti, :])
            stats_chain(mv1, st1r, st1n, t0i, t0i + np_, nc.vector)

        # ---- sweep 2: LN1 apply, transpose, C, D (+ LN2 stats) ----
        for gi, (g0, gsz, tis) in enumerate(GROUPS):
            for t0i, np_ in PAIRS:
                if t0i not in tis:
                    continue
                pt = ps.tile([128, 2, 2, 128], BF16, tag="ptr", bufs=2)
                for j in range(np_):
                    ti = t0i + j
                    sz = _tsz(ti)
                    nc.scalar.activation(xna[:sz, ti, :], r1a[:sz, ti, :],
                                         AF.Identity,
                                         bias=st1n[:sz, ti, :],
                                         scale=st1r[:sz, ti, :])
                    for c in range(2):
                        nc.tensor.transpose(
                            pt[:, j, c, :sz],
                            xna[:sz, ti, c * 128:c * 128 + 128],
                            idn[:sz, :sz])
                t0 = t0i * 128
                tw = sum(_tsz(t0i + j) for j in range(np_))
                if np_ == 2:
                    src = pt.transpose((0, 2, 1, 3))
                    dst = xnTa[:, :, t0:t0 + 256].rearrange(
                        "p c (a b) -> p c a b", a=2)
                    if t0i % 4 == 0:
                        nc.vector.tensor_copy(dst, src)
                    else:
                        nc.scalar.copy(dst, src)
                else:
                    nc.vector.tensor_copy(xnTa[:, :, t0:t0 + tw],
                                          pt[:, 0, :, :tw])
            # C over the whole group: hT = relu(Wl1g.T @ xnT + bl1row)
            for m in range(8):
                pc = ps.tile([128, 512], F32, tag="pca", bufs=2)
                for k in range(2):
                    nc.tensor.matmul(pc[:, :gsz],
                                     wl1[:, k, m * 128:m * 128 + 128],
                                     xnTa[:, k, g0:g0 + gsz],
                                     start=(k == 0), stop=(k == 1))
                if m % 2 == 0:
                    nc.scalar.activation(hta[:, m, g0:g0 + gsz], pc[:, :gsz],
                                         AF.Relu, bias=prm[:, 5 + m:6 + m])
                else:
                    nc.vector.tensor_scalar(hta[:, m, g0:g0 + gsz],
                                            pc[:, :gsz], prm[:, 5 + m:6 + m],
                                            0.0, ALU.add, ALU.max)
            # D, paired into 2-slot PSUM tiles
            for t0i, np_ in PAIRS:
                if t0i not in tis:
                    continue
                pdp = ps.tile([128, 2, 256], F32, tag="pd", bufs=2)
                for j in range(np_):
                    ti = t0i + j
                    sz = _tsz(ti)
                    t0 = ti * 128
                    for k in range(8):
                        nc.tensor.matmul(pdp[:sz, j, :], hta[:, k, t0:t0 + sz],
                                         wl2[:, k, :],
                                         start=(k == 0), stop=False)
                    for k in range(2):
                        nc.tensor.matmul(pdp[:sz, j, :],
                                         xnTa[:, k, t0:t0 + sz],
                                         dg1[:, k, :], start=False, stop=False)
                    nc.tensor.matmul(pdp[:sz, j, :], ones[0:1, :sz],
                                     rows[:, :], start=False, stop=True)
                sz0 = _tsz(t0i + np_ - 1)
                if with_A:
                    if np_ == 2:
                        nc.vector.tensor_copy(r2a[:, t0i:t0i + 2, :],
                                              pdp[:, :, :])
                    else:
                        nc.vector.tensor_copy(r2a[:sz0, t0i, :],
                                              pdp[:sz0, 0, :])
                    for j in range(np_):
                        ti = t0i + j
                        sz = _tsz(ti)
                        nc.vector.bn_stats(bst2[:sz, ti, :], r2a[:sz, ti, :])
                        nc.vector.bn_aggr(mv2[:sz, ti, :], bst2[:sz, ti, :])
                    stats_chain(mv2, st2r, st2n, t0i, t0i + np_, nc.vector)
                else:
                    if np_ == 2:
                        if t0i % 4 == 0:
                            nc.vector.tensor_copy(r2b[:, t0i:t0i + 2, :],
                                                  pdp[:, :, :])
                        else:
                            nc.scalar.copy(r2b[:, t0i:t0i + 2, :],
                                           pdp[:, :, :])
                        dma_eng = nc.sync if t0i % 4 == 0 else nc.scalar
                        dma_eng.dma_start(
                            out_d[:, t0i * 256:(t0i + 2) * 256],
                            r2b[:, t0i:t0i + 2, :])
                    else:
                        nc.vector.tensor_copy(r2b[:sz0, t0i, :],
                                              pdp[:sz0, 0, :])
                        nc.sync.dma_start(
                            out_d[:, t0i * 256:(t0i + 1) * 256],
                            r2b[:, t0i, :])


        # ---- sweep 3 (with_A): LN2 apply + next-layer projections ----
        if with_A:
            for g0, gsz, tis in GROUPS:
                for t0i, np_ in PAIRS:
                    if t0i not in tis:
                        continue
                    pt2 = ps.tile([128, 2, 2, 128], BF16, tag="ptr", bufs=2)
                    for j in range(np_):
                        ti = t0i + j
                        sz = _tsz(ti)
                        nc.scalar.activation(xout[:sz, ti, :], r2a[:sz, ti, :],
                                             AF.Identity,
                                             bias=st2n[:sz, ti, :],
                                             scale=st2r[:sz, ti, :])
                        for c in range(2):
                            nc.tensor.transpose(
                                pt2[:, j, c, :sz],
                                xout[:sz, ti, c * 128:c * 128 + 128],
                                idn[:sz, :sz])
                    t0 = t0i * 128
                    tw = sum(_tsz(t0i + j) for j in range(np_))
                    if np_ == 2:
                        src = pt2.transpose((0, 2, 1, 3))
                        dst = xn2Ta[:, :, t0:t0 + 256].rearrange(
                            "p c (a b) -> p c a b", a=2)
                        nc.scalar.copy(dst, src)
                        # q1T = g2*xn2T + (be2+pos)T, fused into the drain
                        for c in range(2):
                            nc.vector.scalar_tensor_tensor(
                                q1Ts[:, c, t0:t0 + 256].rearrange(
                                    "p (a b) -> p a b", a=2),
                                pt2[:, :, c, :], prm[:, 1 + c:2 + c],
                                posT[:, c, t0:t0 + 256].rearrange(
                                    "p (a b) -> p a b", a=2),
                                op0=ALU.mult, op1=ALU.add)
                    else:
                        nc.scalar.copy(xn2Ta[:, :, t0:t0 + tw],
                                       pt2[:, 0, :, :tw])
                        for c in range(2):
                            nc.vector.scalar_tensor_tensor(
                                q1Ts[:, c, t0:t0 + tw],
                                pt2[:, 0, c, :tw], prm[:, 1 + c:2 + c],
                                posT[:, c, t0:t0 + tw],
                                op0=ALU.mult, op1=ALU.add)
                # val projections first (no q dependency)
                for m in range(2):
                    pa = ps.tile([128, 512], F32, tag="pca", bufs=2)
                    for k in range(2):
                        nc.tensor.matmul(pa[:, :gsz],
                                         wv[:, k, m * 128:m * 128 + 128],
                                         xn2Ta[:, k, g0:g0 + gsz],
                                         start=(k == 0), stop=(k == 1))
                    dst = valTs[:, m, g0:g0 + gsz]
                    if m % 2 == 0:
                        nc.scalar.activation(dst, pa[:, :gsz], AF.Identity,
                                             bias=prm[:, 13 + m:14 + m])
                    else:
                        nc.vector.tensor_scalar(dst, pa[:, :gsz],
                                                prm[:, 13 + m:14 + m],
                                                None, ALU.add)
                for m in range(3):
                    pa = ps.tile([128, 512], F32, tag="pca", bufs=2)
                    for k in range(2):
                        nc.tensor.matmul(pa[:, :gsz],
                                         woa[:, k, m * 128:m * 128 + 128],
                                         q1Ts[:, k, g0:g0 + gsz],
                                         start=(k == 0), stop=(k == 1))
                    dst = oaTs[:, m, g0:g0 + gsz]
                    if m % 2 == 1:
                        nc.scalar.activation(dst, pa[:, :gsz], AF.Identity,
                                             bias=prm[:, 15 + m:16 + m])
                    else:
                        nc.vector.tensor_scalar(dst, pa[:, :gsz],
                                                prm[:, 15 + m:16 + m],
                                                None, ALU.add)
                # output DMAs per group
                lo, hi = tis[0], tis[-1] + 1
                nc.scalar.dma_start(_ccn(valT_d)[:, :, g0:g0 + gsz],
                                    valTs[:, :, g0:g0 + gsz])
                nc.sync.dma_start(
                    oaT_d.rearrange("(c p) n -> p c n", p=128)[:, :,
                                                              g0:g0 + gsz],
                    oaTs[:, :, g0:g0 + gsz])
                nc.gpsimd.dma_start(
                    x1n_d[:, lo * 256:hi * 256], xout[:, lo:hi, :])
    nc.compile()
    return nc


def _run(prog, in_maps):
    trace = bool(os.environ.get("BASS_TRACE"))
    res = run_bass_kernel_spmd(prog, in_maps, core_ids=list(range(NCORE)),
                               trace=trace)
    if res.exec_time_ns:
        HW_EXEC_NS.append(res.exec_time_ns)
    if trace:
        LAST_RES.append(res)
    return res.results


def _bf(a):
    return np.ascontiguousarray(np.asarray(a, np.float32).astype(NPBF))


def _ccn_host(w):
    """[K, M] -> [128, K//128 * M] channel-major pack block."""
    w = np.asarray(w, np.float32)
    k, m = w.shape
    return w.reshape(k // 128, 128, m).transpose(1, 0, 2).reshape(128, -1)


def _chunked(v, nch):
    v = np.asarray(v, np.float32)
    return np.ascontiguousarray(v.reshape(nch, 128).T.astype(np.float32))


def _ref_points(valid_ratios):
    refs = []
    for lvl, (H, W) in enumerate(SHAPES):
        gy, gx = np.meshgrid(np.arange(H, dtype=np.float32) + 0.5,
                             np.arange(W, dtype=np.float32) + 0.5,
                             indexing="ij")
        ry = gy.reshape(-1)[None] / (valid_ratios[:, lvl, 1][:, None] * H)
        rx = gx.reshape(-1)[None] / (valid_ratios[:, lvl, 0][:, None] * W)
        refs.append(np.stack([rx, ry], -1))
    ref = np.concatenate(refs, 1)
    return ref[:, :, None, :] * valid_ratios[:, None]


def _host_sample(value, off, aw, ref_pts):
    N, Lq = off.shape[:2]
    off = off.reshape(N, Lq, N_HEADS, N_LEVELS, N_POINTS, 2)
    aw = aw.reshape(N, Lq, N_HEADS, N_LEVELS, N_POINTS)
    normalizer = np.array([[w, h] for h, w in SHAPES], np.float32)
    loc = (ref_pts[:, :, None, :, None, :]
           + off / normalizer[None, None, None, :, None, :])
    acc = np.zeros((N, N_HEADS, Lq, HEAD_DIM), np.float32)
    for lvl, (H, W) in enumerate(SHAPES):
        s = LEVEL_STARTS[lvl]
        val = value[:, s:s + H * W].transpose(0, 2, 1, 3)
        x = loc[:, :, :, lvl, :, 0] * W - 0.5
        y = loc[:, :, :, lvl, :, 1] * H - 0.5
        x0 = np.floor(x)
        y0 = np.floor(y)
        wx1 = x - x0
        wy1 = y - y0
        ix0 = x0.astype(np.int64)
        iy0 = y0.astype(np.int64)

        def corner(ix, iy, w):
            valid = (ix >= 0) & (ix < W) & (iy >= 0) & (iy < H)
            idx = np.clip(iy, 0, H - 1) * W + np.clip(ix, 0, W - 1)
            idx = idx.transpose(0, 2, 1, 3).reshape(N, N_HEADS, Lq * N_POINTS)
            g = np.take_along_axis(val, idx[..., None], axis=2)
            g = g.reshape(N, N_HEADS, Lq, N_POINTS, HEAD_DIM)
            w = np.where(valid, w, 0.0).transpose(0, 2, 1, 3)
            return g * w[..., None].astype(np.float32)

        sampled = (corner(ix0, iy0, (1 - wx1) * (1 - wy1))
                   + corner(ix0 + 1, iy0, wx1 * (1 - wy1))
                   + corner(ix0, iy0 + 1, (1 - wx1) * wy1)
                   + corner(ix0 + 1, iy0 + 1, wx1 * wy1))
        acc += (sampled * aw[:, :, :, lvl].transpose(0, 2, 1, 3)[..., None]
                ).sum(3)
    return acc.transpose(0, 2, 1, 3).reshape(N, Lq, D_MODEL)


def _shardT(fullT):
    return [np.ascontiguousarray(fullT[c // 4, :, (c % 4) * TPC:
                                       (c % 4 + 1) * TPC])
            for c in range(NCORE)]


def _unshardT(parts):
    F = parts[0].shape[0]
    out = np.empty((BATCH, LEN_IN, F), np.float32)
    for c in range(NCORE):
        out[c // 4, (c % 4) * TPC:(c % 4 + 1) * TPC] = \
            np.asarray(parts[c], np.float32).T
    return out


def _unshard_pm(parts):  # partition-major parts [128, NT*256]
    out = np.empty((BATCH, LEN_IN, 256), np.float32)
    for c in range(NCORE):
        a = np.asarray(parts[c], np.float32).reshape(128, NT, 256)
        a = a.transpose(1, 0, 2).reshape(NT * 128, 256)[:TPC]
        out[c // 4, (c % 4) * TPC:(c % 4 + 1) * TPC] = a
    return out


_IDENT = np.eye(128, dtype=np.float32)


def kernel(src, pos, valid_ratios, Wv, bv, Woff, boff, Wa, ba, Wo, bo,
           g1, be1, Wl1, bl1, Wl2, bl2, g2, be2):
    src = np.asarray(src, np.float32)
    pos = np.asarray(pos, np.float32)
    valid_ratios = np.asarray(valid_ratios, np.float32)
    asf = lambda a: np.asarray(a, np.float32)
    HW_EXEC_NS.clear()
    LAST_RES.clear()

    if "A" not in _PROGS:
        _PROGS["A"] = _build_A()
        _PROGS["BCDA"] = _build_BCDA(with_A=True)
        _PROGS["BCD"] = _build_BCDA(with_A=False)

    ref_pts = _ref_points(valid_ratios)

    Woa = [np.concatenate([asf(Woff[l]), asf(Wa[l])], axis=1)
           for l in range(2)]
    bva = [np.concatenate([asf(bv[l]), asf(boff[l]), asf(ba[l])])
           for l in range(2)]
    Wl1g = [asf(g1[l])[:, None] * asf(Wl1[l]) for l in range(2)]
    bl1f = [asf(bl1[l]) + asf(be1[l]) @ asf(Wl1[l]) for l in range(2)]
    # layer-1 value-proj with layer-0 g2/be2 folded in (q-path keeps
    # plain Woa; q is built on device as g2*xn2 + be2 + pos)
    Wv1f = asf(g2[0])[:, None] * asf(Wv[1])
    bva1f = np.concatenate([asf(bv[1]) + asf(be2[0]) @ asf(Wv[1]),
                            bva[1][256:]])
    cr = [asf(be1[l]) + asf(bl2[l]) for l in range(2)]
    rows = [np.ascontiguousarray(cr[l][None, :].astype(NPBF))
            for l in range(2)]
    prm = [np.concatenate([np.full((128, 1), 1e-5, np.float32),
                           _chunked(g2[l], 2), _chunked(be2[l], 2),
                           _chunked(bl1f[l], 8),
                           _chunked(bva1f if l == 0 else np.zeros(640), 5)],
                          axis=1) for l in range(2)]
    dg1 = [np.diag(asf(g1[l])) for l in range(2)]

    # packed weight blobs
    wpkA = np.concatenate([_ccn_host(Wv[0]), _ccn_host(Woa[0])],
                          axis=1).astype(NPBF)
    wpk1 = [np.concatenate([_ccn_host(Wo[l]), _IDENT],
                           axis=1).astype(NPBF) for l in range(2)]
    wpk2 = [np.concatenate([_ccn_host(Wl1g[l]), _ccn_host(dg1[l]),
                            _ccn_host(Wl2[l])], axis=1).astype(NPBF)
            for l in range(2)]
    wpk3 = np.concatenate([_ccn_host(Wv1f), _ccn_host(Woa[1])],
                          axis=1).astype(NPBF)

    xT = np.ascontiguousarray(src.transpose(0, 2, 1))
    qT = np.ascontiguousarray((src + pos).transpose(0, 2, 1))
    # q for layer-1 projections is g2*xn2 + (be2 + pos); fold be2 into pos
    posbT = np.ascontiguousarray(
        (pos + asf(be2[0])[None, None, :]).transpose(0, 2, 1))
    xTs = _shardT(xT.astype(NPBF))
    qTs = _shardT(qT.astype(NPBF))
    posTs = _shardT(posbT.astype(NPBF))

    # ---- launch 1: layer-0 projections ----
    in_maps = [{
        "xT": xTs[c], "qT": qTs[c],
        "wpk": wpkA, "prm": _chunked(bva[0], 5),
    } for c in range(NCORE)]
    resA = _run(_PROGS["A"], in_maps)

    def gather_attn(value, offaw, layer, x_full):
        aw = offaw[:, :, 256:].reshape(BATCH, LEN_IN, N_HEADS, 16)
        aw = aw - aw.max(-1, keepdims=True)
        e = np.exp(aw)
        aw = (e / e.sum(-1, keepdims=True)).reshape(BATCH, LEN_IN, 128)
        attn = _host_sample(value.reshape(BATCH, LEN_IN, N_HEADS, HEAD_DIM),
                            offaw[:, :, :256], aw, ref_pts)
        attnT = np.ascontiguousarray(attn.transpose(0, 2, 1))
        xbf = (x_full + asf(bo[layer])[None, None, :]).transpose(0, 2, 1)
        return (_shardT(attnT.astype(NPBF)),
                _shardT(np.ascontiguousarray(xbf).astype(NPBF)))

    # ---- launch 2: layer-0 BCD + layer-1 projections ----
    value = _unshardT([resA[c]["valT"] for c in range(NCORE)])
    offaw = _unshardT([resA[c]["offawT"] for c in range(NCORE)])
    attnTs, xbs = gather_attn(value, offaw, 0, src)
    in_maps = [{
        "attnT": attnTs[c], "xbT": xbs[c],
        "wpk1": wpk1[0], "wpk2": wpk2[0], "wpk3": wpk3,
        "rows": rows[0], "prm": prm[0], "posT": posTs[c],
    } for c in range(NCORE)]
    resB = _run(_PROGS["BCDA"], in_maps)

    # x1 = g2*xn2 + be2 (host applies the folded affine)
    xn2 = _unshard_pm([resB[c]["x1n"] for c in range(NCORE)])
    x1 = xn2 * asf(g2[0])[None, None, :] + asf(be2[0])[None, None, :]

    # ---- launch 3: layer-1 BCD -> final ----
    val1 = _unshardT([resB[c]["valT"] for c in range(NCORE)])
    oa1 = _unshardT([resB[c]["offawT"] for c in range(NCORE)])
    attnTs, xbs = gather_attn(val1, oa1, 1, x1)
    in_maps = [{
        "attnT": attnTs[c], "xbT": xbs[c],
        "wpk1": wpk1[1], "wpk2": wpk2[1],
        "rows": rows[1], "prm": prm[1],
    } for c in range(NCORE)]
    resC = _run(_PROGS["BCD"], in_maps)

    # host LN2 + affine for the final layer
    r2 = _unshard_pm([resC[c]["out"] for c in range(NCORE)])
    m = r2.mean(-1, keepdims=True)
    v = np.square(r2 - m).mean(-1, keepdims=True)
    xn = (r2 - m) / np.sqrt(v + 1e-5)
    return (xn * asf(g2[1])[None, None, :]
            + asf(be2[1])[None, None, :]).astype(np.float32)
